# revision 1
# baseline (speedup 1.0000x reference)
"""DepthLSSTransform Trainium kernel: 3 SPMD launches over 8 NeuronCores.

Launch A: per-camera conv pipeline (dtransform + depthnet + softmax) on
          24-row bands (one 16-row + one 8-row segment per core).
Launch B: bev_pool segment-sum via one-hot matmuls over a host-built
          virtual-window schedule (sorted-by-voxel points).
Launch C: BEV downsample convs, spatially sharded.
Host: geometry/voxel indices, scheduling, gathers, folds (orchestration).
"""
import numpy as np
import ml_dtypes

import concourse.bass as bass
import concourse.tile as tile
from concourse import bacc, mybir
from concourse.bass_utils import run_bass_kernel_spmd

dt = mybir.dt
bf16 = ml_dtypes.bfloat16

# ---- problem constants (hardcoded per contract) ----
B, N = 1, 6
CIN, CIMG, DD = 256, 80, 59
FH, FW, IH, IW = 32, 88, 256, 704
XY0, DXY, NX = -54.0, 0.3, 360
Z0, DZ, NZ = -10.0, 20.0, 1
NPTS = N * DD * FH * FW
NPIX = N * FH * FW
NCORES = 8
QV = 4                      # chunks of 128 points per virtual window

# per-core segments: (camera, h0) for seg A (16 rows) and seg B (8 rows)
SEG_A = [(0, 0), (1, 0), (1, 16), (2, 16), (3, 0), (4, 0), (4, 16), (5, 16)]
SEG_B = [(0, 16), (0, 24), (2, 0), (2, 8), (3, 16), (3, 24), (5, 0), (5, 8)]
# band pixel ranges in global row order (row = n*32 + h)
ROWS_OF_CORE = [[(SEG_A[c][0] * FH + SEG_A[c][1] + r) for r in range(16)] +
                [(SEG_B[c][0] * FH + SEG_B[c][1] + r) for r in range(8)]
                for c in range(NCORES)]

# segment geometry: rows16 segment: d rows [8h0-34, 8h0+158) (192), dt2 out
# rows [2h0-8, 2h0+39) (47), dt3 [h0-3, h0+19) (22), dn1 [h0-1, h0+17) (18)
SEGS = [dict(nout=16, nd=192, nq=48, nt2=47, nt3=22, nn1=18),
        dict(nout=8, nd=128, nq=32, nt2=31, nt3=14, nn1=10)]


def _seg_ranges(h0, S):
    return dict(d0=8 * h0 - 34, q0=2 * h0 - 8, t0=h0 - 3, r0=h0 - 1, o0=h0)


# ---------------------------------------------------------------- launch A
def build_launch_a(debug=False, psum_bufs=3, work_bufs=3, stages=9):
    nc = bacc.Bacc("TRN2", target_bir_lowering=False, debug=False,
                   num_devices=NCORES)
    AP = {}

    def inp(name, shape, dtype=dt.bfloat16):
        AP[name] = nc.dram_tensor(name, shape, dtype, kind="ExternalInput").ap()
        return AP[name]

    # per segment inputs (s = 0: 16-row, 1: 8-row)
    for s, S in enumerate(SEGS):
        inp(f"dph{s}", [128, S["nq"], 177])
        inp(f"masks{s}", [128, S["nq"] + S["nt2"] + S["nt3"] + S["nn1"]])
        inp(f"xseg{s}", [CIN, S["nt3"], FW])            # x_img slice (zeroed oob)
    # packed f32 constants: [alpha, beta, s_dt2, t_dt2, s_dt3, t_dt3,
    #  s_dn1(2), t_dn1(2), s_dn2(2), t_dn2(2), b_dn3(139)] -> [128, 153]
    inp("consts", [128, 153], dt.float32)
    # conv weights (host-prepped layouts)
    inp("w_dt2", [4, 128, 32])                          # groups (dky,dmx)
    inp("w_dt3", [9, 128, 64])
    inp("w_dn1", [9, 3, 128, 256])                      # tap, icchunk(128,128,64pad) -> 256
    inp("w_dn2", [9, 2, 128, 256])
    inp("w_dn3", [2, 128, 139])

    DBG = {}
    dbg_specs = [] if not debug else [("dbg_t1", [128, SEGS[0]["nq"], 177], dt.bfloat16),
                        ("dbg_dt2o", [32, SEGS[0]["nt2"] + 1, 180], dt.bfloat16),
                        ("dbg_dtc", [64, SEGS[0]["nt3"], 92], dt.bfloat16),
                        ("dbg_n1o", [128, SEGS[0]["nn1"], 92], dt.bfloat16),
                        ("dbg_n2o", [128, SEGS[0]["nout"], 88], dt.bfloat16)]
    for nm, sh, dty in dbg_specs:
        DBG[nm] = nc.dram_tensor(nm, sh, dty, kind="ExternalOutput").ap()
    out_depth = nc.dram_tensor("out_depth", [24 * FW, DD], dt.float32,
                               kind="ExternalOutput").ap()
    out_feat = nc.dram_tensor("out_feat", [24 * FW, CIMG], dt.bfloat16,
                              kind="ExternalOutput").ap()

    # HBM scratch
    scr = {}
    for s, S in enumerate(SEGS):
        scr[f"dt2o{s}"] = nc.dram_tensor(f"dt2o{s}", [32, S["nt2"] + 1, 2, 90], dt.bfloat16).ap()

    RELU = mybir.ActivationFunctionType.Relu
    with tile.TileContext(nc) as tc:
        with tc.tile_pool(name="const", bufs=1) as cpool, \
             tc.tile_pool(name="work", bufs=work_bufs) as wpool, \
             tc.tile_pool(name="big", bufs=1) as bpool, \
             tc.tile_pool(name="psum", bufs=psum_bufs, space="PSUM") as ppool:
            # ---- load packed constants in one DMA ----
            cts = cpool.tile([128, 153], dt.float32, name="cts")
            nc.sync.dma_start(out=cts[:], in_=AP["consts"])
            ct = {"dt1_alpha": cts[:, 0:1], "dt1_beta": cts[:, 1:2],
                  "s_dt2": cts[:, 2:3], "t_dt2": cts[:, 3:4],
                  "s_dt3": cts[:, 4:5], "t_dt3": cts[:, 5:6],
                  "s_dn1": cts[:, 6:8], "t_dn1": cts[:, 8:10],
                  "s_dn2": cts[:, 10:12], "t_dn2": cts[:, 12:14],
                  "b_dn3": cts[:, 14:153]}
            wt = {}
            for nm, pat in [("w_dt2", "g p o -> p g o"),
                            ("w_dt3", "g p o -> p g o"),
                            ("w_dn1", "t i p o -> p (t i) o"),
                            ("w_dn2", "t i p o -> p (t i) o"),
                            ("w_dn3", "g p o -> p g o")]:
                sh = list(AP[nm].shape)
                wt[nm] = cpool.tile([sh[-2], int(np.prod(sh[:-2])), sh[-1]],
                                    dt.bfloat16, tag=nm, name=f'wt_{nm}')
                nc.sync.dma_start(out=wt[nm][:], in_=AP[nm].rearrange(pat))

            feat_sb = {}
            depth_sb = {}
            for s, S in enumerate(SEGS):
                nq, nt2, nt3, nn1, nout = S["nq"], S["nt2"], S["nt3"], S["nn1"], S["nout"]
                # ======== dt1 : affine + relu + row-mask on host-phased d ====
                dph = bpool.tile([128, nq, 177], dt.bfloat16, tag=f"dph{s}")
                for qq in range(0, nq, nq // 4):
                    nqq = min(nq // 4, nq - qq)
                    nc.sync.dma_start(out=dph[:, qq:qq + nqq, :],
                                      in_=AP[f"dph{s}"][:, qq:qq + nqq, :])
                t1 = bpool.tile([128, nq, 177], dt.bfloat16, tag=f"t1{s}")
                mall = wpool.tile([128, nq + nt2 + nt3 + nn1], dt.bfloat16,
                                  tag=f"msk{s}", name="mall")
                nc.sync.dma_start(out=mall[:], in_=AP[f"masks{s}"])
                QCH = nq // 4
                for qq in range(0, nq, QCH):
                    nqq = min(QCH, nq - qq)
                    sl = (slice(None), slice(qq, qq + nqq), slice(None))
                    nc.vector.tensor_scalar(out=t1[sl], in0=dph[sl],
                                            scalar1=ct["dt1_alpha"][:, 0:1],
                                            scalar2=ct["dt1_beta"][:, 0:1],
                                            op0=mybir.AluOpType.mult,
                                            op1=mybir.AluOpType.add)
                    nc.vector.tensor_scalar(out=t1[sl], in0=t1[sl], scalar1=0.0,
                                            scalar2=None, op0=mybir.AluOpType.max)
                    mb = bass.AP(mall.tensor, mall.offset + qq,
                                 [mall.ap[0], [1, nqq], [0, 177]])
                    nc.vector.tensor_tensor(out=t1[sl], in0=t1[sl], in1=mb,
                                            op=mybir.AluOpType.mult)
                    nc.vector.memset(t1[:, qq:qq + nqq, 0:1], 0.0)
                if s == 0 and debug:
                    nc.sync.dma_start(out=DBG["dbg_t1"], in_=t1[:])

                if stages < 2:
                    continue
                # ======== dt2 ========
                o2 = bpool.tile([32, nt2 + 1, 180], dt.bfloat16, tag=f"o2{s}")
                nc.vector.memset(o2[:], 0.0)
                m2 = bass.AP(mall.tensor, mall.offset + nq, [mall.ap[0], [1, nt2]])
                RPP2 = 2
                for q0 in range(0, nt2, RPP2):
                    nr = min(RPP2, nt2 - q0)
                    ps = ppool.tile([32, nr, 176], dt.float32, tag=f"ps{s}", name="ps2")
                    gi = 0
                    for dky in range(2):
                        for dmx in range(2):
                            g = dky * 2 + dmx
                            rhs = bass.AP(
                                t1.tensor, t1.offset + (q0 + dky) * 177 + dmx,
                                [t1.ap[0], [177, nr], [1, 176]])
                            nc.tensor.matmul(ps[:], wt["w_dt2"][:, g, :], rhs,
                                             start=(gi == 0), stop=(gi == 3))
                            gi += 1
                    ev = wpool.tile([32, nr, 176], dt.bfloat16, tag=f"ev2{s}")
                    nc.scalar.activation(ev[:], ps[:], RELU,
                                         bias=ct["t_dt2"][0:32, 0:1],
                                         scale=ct["s_dt2"][0:32, 0:1])
                    mbb = bass.AP(m2.tensor, m2.offset + q0,
                                  [[m2.ap[0][0], 32], [1, nr], [0, 176]])
                    # write col c at (c%2)*90 + c//2 + 1  (phase-split layout)
                    o2dst = bass.AP(o2.tensor, o2.offset + q0 * 180 + 1,
                                    [[o2.ap[0][0], 32], [180, nr],
                                     [1, 88], [90, 2]])
                    nc.vector.tensor_tensor(out=o2dst, in0=ev[:], in1=mbb,
                                            op=mybir.AluOpType.mult)
                nc.sync.dma_start(out=scr[f"dt2o{s}"],
                                  in_=o2.rearrange("p q (b x) -> p q b x", b=2))
                if s == 0 and debug:
                    nc.sync.dma_start(out=DBG["dbg_dt2o"], in_=o2[:])

                if stages < 3:
                    continue
                # ======== dt3 ========
                nry3 = nt3 + 2
                ph3 = bpool.tile([128, nry3, 90], dt.bfloat16, tag=f"ph3{s}")
                sd2 = scr[f"dt2o{s}"]
                for a2 in range(2):
                    for b2 in range(2):
                        pap3 = bass.AP(sd2.tensor,
                                       sd2.offset + a2 * 180 + b2 * 90,
                                       [[(nt2 + 1) * 180, 32],
                                        [2 * 180, nry3], [1, 90]])
                        nc.sync.dma_start(
                            out=ph3[(a2 * 2 + b2) * 32:(a2 * 2 + b2 + 1) * 32],
                            in_=pap3)
                # concat input tile: [64 dt3 | pad] plus x_img tiles
                dtc = bpool.tile([64, nt3, 92], dt.bfloat16, tag=f"dtc{s}")
                nc.vector.memset(dtc[:], 0.0)
                m3 = bass.AP(mall.tensor, mall.offset + nq + nt2,
                             [mall.ap[0], [1, nt3]])
                RPP3 = 4
                for t0 in range(0, nt3, RPP3):
                    nr = min(RPP3, nt3 - t0)
                    ps = ppool.tile([64, nr, 88], dt.float32, tag=f"ps{s}")
                    gi = 0
                    for dky in range(3):
                        for dmx in range(3):
                            g = dky * 3 + dmx
                            rhs = bass.AP(ph3.tensor,
                                          ph3.offset + (t0 + dky) * 90 + dmx,
                                          [ph3.ap[0], [90, nr], [1, 88]])
                            nc.tensor.matmul(ps[:], wt["w_dt3"][:, g, :], rhs,
                                             start=(gi == 0), stop=(gi == 8))
                            gi += 1
                    ev = wpool.tile([64, nr, 88], dt.bfloat16, tag=f"ev3{s}")
                    nc.scalar.activation(ev[:], ps[:], RELU,
                                         bias=ct["t_dt3"][0:64, 0:1],
                                         scale=ct["s_dt3"][0:64, 0:1])
                    mbb = bass.AP(m3.tensor, m3.offset + t0,
                                  [m3.ap[0], [1, nr], [0, 88]])
                    nc.vector.tensor_tensor(out=dtc[:, t0:t0 + nr, 2:90],
                                            in0=ev[:], in1=mbb[0:64],
                                            op=mybir.AluOpType.mult)

                if s == 0 and debug:
                    nc.sync.dma_start(out=DBG["dbg_dtc"], in_=dtc[:])
                if stages < 4:
                    continue
                # ======== dn1 ========
                xs = []
                for g in range(2):
                    xt = bpool.tile([128, nt3, 92], dt.bfloat16, tag=f"x{g}_{s}",
                                     name=f"xseg_t{g}")
                    nc.vector.memset(xt[:], 0.0)
                    nc.sync.dma_start(
                        out=xt[:, :, 2:90],
                        in_=AP[f"xseg{s}"][g * 128:(g + 1) * 128])
                    xs.append(xt)
                mn1 = bass.AP(mall.tensor, mall.offset + nq + nt2 + nt3,
                              [mall.ap[0], [1, nn1]])
                n1o = []
                for g in range(2):
                    t = bpool.tile([128, nn1, 92], dt.bfloat16, tag=f"n1o{g}_{s}")
                    nc.vector.memset(t[:], 0.0)
                    n1o.append(t)
                RPP = 5
                for ocg in range(2):
                    for r0 in range(0, nn1, RPP):
                        nr = min(RPP, nn1 - r0)
                        ps = ppool.tile([128, nr, 88], dt.float32, tag=f"ps{s}")
                        gi = 0
                        for ky in range(3):
                            for kx in range(3):
                                tap = ky * 3 + kx
                                for icc, srcT in enumerate((xs[0], xs[1], dtc)):
                                    kk = 128 if icc < 2 else 64
                                    rhs = bass.AP(
                                        srcT.tensor,
                                        srcT.offset + (r0 + ky + 1) * 92 + kx + 1,
                                        [srcT.ap[0], [92, nr], [1, 88]])
                                    lhs = wt["w_dn1"][0:kk, tap * 3 + icc,
                                                      ocg * 128:(ocg + 1) * 128]
                                    nc.tensor.matmul(ps[:], lhs, rhs,
                                                     start=(gi == 0),
                                                     stop=(gi == 26))
                                    gi += 1
                        ev = wpool.tile([128, nr, 88], dt.bfloat16, tag=f"evn1{s}")
                        nc.scalar.activation(ev[:], ps[:], RELU,
                                             bias=ct["t_dn1"][:, ocg:ocg + 1],
                                             scale=ct["s_dn1"][:, ocg:ocg + 1])
                        mbb = bass.AP(mn1.tensor, mn1.offset + r0,
                                      [mn1.ap[0], [1, nr], [0, 88]])
                        nc.vector.tensor_tensor(
                            out=n1o[ocg][:, r0:r0 + nr, 2:90],
                            in0=ev[:], in1=mbb, op=mybir.AluOpType.mult)

                if s == 0 and debug:
                    nc.sync.dma_start(out=DBG["dbg_n1o"], in_=n1o[0][:])
                if stages < 5:
                    continue
                # ======== dn2 ========
                n2o = []
                for g in range(2):
                    n2o.append(bpool.tile([128, nout, 88], dt.bfloat16,
                                          tag=f"n2o{g}_{s}", name=f"n2o{g}"))
                for ocg in range(2):
                    for r0 in range(0, nout, RPP):
                        nr = min(RPP, nout - r0)
                        ps = ppool.tile([128, nr, 88], dt.float32, tag=f"ps{s}")
                        gi = 0
                        for ky in range(3):
                            for kx in range(3):
                                tap = ky * 3 + kx
                                for icc in range(2):
                                    rhs = bass.AP(
                                        n1o[icc].tensor,
                                        n1o[icc].offset + (r0 + ky) * 92 + kx + 1,
                                        [n1o[icc].ap[0], [92, nr], [1, 88]])
                                    lhs = wt["w_dn2"][:, tap * 2 + icc,
                                                      ocg * 128:(ocg + 1) * 128]
                                    nc.tensor.matmul(ps[:], lhs, rhs,
                                                     start=(gi == 0),
                                                     stop=(gi == 17))
                                    gi += 1
                        ev = wpool.tile([128, nr, 88], dt.bfloat16, tag=f"evn2{s}")
                        nc.scalar.activation(ev[:], ps[:], RELU,
                                             bias=ct["t_dn2"][:, ocg:ocg + 1],
                                             scale=ct["s_dn2"][:, ocg:ocg + 1])
                        nc.vector.tensor_copy(n2o[ocg][:, r0:r0 + nr, :], ev[:])

                if s == 0 and debug:
                    nc.sync.dma_start(out=DBG["dbg_n2o"], in_=n2o[0][:])
                if stages < 6:
                    continue
                # ======== dn3 + softmax + feat ========
                npix = nout * FW
                feat_sb[s] = bpool.tile([128, (npix + 127) // 128, CIMG],
                                        dt.bfloat16, tag=f"feat{s}", name=f"feat_sb{s}")
                depth_sb[s] = bpool.tile([128, (npix + 127) // 128, DD],
                                         dt.float32, tag=f"depth{s}", name=f"depth_sb{s}")
                n2f = [t.rearrange("p a b -> p (a b)") for t in n2o]
                for pc in range((npix + 127) // 128):
                    m = min(128, npix - pc * 128)
                    ps = ppool.tile([m, 139], dt.float32, tag=f"ps{s}")
                    for icc in range(2):
                        nc.tensor.matmul(ps[:], n2f[icc][:, pc * 128:pc * 128 + m],
                                         wt["w_dn3"][:, icc, :],
                                         start=(icc == 0), stop=(icc == 1))
                    # add bias via vector then softmax over first 59
                    lg = wpool.tile([m, 139], dt.float32, tag=f"lg{s}")
                    nc.vector.tensor_tensor(out=lg[:], in0=ps[:],
                                            in1=ct["b_dn3"][0:m],
                                            op=mybir.AluOpType.add)
                    mx = wpool.tile([m, 1], dt.float32, tag=f"mx{s}")
                    nc.vector.reduce_max(mx[:], lg[:, 0:DD],
                                         axis=mybir.AxisListType.X, negate=True)
                    ex = wpool.tile([m, DD], dt.float32, tag=f"ex{s}")
                    nc.scalar.activation(ex[:], lg[:, 0:DD],
                                         mybir.ActivationFunctionType.Exp,
                                         bias=mx[:, 0:1], scale=1.0)
                    sm = wpool.tile([m, 1], dt.float32, tag=f"sm{s}")
                    nc.vector.reduce_sum(sm[:], ex[:], axis=mybir.AxisListType.X)
                    rc = wpool.tile([m, 1], dt.float32, tag=f"rc{s}")
                    nc.vector.reciprocal(rc[:], sm[:])
                    nc.vector.tensor_scalar(out=depth_sb[s][0:m, pc, :], in0=ex[:],
                                            scalar1=rc[:, 0:1], scalar2=None,
                                            op0=mybir.AluOpType.mult)
                    nc.vector.tensor_copy(feat_sb[s][0:m, pc, :],
                                          lg[:, DD:DD + CIMG])

            # DMA outputs: global pix index = seg-A pix then seg-B pix
            for s, S in (enumerate(SEGS) if stages >= 6 else []):
                npix = S["nout"] * FW
                base = 0 if s == 0 else 16 * FW
                nfull = npix // 128
                dsl = out_depth[base:base + nfull * 128].rearrange(
                    "(a p) d -> p a d", p=128)
                nc.sync.dma_start(out=dsl, in_=depth_sb[s][:, 0:nfull, :])
                fsl = out_feat[base:base + nfull * 128].rearrange(
                    "(a p) d -> p a d", p=128)
                nc.sync.dma_start(out=fsl, in_=feat_sb[s][:, 0:nfull, :])
                rem = npix - nfull * 128
                if rem:
                    nc.sync.dma_start(
                        out=out_depth[base + nfull * 128:base + npix],
                        in_=depth_sb[s][0:rem, nfull, :])
                    nc.sync.dma_start(
                        out=out_feat[base + nfull * 128:base + npix],
                        in_=feat_sb[s][0:rem, nfull, :])
    nc.compile()
    return nc


# ------------------------------------------------------------ host helpers
def _host_geometry(rots, trans, intr, post_rots, post_trans):
    import jax
    import jax.numpy as jnp
    with jax.default_device(jax.devices("cpu")[0]):
        f32 = jnp.float32
        ds = jnp.arange(1.0, 60.0, 1.0, dtype=f32)
        xs = jnp.linspace(0.0, IW - 1.0, FW, dtype=f32)
        ys = jnp.linspace(0.0, IH - 1.0, FH, dtype=f32)
        dm = jnp.broadcast_to(ds[:, None, None], (DD, FH, FW))
        xm = jnp.broadcast_to(xs[None, None, :], (DD, FH, FW))
        ym = jnp.broadcast_to(ys[None, :, None], (DD, FH, FW))
        fr = jnp.stack([xm, ym, dm], -1)
        pts = fr[None, None] - jnp.asarray(post_trans)[:, :, None, None, None, :]
        pts = jnp.einsum("bnij,bndhwj->bndhwi",
                         jnp.linalg.inv(jnp.asarray(post_rots)), pts)
        pts = jnp.concatenate([pts[..., :2] * pts[..., 2:3], pts[..., 2:3]], -1)
        comb = jnp.einsum("bnij,bnjk->bnik", jnp.asarray(rots),
                          jnp.linalg.inv(jnp.asarray(intr)))
        pts = jnp.einsum("bnij,bndhwj->bndhwi", comb, pts) \
            + jnp.asarray(trans)[:, :, None, None, None, :]
        lo = jnp.array([XY0, XY0, Z0], dtype=f32)
        dxv = jnp.array([DXY, DXY, DZ], dtype=f32)
        g = ((pts - lo) / dxv).astype(jnp.int32).reshape(-1, 3)
        kept = ((g[:, 0] >= 0) & (g[:, 0] < NX) & (g[:, 1] >= 0) & (g[:, 1] < NX)
                & (g[:, 2] >= 0) & (g[:, 2] < NZ))
        flat = (g[:, 2] * NX + g[:, 0]) * NX + g[:, 1]
        return np.asarray(flat, np.int64), np.asarray(kept)


def _prep_a_inputs(inputs):
    """Build per-core input maps for launch A."""
    d = np.asarray(inputs["d"], np.float32).reshape(N, IH, IW)
    x_img = np.asarray(inputs["x_img"], np.float32)

    # dt1 folded affine: relu(alpha*d + beta), alpha = s*w, beta = s*b + t
    a1 = (inputs["dt1_s"] * inputs["dt1_w"][:, 0, 0, 0]).astype(np.float32)
    b1 = (inputs["dt1_s"] * inputs["dt1_b"] + inputs["dt1_t"]).astype(np.float32)
    cab = np.arange(128)
    dt1_alpha = a1[cab // 16][:, None]
    dt1_beta = b1[cab // 16][:, None]

    def wprep_dt2():
        w = np.asarray(inputs["dt2_w"], np.float32)      # [32,8,5,5]
        out = np.zeros((4, 128, 32), np.float32)
        for ky in range(5):
            for kx in range(5):
                a, dky = ky % 4, ky // 4
                bph, dmx = (kx + 2) % 4, (kx + 2) // 4
                g = dky * 2 + dmx
                rows = (np.arange(8)) * 16 + a * 4 + bph
                out[g, rows, :] = w[:, :, ky, kx].T
        return out.astype(bf16)

    def wprep_dt3():
        w = np.asarray(inputs["dt3_w"], np.float32)      # [64,32,5,5]
        out = np.zeros((9, 128, 64), np.float32)
        for ky in range(5):
            for kx in range(5):
                a, dky = ky % 2, ky // 2
                bph, dmx = kx % 2, (kx + 2) // 2 - 1
                g = dky * 3 + dmx
                rows = (a * 2 + bph) * 32 + np.arange(32)
                out[g, rows, :] = w[:, :, ky, kx].T
        return out.astype(bf16)

    def wprep_3x3(w, icc_sizes):
        O, I = w.shape[0], w.shape[1]
        nic = len(icc_sizes)
        out = np.zeros((9, nic, 128, O), np.float32)
        for ky in range(3):
            for kx in range(3):
                tap = ky * 3 + kx
                ic0 = 0
                for icc, sz in enumerate(icc_sizes):
                    out[tap, icc, 0:sz, :] = w[:, ic0:ic0 + sz, ky, kx].T
                    ic0 += sz
        return out.astype(bf16)

    # NOTE: dn1 input concat order is [dt3(64) | x_img(256)] in the reference;
    # our matmul chunks are (x0:128, x1:128, dt3:64) -> weight cols must match:
    w_dn1_full = np.asarray(inputs["dn1_w"], np.float32)
    w_dn1 = np.zeros((9, 3, 128, 256), np.float32)
    for ky in range(3):
        for kx in range(3):
            tap = ky * 3 + kx
            w_dn1[tap, 0, :, :] = w_dn1_full[:, 64:192, ky, kx].T
            w_dn1[tap, 1, :, :] = w_dn1_full[:, 192:320, ky, kx].T
            w_dn1[tap, 2, 0:64, :] = w_dn1_full[:, 0:64, ky, kx].T
    w_dn1 = w_dn1.astype(bf16)
    w_dn2 = wprep_3x3(np.asarray(inputs["dn2_w"], np.float32), [128, 128])
    w_dn3 = np.asarray(inputs["dn3_w"], np.float32)[:, :, 0, 0]  # [139, 256]
    w_dn3p = np.zeros((2, 128, 139), np.float32)
    w_dn3p[0] = w_dn3[:, 0:128].T
    w_dn3p[1] = w_dn3[:, 128:256].T

    def fold_bias(b, s, t):
        # conv bias b then bn scale/shift: relu(s*(x+b) + t) = relu(s*x + (s*b+t))
        return np.asarray(s, np.float32), np.asarray(s * b + t, np.float32)

    s2, t2 = fold_bias(inputs["dt2_b"], inputs["dt2_s"], inputs["dt2_t"])
    s3, t3 = fold_bias(inputs["dt3_b"], inputs["dt3_s"], inputs["dt3_t"])
    sn1, tn1 = fold_bias(inputs["dn1_b"], inputs["dn1_s"], inputs["dn1_t"])
    sn2, tn2 = fold_bias(inputs["dn2_b"], inputs["dn2_s"], inputs["dn2_t"])
    b_dn3 = np.broadcast_to(np.asarray(inputs["dn3_b"], np.float32)[None, :],
                            (128, 139)).copy()

    consts = np.zeros((128, 153), np.float32)
    consts[:, 0] = dt1_alpha[:, 0]
    consts[:, 1] = dt1_beta[:, 0]
    consts[:, 2] = np.tile(s2, 4)
    consts[:, 3] = np.tile(t2, 4)
    consts[:, 4] = np.tile(s3, 2)
    consts[:, 5] = np.tile(t3, 2)
    consts[:, 6:8] = sn1.reshape(2, 128).T
    consts[:, 8:10] = tn1.reshape(2, 128).T
    consts[:, 10:12] = sn2.reshape(2, 128).T
    consts[:, 12:14] = tn2.reshape(2, 128).T
    consts[:, 14:153] = b_dn3
    shared = dict(
        consts=consts,
        w_dt2=wprep_dt2(), w_dt3=wprep_dt3(), w_dn1=w_dn1, w_dn2=w_dn2,
        w_dn3=w_dn3p.astype(bf16),
    )

    maps = []
    for c in range(NCORES):
        m = dict(shared)
        for s, (cam, h0) in enumerate([SEG_A[c], SEG_B[c]]):
            S = SEGS[s]
            d0 = 8 * h0 - 34
            dseg = np.zeros((S["nd"], 712), np.float32)
            lo, hi = max(0, d0), min(IH, d0 + S["nd"])
            if hi > lo:
                dseg[lo - d0:hi - d0, 4:708] = d[cam, lo:hi]
            nq = S["nq"]
            ph = dseg.reshape(nq, 4, 178, 4)[:, :, :177, :]     # ry a rx b
            ph = ph.transpose(1, 3, 0, 2)                        # a b ry rx
            m[f"dph{s}"] = np.broadcast_to(
                ph[None], (8, 4, 4, nq, 177)).reshape(128, nq, 177).astype(bf16)
            # dmask: partition (c8,a,b) x ry -> valid(4ry+a)
            ry = np.arange(S["nd"] // 4)
            rows = 4 * ry[None, :] + (cab[:, None] // 4) % 4
            dmask = (((rows + d0) >= 0) & ((rows + d0) < IH))
            q0, t0, r0 = 2 * h0 - 8, h0 - 3, h0 - 1
            qr = np.arange(S["nt2"]) + q0
            m2m = np.broadcast_to(((qr >= 0) & (qr < 64))[None, :],
                                  (128, S["nt2"]))
            tr = np.arange(S["nt3"]) + t0
            m3m = np.broadcast_to(((tr >= 0) & (tr < FH))[None, :],
                                  (128, S["nt3"]))
            rr = np.arange(S["nn1"]) + r0
            mn1m = np.broadcast_to(((rr >= 0) & (rr < FH))[None, :],
                                   (128, S["nn1"]))
            m[f"masks{s}"] = np.concatenate(
                [dmask, m2m, m3m, mn1m], axis=1).astype(bf16)
            xseg = np.zeros((CIN, S["nt3"], FW), np.float32)
            lo2, hi2 = max(0, t0), min(FH, t0 + S["nt3"])
            if hi2 > lo2:
                xseg[:, lo2 - t0:hi2 - t0, :] = x_img[cam, :, lo2:hi2, :]
            m[f"xseg{s}"] = xseg.astype(bf16)
        maps.append(m)
    return maps


# ---------------------------------------------------------------- launch B
def build_launch_b(W):
    """W windows x QV chunks of 128 points; scatter-sum into virtual windows."""
    nc = bacc.Bacc("TRN2", target_bir_lowering=False, debug=False,
                   num_devices=NCORES)
    NCH = W * QV
    pb = nc.dram_tensor("pb", [128, NCH, CIMG], dt.bfloat16,
                        kind="ExternalInput").ap()
    offv = nc.dram_tensor("offv", [128, NCH], dt.float32,
                          kind="ExternalInput").ap()
    depv = nc.dram_tensor("depv", [128, NCH], dt.float32,
                          kind="ExternalInput").ap()
    povirt = nc.dram_tensor("povirt", [128, W, CIMG], dt.float32,
                            kind="ExternalOutput").ap()
    BQ = 32                     # chunks per input batch
    BW = 8                      # windows per output batch
    with tile.TileContext(nc) as tc:
        with tc.tile_pool(name="const", bufs=1) as cpool,              tc.tile_pool(name="io", bufs=4) as iop,              tc.tile_pool(name="g", bufs=8) as gp,              tc.tile_pool(name="ps", bufs=2, space="PSUM") as pp:
            iota4 = cpool.tile([128, 1, 128], dt.bfloat16, name="iota4")
            nc.gpsimd.iota(iota4[:], pattern=[[0, 1], [1, 128]], base=0,
                           channel_multiplier=0,
                           allow_small_or_imprecise_dtypes=True)
            offt = cpool.tile([128, NCH], dt.float32, name="offt")
            nc.sync.dma_start(out=offt[:], in_=offv)
            dept = cpool.tile([128, NCH], dt.float32, name="dept")
            nc.sync.dma_start(out=dept[:], in_=depv)
            for w0 in range(0, W, BW):
                nw = min(BW, W - w0)
                ot = iop.tile([128, BW, CIMG], dt.float32, tag="ot", name="ot")
                for wi in range(nw):
                    w = w0 + wi
                    k0 = w * QV
                    if k0 % BQ == 0:
                        pbt = iop.tile([128, BQ, CIMG], dt.bfloat16,
                                       tag="pbt", name="pbt")
                        nb = min(BQ, NCH - k0)
                        nc.sync.dma_start(out=pbt[:, 0:nb, :],
                                          in_=pb[:, k0:k0 + nb, :])
                    ps = pp.tile([128, CIMG], dt.float32, tag="ps", name="ps")
                    for q in range(QV):
                        k = k0 + q
                        g = gp.tile([128, 128], dt.bfloat16, tag="g", name="g")
                        nc.vector.tensor_scalar(
                            out=g[:], in0=iota4[:, 0, :], scalar1=offt[:, k:k + 1],
                            scalar2=dept[:, k:k + 1],
                            op0=mybir.AluOpType.is_equal,
                            op1=mybir.AluOpType.mult)
                        nc.tensor.matmul(ps[:], g[:], pbt[:, k % BQ, :],
                                         start=(q == 0), stop=(q == QV - 1))
                    nc.scalar.activation(ot[:, wi, :], ps[:],
                                         mybir.ActivationFunctionType.Copy)
                nc.sync.dma_start(out=povirt[:, w0:w0 + nw, :],
                                  in_=ot[:, 0:nw, :])
    nc.compile()
    return nc


# ---------------------------------------------------------------- launch C
C_OUT_ROWS = 23              # ds2-out rows per core (8*23 = 184 >= 180)


def build_launch_c():
    nc = bacc.Bacc("TRN2", target_bir_lowering=False, debug=False,
                   num_devices=NCORES)
    NR1 = C_OUT_ROWS + 2                         # ds1-out rows incl halo (25)
    NRP = 2 * NR1 + 1                            # pooled rows needed (51)
    slab = nc.dram_tensor("slab", [CIMG, NRP, 362], dt.bfloat16,
                          kind="ExternalInput").ap()
    m1 = nc.dram_tensor("m1", [128, NR1], dt.bfloat16, kind="ExternalInput").ap()
    wd1 = nc.dram_tensor("wd1", [9, CIMG, CIMG], dt.bfloat16,
                         kind="ExternalInput").ap()
    wd2 = nc.dram_tensor("wd2", [9, CIMG, CIMG], dt.bfloat16,
                         kind="ExternalInput").ap()
    sb1 = nc.dram_tensor("sb1", [CIMG, 2], dt.float32, kind="ExternalInput").ap()
    sb2 = nc.dram_tensor("sb2", [CIMG, 2], dt.float32, kind="ExternalInput").ap()
    yout = nc.dram_tensor("yout", [CIMG, C_OUT_ROWS, 180], dt.float32,
                          kind="ExternalOutput").ap()
    RELU = mybir.ActivationFunctionType.Relu
    with tile.TileContext(nc) as tc:
        with tc.tile_pool(name="const", bufs=1) as cpool,              tc.tile_pool(name="work", bufs=2) as wp,              tc.tile_pool(name="big", bufs=1) as bp,              tc.tile_pool(name="ps", bufs=3, space="PSUM") as pp:
            slabt = bp.tile([CIMG, NRP, 362], dt.bfloat16, name="slabt")
            for rr in range(0, NRP, 13):
                nrr = min(13, NRP - rr)
                nc.sync.dma_start(out=slabt[:, rr:rr + nrr, :],
                                  in_=slab[:, rr:rr + nrr, :])
            w1 = cpool.tile([CIMG, 9, CIMG], dt.bfloat16, name="w1")
            nc.sync.dma_start(out=w1[:], in_=wd1.rearrange("t p o -> p t o"))
            w2 = cpool.tile([CIMG, 9, CIMG], dt.bfloat16, name="w2")
            nc.sync.dma_start(out=w2[:], in_=wd2.rearrange("t p o -> p t o"))
            sb1t = cpool.tile([CIMG, 2], dt.float32, name="sb1t")
            nc.sync.dma_start(out=sb1t[:], in_=sb1)
            sb2t = cpool.tile([CIMG, 2], dt.float32, name="sb2t")
            nc.sync.dma_start(out=sb2t[:], in_=sb2)
            m1t = wp.tile([128, NR1], dt.bfloat16, name="m1t")
            nc.sync.dma_start(out=m1t[:], in_=m1)
            h1 = bp.tile([CIMG, NR1, 182], dt.bfloat16, name="h1")
            nc.vector.memset(h1[:, :, 0:1], 0.0)
            nc.vector.memset(h1[:, :, 181:182], 0.0)
            # ds1: stride-2 3x3; out row t reads slab rows 2t..2t+2 (slab row 0
            # = pooled row 2o0-3, so out row t (global o0-1+t) reads
            # 2(o0-1+t)-1..+1 - (2o0-3) = 2t..2t+2); col c reads 2c..2c+2
            RP = 2
            for t0 in range(0, NR1, RP):
                nr = min(RP, NR1 - t0)
                ps = pp.tile([CIMG, nr, 180], dt.float32, tag="ps1", name="ps")
                gi = 0
                for ky in range(3):
                    for kx in range(3):
                        rhs = bass.AP(slabt.tensor,
                                      slabt.offset + (2 * t0 + ky) * 362 + kx,
                                      [slabt.ap[0], [2 * 362, nr], [2, 180]])
                        nc.tensor.matmul(ps[:], w1[:, ky * 3 + kx, :], rhs,
                                         start=(gi == 0), stop=(gi == 8))
                        gi += 1
                ev = wp.tile([CIMG, nr, 180], dt.bfloat16, tag="ev", name="ev")
                nc.scalar.activation(ev[:], ps[:], RELU, bias=sb1t[:, 1:2],
                                     scale=sb1t[:, 0:1])
                mbb = bass.AP(m1t.tensor, m1t.offset + t0,
                              [[m1t.ap[0][0], CIMG], [1, nr], [0, 180]])
                nc.vector.tensor_tensor(out=h1[:, t0:t0 + nr, 1:181],
                                        in0=ev[:], in1=mbb,
                                        op=mybir.AluOpType.mult)
            # ds2: 3x3 pad 1: out row o reads h1 rows o..o+2, col c: c..c+2
            yo = bp.tile([CIMG, C_OUT_ROWS, 180], dt.float32, name="yo")
            for o0 in range(0, C_OUT_ROWS, RP):
                nr = min(RP, C_OUT_ROWS - o0)
                ps = pp.tile([CIMG, nr, 180], dt.float32, tag="ps2", name="ps")
                gi = 0
                for ky in range(3):
                    for kx in range(3):
                        rhs = bass.AP(h1.tensor,
                                      h1.offset + (o0 + ky) * 182 + kx,
                                      [h1.ap[0], [182, nr], [1, 180]])
                        nc.tensor.matmul(ps[:], w2[:, ky * 3 + kx, :], rhs,
                                         start=(gi == 0), stop=(gi == 8))
                        gi += 1
                nc.scalar.activation(yo[:, o0:o0 + nr, :], ps[:], RELU,
                                     bias=sb2t[:, 1:2], scale=sb2t[:, 0:1])
                nc.sync.dma_start(out=yout[:, o0:o0 + nr, :],
                                  in_=yo[:, o0:o0 + nr, :])
    nc.compile()
    return nc


_CACHE = {}


def run_launch_a(inputs):
    if "A" not in _CACHE:
        _CACHE["A"] = build_launch_a()
    nc = _CACHE["A"]
    maps = _prep_a_inputs(inputs)
    res = run_bass_kernel_spmd(nc, maps, list(range(NCORES)))
    depth = np.zeros((NPIX, DD), np.float32)
    feat = np.zeros((NPIX, CIMG), np.float32)
    for c in range(NCORES):
        r = res.results[c]
        for s, (cam, h0) in enumerate([SEG_A[c], SEG_B[c]]):
            S = SEGS[s]
            npix = S["nout"] * FW
            base = (cam * FH + h0) * FW
            off = 0 if s == 0 else 16 * FW
            depth[base:base + npix] = r["out_depth"][off:off + npix]
            feat[base:base + npix] = r["out_feat"][off:off + npix].astype(np.float32)
    return depth, feat


def _build_schedule(flat, kept):
    """Sort kept points by (core, local voxel); emit fixed-quota virtual
    windows of QV*128 points with vox-span < 128. Returns per-core schedule
    dicts + W (max window count, rounded to 8)."""
    pts = np.arange(NPTS)
    rem = pts % (DD * FH * FW)
    d_i = rem // (FH * FW)
    col = (pts // (DD * FH * FW)) * (FH * FW) + rem % (FH * FW)
    vox = flat
    vx = (vox // NX).astype(np.int32)

    keep_idx = np.where(kept)[0]
    cnt = np.bincount(vx[keep_idx], minlength=NX)
    order = np.argsort(-cnt, kind="stable")
    core_of_row = np.zeros(NX, np.int32)
    load = np.zeros(NCORES, np.int64)
    for r in order:
        c = int(np.argmin(load))
        core_of_row[r] = c
        load[c] += cnt[r]

    row_rank = np.zeros(NX, np.int64)
    rows_of = []
    for c in range(NCORES):
        rows = np.where(core_of_row == c)[0]
        rows_of.append(rows)
        row_rank[rows] = np.arange(len(rows))

    schedules = []
    for c in range(NCORES):
        sel = keep_idx[core_of_row[vx[keep_idx]] == c]
        vloc = row_rank[vx[sel]] * NX + (vox[sel] % NX)
        o = np.argsort(vloc, kind="stable")
        sel, vloc = sel[o], vloc[o]
        win = []                      # (start, end, base)
        i, n = 0, len(sel)
        while i < n:
            base = vloc[i]
            j = min(i + QV * 128, n)
            hi = np.searchsorted(vloc, base + 128, "left")
            j = min(j, hi)
            win.append((i, j, base))
            i = j
        schedules.append(dict(sel=sel, vloc=vloc, win=win, col=col[sel],
                              d_i=d_i[sel], rows=rows_of[c]))
    W = max(len(s["win"]) for s in schedules)
    W = (W + 7) // 8 * 8
    return schedules, W


def _prep_b_inputs(schedules, W, depth_rows, featflat_bf):
    maps = []
    NCH = W * QV
    for sch in schedules:
        pb = np.zeros((128, NCH, CIMG), bf16)
        offv = np.zeros((128, NCH), np.float32)
        depv = np.zeros((128, NCH), np.float32)  # cast to bf16 at the end
        col, d_i, vloc = sch["col"], sch["d_i"], sch["vloc"]
        dvals = depth_rows[col, d_i]
        for w, (i, j, base) in enumerate(sch["win"]):
            L = j - i
            nch = (L + 127) // 128
            gath = featflat_bf[col[i:j]]
            for q in range(nch):
                lo, hi = q * 128, min((q + 1) * 128, L)
                k = w * QV + q
                pb[0:hi - lo, k] = gath[lo:hi]
                offv[0:hi - lo, k] = vloc[i + lo:i + hi] - base
                depv[0:hi - lo, k] = dvals[i + lo:i + hi]
        maps.append(dict(pb=pb, offv=offv, depv=depv))
    return maps


def _prep_c_inputs(inputs, pooled_t):
    """pooled_t: [CIMG, 360, 360] f32 -> per-core slabs + masks + weights."""
    NR1 = C_OUT_ROWS + 2
    NRP = 2 * NR1 + 1
    w1 = np.asarray(inputs["ds1_w"], np.float32)
    w2 = np.asarray(inputs["ds2_w"], np.float32)
    wd1 = np.stack([w1[:, :, ky, kx].T for ky in range(3) for kx in range(3)])
    wd2 = np.stack([w2[:, :, ky, kx].T for ky in range(3) for kx in range(3)])
    sb1 = np.stack([np.asarray(inputs["ds1_s"], np.float32),
                    np.asarray(inputs["ds1_t"], np.float32)], 1)
    sb2 = np.stack([np.asarray(inputs["ds2_s"], np.float32),
                    np.asarray(inputs["ds2_t"], np.float32)], 1)
    shared = dict(wd1=wd1.astype(bf16), wd2=wd2.astype(bf16), sb1=sb1, sb2=sb2)
    maps = []
    pt_bf = pooled_t.astype(bf16)
    for c in range(NCORES):
        o0g = C_OUT_ROWS * c
        p0 = 2 * o0g - 3
        slab = np.zeros((CIMG, NRP, 362), bf16)
        lo, hi = max(0, p0), min(NX, p0 + NRP)
        if hi > lo:
            slab[:, lo - p0:hi - p0, 1:361] = pt_bf[:, lo:hi, :]
        t1g = np.arange(NR1) + (o0g - 1)
        m1 = np.broadcast_to(((t1g >= 0) & (t1g < 180))[None, :],
                             (128, NR1)).astype(bf16)
        maps.append(dict(shared, slab=slab, m1=np.ascontiguousarray(m1)))
    return maps


def kernel(**inputs):
    inputs = {k: np.asarray(v) for k, v in inputs.items()}
    flat, kept = _host_geometry(inputs["cam2lidar_rots"],
                                inputs["cam2lidar_trans"], inputs["intrins"],
                                inputs["post_rots"], inputs["post_trans"])
    depth_rows, feat_rows = run_launch_a(inputs)
    featflat_bf = feat_rows.astype(bf16)

    schedules, W = _build_schedule(flat, kept)
    key = ("B", W)
    if key not in _CACHE:
        _CACHE[key] = build_launch_b(W)
    bmaps = _prep_b_inputs(schedules, W, depth_rows, featflat_bf)
    res_b = run_bass_kernel_spmd(_CACHE[key], bmaps, list(range(NCORES)))

    pooled = np.zeros((NX * NX, CIMG), np.float32)
    for c, sch in enumerate(schedules):
        virt = res_b.results[c]["povirt"].transpose(1, 0, 2)  # -> [W, 128, C]
        rows_arr = sch["rows"]
        nloc = len(rows_arr) * NX
        for w, (i, j, base) in enumerate(sch["win"]):
            span = min(128, nloc - base)
            lidx = base + np.arange(span)
            ridx = rows_arr[lidx // NX] * NX + (lidx % NX)
            pooled[ridx] += virt[w][:span]
    pooled_t = np.ascontiguousarray(
        pooled.reshape(NX, NX, CIMG).transpose(2, 0, 1))

    if "C" not in _CACHE:
        _CACHE["C"] = build_launch_c()
    cmaps = _prep_c_inputs(inputs, pooled_t)
    res_c = run_bass_kernel_spmd(_CACHE["C"], cmaps, list(range(NCORES)))
    out = np.zeros((1, CIMG, 180, 180), np.float32)
    for c in range(NCORES):
        o0g = C_OUT_ROWS * c
        nr = min(C_OUT_ROWS, 180 - o0g)
        if nr > 0:
            out[0, :, o0g:o0g + nr, :] = res_c.results[c]["yout"][:, 0:nr, :]
    return out



# revision 4
# speedup vs baseline: 1.4280x; 1.4280x over previous
"""DepthLSSTransform Trainium kernel: 3 SPMD launches over 8 NeuronCores.

Launch A: per-camera conv pipeline (dtransform + depthnet + softmax) on
          24-row bands (one 16-row + one 8-row segment per core).
Launch B: bev_pool segment-sum via one-hot matmuls over a host-built
          virtual-window schedule (sorted-by-voxel points).
Launch C: BEV downsample convs, spatially sharded.
Host: geometry/voxel indices, scheduling, gathers, folds (orchestration).
"""
import numpy as np
import ml_dtypes

import concourse.bass as bass
import concourse.tile as tile
from concourse import bacc, mybir
from concourse.bass_utils import run_bass_kernel_spmd

dt = mybir.dt
bf16 = ml_dtypes.bfloat16

# ---- problem constants (hardcoded per contract) ----
B, N = 1, 6
CIN, CIMG, DD = 256, 80, 59
FH, FW, IH, IW = 32, 88, 256, 704
XY0, DXY, NX = -54.0, 0.3, 360
Z0, DZ, NZ = -10.0, 20.0, 1
NPTS = N * DD * FH * FW
NPIX = N * FH * FW
NCORES = 8
QV = 4                      # chunks of 128 points per virtual window

# per-core segments: (camera, h0) for seg A (16 rows) and seg B (8 rows)
SEG_A = [(0, 0), (1, 0), (1, 16), (2, 16), (3, 0), (4, 0), (4, 16), (5, 16)]
SEG_B = [(0, 16), (0, 24), (2, 0), (2, 8), (3, 16), (3, 24), (5, 0), (5, 8)]
# band pixel ranges in global row order (row = n*32 + h)
ROWS_OF_CORE = [[(SEG_A[c][0] * FH + SEG_A[c][1] + r) for r in range(16)] +
                [(SEG_B[c][0] * FH + SEG_B[c][1] + r) for r in range(8)]
                for c in range(NCORES)]

# segment geometry: rows16 segment: d rows [8h0-34, 8h0+158) (192), dt2 out
# rows [2h0-8, 2h0+39) (47), dt3 [h0-3, h0+19) (22), dn1 [h0-1, h0+17) (18)
SEGS = [dict(nout=16, nd=192, nq=48, nt2=47, nt3=22, nn1=18),
        dict(nout=8, nd=128, nq=32, nt2=31, nt3=14, nn1=10)]


def _seg_ranges(h0, S):
    return dict(d0=8 * h0 - 34, q0=2 * h0 - 8, t0=h0 - 3, r0=h0 - 1, o0=h0)


# ---------------------------------------------------------------- launch A
def build_launch_a(debug=False, psum_bufs=3, work_bufs=3, stages=9):
    nc = bacc.Bacc("TRN2", target_bir_lowering=False, debug=False,
                   num_devices=NCORES)
    AP = {}

    def inp(name, shape, dtype=dt.bfloat16):
        AP[name] = nc.dram_tensor(name, shape, dtype, kind="ExternalInput").ap()
        return AP[name]

    # per segment inputs (s = 0: 16-row, 1: 8-row)
    for s, S in enumerate(SEGS):
        inp(f"dph{s}", [128, S["nq"], 177])
        inp(f"masks{s}", [128, S["nq"] + S["nt2"] + S["nt3"] + S["nn1"]])
        inp(f"xseg{s}", [CIN, S["nt3"], FW])            # x_img slice (zeroed oob)
    # packed f32 constants: [alpha, beta, s_dt2, t_dt2, s_dt3, t_dt3,
    #  s_dn1(2), t_dn1(2), s_dn2(2), t_dn2(2), b_dn3(139)] -> [128, 153]
    inp("consts", [128, 153], dt.float32)
    # conv weights (host-prepped layouts)
    inp("w_dt2", [4, 128, 32])                          # groups (dky,dmx)
    inp("w_dt3", [9, 128, 64])
    inp("w_dn1", [9, 3, 128, 256])                      # tap, icchunk(128,128,64pad) -> 256
    inp("w_dn2", [9, 2, 128, 256])
    inp("w_dn3", [2, 128, 139])

    DBG = {}
    dbg_specs = [] if not debug else [("dbg_t1", [128, SEGS[0]["nq"], 177], dt.bfloat16),
                        ("dbg_dt2o", [32, SEGS[0]["nt2"] + 1, 180], dt.bfloat16),
                        ("dbg_dtc", [64, SEGS[0]["nt3"], 92], dt.bfloat16),
                        ("dbg_n1o", [128, SEGS[0]["nn1"], 92], dt.bfloat16),
                        ("dbg_n2o", [128, SEGS[0]["nout"], 88], dt.bfloat16)]
    for nm, sh, dty in dbg_specs:
        DBG[nm] = nc.dram_tensor(nm, sh, dty, kind="ExternalOutput").ap()
    out_depth = nc.dram_tensor("out_depth", [24 * FW, DD], dt.float32,
                               kind="ExternalOutput").ap()
    out_feat = nc.dram_tensor("out_feat", [24 * FW, CIMG], dt.bfloat16,
                              kind="ExternalOutput").ap()

    # HBM scratch
    scr = {}
    for s, S in enumerate(SEGS):
        scr[f"dt2o{s}"] = nc.dram_tensor(f"dt2o{s}", [32, S["nt2"] + 1, 2, 90], dt.bfloat16).ap()

    RELU = mybir.ActivationFunctionType.Relu
    with tile.TileContext(nc) as tc:
        with tc.tile_pool(name="const", bufs=1) as cpool, \
             tc.tile_pool(name="work", bufs=work_bufs) as wpool, \
             tc.tile_pool(name="big", bufs=1) as bpool, \
             tc.tile_pool(name="psum", bufs=psum_bufs, space="PSUM") as ppool:
            # ---- load packed constants in one DMA ----
            cts = cpool.tile([128, 153], dt.float32, name="cts")
            nc.sync.dma_start(out=cts[:], in_=AP["consts"])
            ct = {"dt1_alpha": cts[:, 0:1], "dt1_beta": cts[:, 1:2],
                  "s_dt2": cts[:, 2:3], "t_dt2": cts[:, 3:4],
                  "s_dt3": cts[:, 4:5], "t_dt3": cts[:, 5:6],
                  "s_dn1": cts[:, 6:8], "t_dn1": cts[:, 8:10],
                  "s_dn2": cts[:, 10:12], "t_dn2": cts[:, 12:14],
                  "b_dn3": cts[:, 14:153]}
            wt = {}
            for nm, pat in [("w_dt2", "g p o -> p g o"),
                            ("w_dt3", "g p o -> p g o"),
                            ("w_dn1", "t i p o -> p (t i) o"),
                            ("w_dn2", "t i p o -> p (t i) o"),
                            ("w_dn3", "g p o -> p g o")]:
                sh = list(AP[nm].shape)
                wt[nm] = cpool.tile([sh[-2], int(np.prod(sh[:-2])), sh[-1]],
                                    dt.bfloat16, tag=nm, name=f'wt_{nm}')
                nc.sync.dma_start(out=wt[nm][:], in_=AP[nm].rearrange(pat))

            feat_sb = {}
            depth_sb = {}
            for s, S in enumerate(SEGS):
                nq, nt2, nt3, nn1, nout = S["nq"], S["nt2"], S["nt3"], S["nn1"], S["nout"]
                # ======== dt1 : affine + relu + row-mask on host-phased d ====
                dph = bpool.tile([128, nq, 177], dt.bfloat16, tag=f"dph{s}")
                for qq in range(0, nq, nq // 4):
                    nqq = min(nq // 4, nq - qq)
                    nc.sync.dma_start(out=dph[:, qq:qq + nqq, :],
                                      in_=AP[f"dph{s}"][:, qq:qq + nqq, :])
                t1 = bpool.tile([128, nq, 177], dt.bfloat16, tag=f"t1{s}")
                mall = wpool.tile([128, nq + nt2 + nt3 + nn1], dt.bfloat16,
                                  tag=f"msk{s}", name="mall")
                nc.sync.dma_start(out=mall[:], in_=AP[f"masks{s}"])
                QCH = nq // 4
                for qq in range(0, nq, QCH):
                    nqq = min(QCH, nq - qq)
                    sl = (slice(None), slice(qq, qq + nqq), slice(None))
                    nc.vector.tensor_scalar(out=t1[sl], in0=dph[sl],
                                            scalar1=ct["dt1_alpha"][:, 0:1],
                                            scalar2=ct["dt1_beta"][:, 0:1],
                                            op0=mybir.AluOpType.mult,
                                            op1=mybir.AluOpType.add)
                    nc.vector.tensor_scalar(out=t1[sl], in0=t1[sl], scalar1=0.0,
                                            scalar2=None, op0=mybir.AluOpType.max)
                    mb = bass.AP(mall.tensor, mall.offset + qq,
                                 [mall.ap[0], [1, nqq], [0, 177]])
                    nc.vector.tensor_tensor(out=t1[sl], in0=t1[sl], in1=mb,
                                            op=mybir.AluOpType.mult)
                    nc.vector.memset(t1[:, qq:qq + nqq, 0:1], 0.0)
                if s == 0 and debug:
                    nc.sync.dma_start(out=DBG["dbg_t1"], in_=t1[:])

                if stages < 2:
                    continue
                # ======== dt2 ========
                o2 = bpool.tile([32, nt2 + 1, 180], dt.bfloat16, tag=f"o2{s}")
                nc.vector.memset(o2[:], 0.0)
                m2 = bass.AP(mall.tensor, mall.offset + nq, [mall.ap[0], [1, nt2]])
                RPP2 = 2
                for q0 in range(0, nt2, RPP2):
                    nr = min(RPP2, nt2 - q0)
                    ps = ppool.tile([32, nr, 176], dt.float32, tag=f"ps{s}", name="ps2")
                    gi = 0
                    for dky in range(2):
                        for dmx in range(2):
                            g = dky * 2 + dmx
                            rhs = bass.AP(
                                t1.tensor, t1.offset + (q0 + dky) * 177 + dmx,
                                [t1.ap[0], [177, nr], [1, 176]])
                            nc.tensor.matmul(ps[:], wt["w_dt2"][:, g, :], rhs,
                                             start=(gi == 0), stop=(gi == 3))
                            gi += 1
                    ev = wpool.tile([32, nr, 176], dt.bfloat16, tag=f"ev2{s}")
                    nc.scalar.activation(ev[:], ps[:], RELU,
                                         bias=ct["t_dt2"][0:32, 0:1],
                                         scale=ct["s_dt2"][0:32, 0:1])
                    mbb = bass.AP(m2.tensor, m2.offset + q0,
                                  [[m2.ap[0][0], 32], [1, nr], [0, 176]])
                    # write col c at (c%2)*90 + c//2 + 1  (phase-split layout)
                    o2dst = bass.AP(o2.tensor, o2.offset + q0 * 180 + 1,
                                    [[o2.ap[0][0], 32], [180, nr],
                                     [1, 88], [90, 2]])
                    nc.vector.tensor_tensor(out=o2dst, in0=ev[:], in1=mbb,
                                            op=mybir.AluOpType.mult)
                nc.sync.dma_start(out=scr[f"dt2o{s}"],
                                  in_=o2.rearrange("p q (b x) -> p q b x", b=2))
                if s == 0 and debug:
                    nc.sync.dma_start(out=DBG["dbg_dt2o"], in_=o2[:])

                if stages < 3:
                    continue
                # ======== dt3 ========
                nry3 = nt3 + 2
                ph3 = bpool.tile([128, nry3, 90], dt.bfloat16, tag=f"ph3{s}")
                sd2 = scr[f"dt2o{s}"]
                for a2 in range(2):
                    for b2 in range(2):
                        pap3 = bass.AP(sd2.tensor,
                                       sd2.offset + a2 * 180 + b2 * 90,
                                       [[(nt2 + 1) * 180, 32],
                                        [2 * 180, nry3], [1, 90]])
                        nc.sync.dma_start(
                            out=ph3[(a2 * 2 + b2) * 32:(a2 * 2 + b2 + 1) * 32],
                            in_=pap3)
                # concat input tile: [64 dt3 | pad] plus x_img tiles
                dtc = bpool.tile([64, nt3, 92], dt.bfloat16, tag=f"dtc{s}")
                nc.vector.memset(dtc[:], 0.0)
                m3 = bass.AP(mall.tensor, mall.offset + nq + nt2,
                             [mall.ap[0], [1, nt3]])
                RPP3 = 4
                for t0 in range(0, nt3, RPP3):
                    nr = min(RPP3, nt3 - t0)
                    ps = ppool.tile([64, nr, 88], dt.float32, tag=f"ps{s}")
                    gi = 0
                    for dky in range(3):
                        for dmx in range(3):
                            g = dky * 3 + dmx
                            rhs = bass.AP(ph3.tensor,
                                          ph3.offset + (t0 + dky) * 90 + dmx,
                                          [ph3.ap[0], [90, nr], [1, 88]])
                            nc.tensor.matmul(ps[:], wt["w_dt3"][:, g, :], rhs,
                                             start=(gi == 0), stop=(gi == 8))
                            gi += 1
                    ev = wpool.tile([64, nr, 88], dt.bfloat16, tag=f"ev3{s}")
                    nc.scalar.activation(ev[:], ps[:], RELU,
                                         bias=ct["t_dt3"][0:64, 0:1],
                                         scale=ct["s_dt3"][0:64, 0:1])
                    mbb = bass.AP(m3.tensor, m3.offset + t0,
                                  [m3.ap[0], [1, nr], [0, 88]])
                    nc.vector.tensor_tensor(out=dtc[:, t0:t0 + nr, 2:90],
                                            in0=ev[:], in1=mbb[0:64],
                                            op=mybir.AluOpType.mult)

                if s == 0 and debug:
                    nc.sync.dma_start(out=DBG["dbg_dtc"], in_=dtc[:])
                if stages < 4:
                    continue
                # ======== dn1 ========
                xs = []
                for g in range(2):
                    xt = bpool.tile([128, nt3, 92], dt.bfloat16, tag=f"x{g}_{s}",
                                     name=f"xseg_t{g}")
                    nc.vector.memset(xt[:], 0.0)
                    nc.sync.dma_start(
                        out=xt[:, :, 2:90],
                        in_=AP[f"xseg{s}"][g * 128:(g + 1) * 128])
                    xs.append(xt)
                mn1 = bass.AP(mall.tensor, mall.offset + nq + nt2 + nt3,
                              [mall.ap[0], [1, nn1]])
                n1o = []
                for g in range(2):
                    t = bpool.tile([128, nn1, 92], dt.bfloat16, tag=f"n1o{g}_{s}")
                    nc.vector.memset(t[:], 0.0)
                    n1o.append(t)
                RPP = 5
                for ocg in range(2):
                    for r0 in range(0, nn1, RPP):
                        nr = min(RPP, nn1 - r0)
                        ps = ppool.tile([128, nr, 88], dt.float32, tag=f"ps{s}")
                        gi = 0
                        for ky in range(3):
                            for kx in range(3):
                                tap = ky * 3 + kx
                                for icc, srcT in enumerate((xs[0], xs[1], dtc)):
                                    kk = 128 if icc < 2 else 64
                                    rhs = bass.AP(
                                        srcT.tensor,
                                        srcT.offset + (r0 + ky + 1) * 92 + kx + 1,
                                        [srcT.ap[0], [92, nr], [1, 88]])
                                    lhs = wt["w_dn1"][0:kk, tap * 3 + icc,
                                                      ocg * 128:(ocg + 1) * 128]
                                    nc.tensor.matmul(ps[:], lhs, rhs,
                                                     start=(gi == 0),
                                                     stop=(gi == 26))
                                    gi += 1
                        ev = wpool.tile([128, nr, 88], dt.bfloat16, tag=f"evn1{s}")
                        nc.scalar.activation(ev[:], ps[:], RELU,
                                             bias=ct["t_dn1"][:, ocg:ocg + 1],
                                             scale=ct["s_dn1"][:, ocg:ocg + 1])
                        mbb = bass.AP(mn1.tensor, mn1.offset + r0,
                                      [mn1.ap[0], [1, nr], [0, 88]])
                        nc.vector.tensor_tensor(
                            out=n1o[ocg][:, r0:r0 + nr, 2:90],
                            in0=ev[:], in1=mbb, op=mybir.AluOpType.mult)

                if s == 0 and debug:
                    nc.sync.dma_start(out=DBG["dbg_n1o"], in_=n1o[0][:])
                if stages < 5:
                    continue
                # ======== dn2 ========
                n2o = []
                for g in range(2):
                    n2o.append(bpool.tile([128, nout, 88], dt.bfloat16,
                                          tag=f"n2o{g}_{s}", name=f"n2o{g}"))
                for ocg in range(2):
                    for r0 in range(0, nout, RPP):
                        nr = min(RPP, nout - r0)
                        ps = ppool.tile([128, nr, 88], dt.float32, tag=f"ps{s}")
                        gi = 0
                        for ky in range(3):
                            for kx in range(3):
                                tap = ky * 3 + kx
                                for icc in range(2):
                                    rhs = bass.AP(
                                        n1o[icc].tensor,
                                        n1o[icc].offset + (r0 + ky) * 92 + kx + 1,
                                        [n1o[icc].ap[0], [92, nr], [1, 88]])
                                    lhs = wt["w_dn2"][:, tap * 2 + icc,
                                                      ocg * 128:(ocg + 1) * 128]
                                    nc.tensor.matmul(ps[:], lhs, rhs,
                                                     start=(gi == 0),
                                                     stop=(gi == 17))
                                    gi += 1
                        ev = wpool.tile([128, nr, 88], dt.bfloat16, tag=f"evn2{s}")
                        nc.scalar.activation(ev[:], ps[:], RELU,
                                             bias=ct["t_dn2"][:, ocg:ocg + 1],
                                             scale=ct["s_dn2"][:, ocg:ocg + 1])
                        nc.vector.tensor_copy(n2o[ocg][:, r0:r0 + nr, :], ev[:])

                if s == 0 and debug:
                    nc.sync.dma_start(out=DBG["dbg_n2o"], in_=n2o[0][:])
                if stages < 6:
                    continue
                # ======== dn3 + softmax + feat ========
                npix = nout * FW
                feat_sb[s] = bpool.tile([128, (npix + 127) // 128, CIMG],
                                        dt.bfloat16, tag=f"feat{s}", name=f"feat_sb{s}")
                depth_sb[s] = bpool.tile([128, (npix + 127) // 128, DD],
                                         dt.float32, tag=f"depth{s}", name=f"depth_sb{s}")
                n2f = [t.rearrange("p a b -> p (a b)") for t in n2o]
                for pc in range((npix + 127) // 128):
                    m = min(128, npix - pc * 128)
                    ps = ppool.tile([m, 139], dt.float32, tag=f"ps{s}")
                    for icc in range(2):
                        nc.tensor.matmul(ps[:], n2f[icc][:, pc * 128:pc * 128 + m],
                                         wt["w_dn3"][:, icc, :],
                                         start=(icc == 0), stop=(icc == 1))
                    # add bias via vector then softmax over first 59
                    lg = wpool.tile([m, 139], dt.float32, tag=f"lg{s}")
                    nc.vector.tensor_tensor(out=lg[:], in0=ps[:],
                                            in1=ct["b_dn3"][0:m],
                                            op=mybir.AluOpType.add)
                    mx = wpool.tile([m, 1], dt.float32, tag=f"mx{s}")
                    nc.vector.reduce_max(mx[:], lg[:, 0:DD],
                                         axis=mybir.AxisListType.X, negate=True)
                    ex = wpool.tile([m, DD], dt.float32, tag=f"ex{s}")
                    nc.scalar.activation(ex[:], lg[:, 0:DD],
                                         mybir.ActivationFunctionType.Exp,
                                         bias=mx[:, 0:1], scale=1.0)
                    sm = wpool.tile([m, 1], dt.float32, tag=f"sm{s}")
                    nc.vector.reduce_sum(sm[:], ex[:], axis=mybir.AxisListType.X)
                    rc = wpool.tile([m, 1], dt.float32, tag=f"rc{s}")
                    nc.vector.reciprocal(rc[:], sm[:])
                    nc.vector.tensor_scalar(out=depth_sb[s][0:m, pc, :], in0=ex[:],
                                            scalar1=rc[:, 0:1], scalar2=None,
                                            op0=mybir.AluOpType.mult)
                    nc.vector.tensor_copy(feat_sb[s][0:m, pc, :],
                                          lg[:, DD:DD + CIMG])

            # DMA outputs: global pix index = seg-A pix then seg-B pix
            for s, S in (enumerate(SEGS) if stages >= 6 else []):
                npix = S["nout"] * FW
                base = 0 if s == 0 else 16 * FW
                nfull = npix // 128
                dsl = out_depth[base:base + nfull * 128].rearrange(
                    "(a p) d -> p a d", p=128)
                nc.sync.dma_start(out=dsl, in_=depth_sb[s][:, 0:nfull, :])
                fsl = out_feat[base:base + nfull * 128].rearrange(
                    "(a p) d -> p a d", p=128)
                nc.sync.dma_start(out=fsl, in_=feat_sb[s][:, 0:nfull, :])
                rem = npix - nfull * 128
                if rem:
                    nc.sync.dma_start(
                        out=out_depth[base + nfull * 128:base + npix],
                        in_=depth_sb[s][0:rem, nfull, :])
                    nc.sync.dma_start(
                        out=out_feat[base + nfull * 128:base + npix],
                        in_=feat_sb[s][0:rem, nfull, :])
    nc.compile()
    return nc


# ------------------------------------------------------------ host helpers
def _host_geometry(rots, trans, intr, post_rots, post_trans):
    import jax
    import jax.numpy as jnp
    with jax.default_device(jax.devices("cpu")[0]):
        f32 = jnp.float32
        ds = jnp.arange(1.0, 60.0, 1.0, dtype=f32)
        xs = jnp.linspace(0.0, IW - 1.0, FW, dtype=f32)
        ys = jnp.linspace(0.0, IH - 1.0, FH, dtype=f32)
        dm = jnp.broadcast_to(ds[:, None, None], (DD, FH, FW))
        xm = jnp.broadcast_to(xs[None, None, :], (DD, FH, FW))
        ym = jnp.broadcast_to(ys[None, :, None], (DD, FH, FW))
        fr = jnp.stack([xm, ym, dm], -1)
        pts = fr[None, None] - jnp.asarray(post_trans)[:, :, None, None, None, :]
        pts = jnp.einsum("bnij,bndhwj->bndhwi",
                         jnp.linalg.inv(jnp.asarray(post_rots)), pts)
        pts = jnp.concatenate([pts[..., :2] * pts[..., 2:3], pts[..., 2:3]], -1)
        comb = jnp.einsum("bnij,bnjk->bnik", jnp.asarray(rots),
                          jnp.linalg.inv(jnp.asarray(intr)))
        pts = jnp.einsum("bnij,bndhwj->bndhwi", comb, pts) \
            + jnp.asarray(trans)[:, :, None, None, None, :]
        lo = jnp.array([XY0, XY0, Z0], dtype=f32)
        dxv = jnp.array([DXY, DXY, DZ], dtype=f32)
        g = ((pts - lo) / dxv).astype(jnp.int32).reshape(-1, 3)
        kept = ((g[:, 0] >= 0) & (g[:, 0] < NX) & (g[:, 1] >= 0) & (g[:, 1] < NX)
                & (g[:, 2] >= 0) & (g[:, 2] < NZ))
        flat = (g[:, 2] * NX + g[:, 0]) * NX + g[:, 1]
        return np.asarray(flat, np.int64), np.asarray(kept)


def _prep_a_inputs(inputs):
    """Build per-core input maps for launch A."""
    d = np.asarray(inputs["d"], np.float32).reshape(N, IH, IW)
    x_img = np.asarray(inputs["x_img"], np.float32)

    # dt1 folded affine: relu(alpha*d + beta), alpha = s*w, beta = s*b + t
    a1 = (inputs["dt1_s"] * inputs["dt1_w"][:, 0, 0, 0]).astype(np.float32)
    b1 = (inputs["dt1_s"] * inputs["dt1_b"] + inputs["dt1_t"]).astype(np.float32)
    cab = np.arange(128)
    dt1_alpha = a1[cab // 16][:, None]
    dt1_beta = b1[cab // 16][:, None]

    def wprep_dt2():
        w = np.asarray(inputs["dt2_w"], np.float32)      # [32,8,5,5]
        out = np.zeros((4, 128, 32), np.float32)
        for ky in range(5):
            for kx in range(5):
                a, dky = ky % 4, ky // 4
                bph, dmx = (kx + 2) % 4, (kx + 2) // 4
                g = dky * 2 + dmx
                rows = (np.arange(8)) * 16 + a * 4 + bph
                out[g, rows, :] = w[:, :, ky, kx].T
        return out.astype(bf16)

    def wprep_dt3():
        w = np.asarray(inputs["dt3_w"], np.float32)      # [64,32,5,5]
        out = np.zeros((9, 128, 64), np.float32)
        for ky in range(5):
            for kx in range(5):
                a, dky = ky % 2, ky // 2
                bph, dmx = kx % 2, (kx + 2) // 2 - 1
                g = dky * 3 + dmx
                rows = (a * 2 + bph) * 32 + np.arange(32)
                out[g, rows, :] = w[:, :, ky, kx].T
        return out.astype(bf16)

    def wprep_3x3(w, icc_sizes):
        O, I = w.shape[0], w.shape[1]
        nic = len(icc_sizes)
        out = np.zeros((9, nic, 128, O), np.float32)
        for ky in range(3):
            for kx in range(3):
                tap = ky * 3 + kx
                ic0 = 0
                for icc, sz in enumerate(icc_sizes):
                    out[tap, icc, 0:sz, :] = w[:, ic0:ic0 + sz, ky, kx].T
                    ic0 += sz
        return out.astype(bf16)

    # NOTE: dn1 input concat order is [dt3(64) | x_img(256)] in the reference;
    # our matmul chunks are (x0:128, x1:128, dt3:64) -> weight cols must match:
    w_dn1_full = np.asarray(inputs["dn1_w"], np.float32)
    w_dn1 = np.zeros((9, 3, 128, 256), np.float32)
    for ky in range(3):
        for kx in range(3):
            tap = ky * 3 + kx
            w_dn1[tap, 0, :, :] = w_dn1_full[:, 64:192, ky, kx].T
            w_dn1[tap, 1, :, :] = w_dn1_full[:, 192:320, ky, kx].T
            w_dn1[tap, 2, 0:64, :] = w_dn1_full[:, 0:64, ky, kx].T
    w_dn1 = w_dn1.astype(bf16)
    w_dn2 = wprep_3x3(np.asarray(inputs["dn2_w"], np.float32), [128, 128])
    w_dn3 = np.asarray(inputs["dn3_w"], np.float32)[:, :, 0, 0]  # [139, 256]
    w_dn3p = np.zeros((2, 128, 139), np.float32)
    w_dn3p[0] = w_dn3[:, 0:128].T
    w_dn3p[1] = w_dn3[:, 128:256].T

    def fold_bias(b, s, t):
        # conv bias b then bn scale/shift: relu(s*(x+b) + t) = relu(s*x + (s*b+t))
        return np.asarray(s, np.float32), np.asarray(s * b + t, np.float32)

    s2, t2 = fold_bias(inputs["dt2_b"], inputs["dt2_s"], inputs["dt2_t"])
    s3, t3 = fold_bias(inputs["dt3_b"], inputs["dt3_s"], inputs["dt3_t"])
    sn1, tn1 = fold_bias(inputs["dn1_b"], inputs["dn1_s"], inputs["dn1_t"])
    sn2, tn2 = fold_bias(inputs["dn2_b"], inputs["dn2_s"], inputs["dn2_t"])
    b_dn3 = np.broadcast_to(np.asarray(inputs["dn3_b"], np.float32)[None, :],
                            (128, 139)).copy()

    consts = np.zeros((128, 153), np.float32)
    consts[:, 0] = dt1_alpha[:, 0]
    consts[:, 1] = dt1_beta[:, 0]
    consts[:, 2] = np.tile(s2, 4)
    consts[:, 3] = np.tile(t2, 4)
    consts[:, 4] = np.tile(s3, 2)
    consts[:, 5] = np.tile(t3, 2)
    consts[:, 6:8] = sn1.reshape(2, 128).T
    consts[:, 8:10] = tn1.reshape(2, 128).T
    consts[:, 10:12] = sn2.reshape(2, 128).T
    consts[:, 12:14] = tn2.reshape(2, 128).T
    consts[:, 14:153] = b_dn3
    shared = dict(
        consts=consts,
        w_dt2=wprep_dt2(), w_dt3=wprep_dt3(), w_dn1=w_dn1, w_dn2=w_dn2,
        w_dn3=w_dn3p.astype(bf16),
    )

    maps = []
    for c in range(NCORES):
        m = dict(shared)
        for s, (cam, h0) in enumerate([SEG_A[c], SEG_B[c]]):
            S = SEGS[s]
            d0 = 8 * h0 - 34
            dseg = np.zeros((S["nd"], 712), np.float32)
            lo, hi = max(0, d0), min(IH, d0 + S["nd"])
            if hi > lo:
                dseg[lo - d0:hi - d0, 4:708] = d[cam, lo:hi]
            nq = S["nq"]
            ph = dseg.reshape(nq, 4, 178, 4)[:, :, :177, :]     # ry a rx b
            ph = ph.transpose(1, 3, 0, 2)                        # a b ry rx
            m[f"dph{s}"] = np.broadcast_to(
                ph[None], (8, 4, 4, nq, 177)).reshape(128, nq, 177).astype(bf16)
            # dmask: partition (c8,a,b) x ry -> valid(4ry+a)
            ry = np.arange(S["nd"] // 4)
            rows = 4 * ry[None, :] + (cab[:, None] // 4) % 4
            dmask = (((rows + d0) >= 0) & ((rows + d0) < IH))
            q0, t0, r0 = 2 * h0 - 8, h0 - 3, h0 - 1
            qr = np.arange(S["nt2"]) + q0
            m2m = np.broadcast_to(((qr >= 0) & (qr < 64))[None, :],
                                  (128, S["nt2"]))
            tr = np.arange(S["nt3"]) + t0
            m3m = np.broadcast_to(((tr >= 0) & (tr < FH))[None, :],
                                  (128, S["nt3"]))
            rr = np.arange(S["nn1"]) + r0
            mn1m = np.broadcast_to(((rr >= 0) & (rr < FH))[None, :],
                                   (128, S["nn1"]))
            m[f"masks{s}"] = np.concatenate(
                [dmask, m2m, m3m, mn1m], axis=1).astype(bf16)
            xseg = np.zeros((CIN, S["nt3"], FW), np.float32)
            lo2, hi2 = max(0, t0), min(FH, t0 + S["nt3"])
            if hi2 > lo2:
                xseg[:, lo2 - t0:hi2 - t0, :] = x_img[cam, :, lo2:hi2, :]
            m[f"xseg{s}"] = xseg.astype(bf16)
        maps.append(m)
    return maps


# ---------------------------------------------------------------- launch B
def build_launch_b(NCH, NV):
    """Per chunk: one [128pix x 80ch] stationary feat tile x host-built
    [128pix x NV voxel-slot] depth-weight matrix -> [80, NV] window sums."""
    nc = bacc.Bacc("TRN2", target_bir_lowering=False, debug=False,
                   num_devices=NCORES)
    wmat = nc.dram_tensor("wmat", [128, NCH, NV], dt.bfloat16,
                          kind="ExternalInput").ap()
    feats = nc.dram_tensor("feats", [128, NCH, CIMG], dt.bfloat16,
                           kind="ExternalInput").ap()
    owin = nc.dram_tensor("owin", [CIMG, NCH, NV], dt.bfloat16,
                          kind="ExternalOutput").ap()
    with tile.TileContext(nc) as tc:
        with tc.tile_pool(name="const", bufs=1) as cpool, \
             tc.tile_pool(name="io", bufs=4) as iop, \
             tc.tile_pool(name="ps", bufs=4, space="PSUM") as pp:
            ft = cpool.tile([128, NCH, CIMG], dt.bfloat16, name="ft")
            nc.sync.dma_start(out=ft[:], in_=feats)
            for k in range(NCH):
                wt = iop.tile([128, NV], dt.bfloat16, tag="wt", name="wt")
                nc.sync.dma_start(out=wt[:], in_=wmat[:, k, :])
                ps = pp.tile([CIMG, NV], dt.float32, tag="ps", name="ps")
                nc.tensor.matmul(ps[:], ft[:, k, :], wt[:],
                                 start=True, stop=True)
                ot = iop.tile([CIMG, NV], dt.bfloat16, tag="ot", name="ot")
                nc.scalar.activation(ot[:], ps[:],
                                     mybir.ActivationFunctionType.Copy)
                nc.sync.dma_start(out=owin[:, k, :], in_=ot[:])
    nc.compile()
    return nc


# ---------------------------------------------------------------- launch C
C_OUT_ROWS = 23              # ds2-out rows per core (8*23 = 184 >= 180)


def build_launch_c():
    nc = bacc.Bacc("TRN2", target_bir_lowering=False, debug=False,
                   num_devices=NCORES)
    NR1 = C_OUT_ROWS + 2                         # ds1-out rows incl halo (25)
    NRP = 2 * NR1 + 1                            # pooled rows needed (51)
    slab = nc.dram_tensor("slab", [CIMG, NRP, 362], dt.bfloat16,
                          kind="ExternalInput").ap()
    m1 = nc.dram_tensor("m1", [128, NR1], dt.bfloat16, kind="ExternalInput").ap()
    wd1 = nc.dram_tensor("wd1", [9, CIMG, CIMG], dt.bfloat16,
                         kind="ExternalInput").ap()
    wd2 = nc.dram_tensor("wd2", [9, CIMG, CIMG], dt.bfloat16,
                         kind="ExternalInput").ap()
    sb1 = nc.dram_tensor("sb1", [CIMG, 2], dt.float32, kind="ExternalInput").ap()
    sb2 = nc.dram_tensor("sb2", [CIMG, 2], dt.float32, kind="ExternalInput").ap()
    yout = nc.dram_tensor("yout", [CIMG, C_OUT_ROWS, 180], dt.float32,
                          kind="ExternalOutput").ap()
    RELU = mybir.ActivationFunctionType.Relu
    with tile.TileContext(nc) as tc:
        with tc.tile_pool(name="const", bufs=1) as cpool,              tc.tile_pool(name="work", bufs=2) as wp,              tc.tile_pool(name="big", bufs=1) as bp,              tc.tile_pool(name="ps", bufs=3, space="PSUM") as pp:
            slabt = bp.tile([CIMG, NRP, 362], dt.bfloat16, name="slabt")
            for rr in range(0, NRP, 13):
                nrr = min(13, NRP - rr)
                nc.sync.dma_start(out=slabt[:, rr:rr + nrr, :],
                                  in_=slab[:, rr:rr + nrr, :])
            w1 = cpool.tile([CIMG, 9, CIMG], dt.bfloat16, name="w1")
            nc.sync.dma_start(out=w1[:], in_=wd1.rearrange("t p o -> p t o"))
            w2 = cpool.tile([CIMG, 9, CIMG], dt.bfloat16, name="w2")
            nc.sync.dma_start(out=w2[:], in_=wd2.rearrange("t p o -> p t o"))
            sb1t = cpool.tile([CIMG, 2], dt.float32, name="sb1t")
            nc.sync.dma_start(out=sb1t[:], in_=sb1)
            sb2t = cpool.tile([CIMG, 2], dt.float32, name="sb2t")
            nc.sync.dma_start(out=sb2t[:], in_=sb2)
            m1t = wp.tile([128, NR1], dt.bfloat16, name="m1t")
            nc.sync.dma_start(out=m1t[:], in_=m1)
            h1 = bp.tile([CIMG, NR1, 182], dt.bfloat16, name="h1")
            nc.vector.memset(h1[:, :, 0:1], 0.0)
            nc.vector.memset(h1[:, :, 181:182], 0.0)
            # ds1: stride-2 3x3; out row t reads slab rows 2t..2t+2 (slab row 0
            # = pooled row 2o0-3, so out row t (global o0-1+t) reads
            # 2(o0-1+t)-1..+1 - (2o0-3) = 2t..2t+2); col c reads 2c..2c+2
            RP = 2
            for t0 in range(0, NR1, RP):
                nr = min(RP, NR1 - t0)
                ps = pp.tile([CIMG, nr, 180], dt.float32, tag="ps1", name="ps")
                gi = 0
                for ky in range(3):
                    for kx in range(3):
                        rhs = bass.AP(slabt.tensor,
                                      slabt.offset + (2 * t0 + ky) * 362 + kx,
                                      [slabt.ap[0], [2 * 362, nr], [2, 180]])
                        nc.tensor.matmul(ps[:], w1[:, ky * 3 + kx, :], rhs,
                                         start=(gi == 0), stop=(gi == 8))
                        gi += 1
                ev = wp.tile([CIMG, nr, 180], dt.bfloat16, tag="ev", name="ev")
                nc.scalar.activation(ev[:], ps[:], RELU, bias=sb1t[:, 1:2],
                                     scale=sb1t[:, 0:1])
                mbb = bass.AP(m1t.tensor, m1t.offset + t0,
                              [[m1t.ap[0][0], CIMG], [1, nr], [0, 180]])
                nc.vector.tensor_tensor(out=h1[:, t0:t0 + nr, 1:181],
                                        in0=ev[:], in1=mbb,
                                        op=mybir.AluOpType.mult)
            # ds2: 3x3 pad 1: out row o reads h1 rows o..o+2, col c: c..c+2
            yo = bp.tile([CIMG, C_OUT_ROWS, 180], dt.float32, name="yo")
            for o0 in range(0, C_OUT_ROWS, RP):
                nr = min(RP, C_OUT_ROWS - o0)
                ps = pp.tile([CIMG, nr, 180], dt.float32, tag="ps2", name="ps")
                gi = 0
                for ky in range(3):
                    for kx in range(3):
                        rhs = bass.AP(h1.tensor,
                                      h1.offset + (o0 + ky) * 182 + kx,
                                      [h1.ap[0], [182, nr], [1, 180]])
                        nc.tensor.matmul(ps[:], w2[:, ky * 3 + kx, :], rhs,
                                         start=(gi == 0), stop=(gi == 8))
                        gi += 1
                nc.scalar.activation(yo[:, o0:o0 + nr, :], ps[:], RELU,
                                     bias=sb2t[:, 1:2], scale=sb2t[:, 0:1])
                nc.sync.dma_start(out=yout[:, o0:o0 + nr, :],
                                  in_=yo[:, o0:o0 + nr, :])
    nc.compile()
    return nc


_CACHE = {}


def run_launch_a(inputs):
    if "A" not in _CACHE:
        _CACHE["A"] = build_launch_a()
    nc = _CACHE["A"]
    maps = _prep_a_inputs(inputs)
    res = run_bass_kernel_spmd(nc, maps, list(range(NCORES)))
    depth = np.zeros((NPIX, DD), np.float32)
    feat = np.zeros((NPIX, CIMG), np.float32)
    for c in range(NCORES):
        r = res.results[c]
        for s, (cam, h0) in enumerate([SEG_A[c], SEG_B[c]]):
            S = SEGS[s]
            npix = S["nout"] * FW
            base = (cam * FH + h0) * FW
            off = 0 if s == 0 else 16 * FW
            depth[base:base + npix] = r["out_depth"][off:off + npix]
            feat[base:base + npix] = r["out_feat"][off:off + npix].astype(np.float32)
    return depth, feat


def _build_chunks(flat, kept, depth_rows):
    """Group points by (camera, column-block); per group build the
    [pix, voxel-slot] depth-weight matrix over the group's voxel union.
    Splits column blocks whose union exceeds the PSUM window (512)."""
    fl = flat.reshape(N, DD, FH, FW)
    kp = kept.reshape(N, DD, FH, FW)
    chunks = []                      # (pix_ids, Wdense[npix, nv], vox_ids)

    def add_group(n, w0, w1):
        nw = w1 - w0
        f = fl[n, :, :, w0:w1]                       # [DD, FH, nw]
        k = kp[n, :, :, w0:w1]
        vids = np.unique(f[k])
        if len(vids) > 512 and nw > 1:
            mid = w0 + nw // 2
            add_group(n, w0, mid)
            add_group(n, mid, w1)
            return
        nv = max(len(vids), 1)
        # pixel local idx = (w - w0) * FH + h; point (d, h, w)
        slot = np.searchsorted(vids, f[k]) if len(vids) else np.zeros(0, np.int64)
        dd, hh, ww = np.nonzero(k)
        pix_loc = ww * FH + hh
        pixcol = n * FH * FW + hh * FW + (ww + w0)
        dep = depth_rows[pixcol, dd]
        Wd = np.bincount(pix_loc * nv + slot, weights=dep,
                         minlength=nw * FH * nv).reshape(nw * FH, nv)
        pix_ids = (n * FH * FW + np.arange(FH)[None, :] * FW
                   + (w0 + np.arange(nw))[:, None]).reshape(-1)
        chunks.append((pix_ids, Wd, vids))

    for n in range(N):
        for w0 in range(0, FW, 4):
            add_group(n, w0, w0 + 4)
    return chunks


def _prep_b_inputs(chunks, featflat_bf):
    """Balance chunks across cores by window size; build per-core maps."""
    order = sorted(range(len(chunks)), key=lambda i: -chunks[i][1].shape[1])
    load = np.zeros(NCORES, np.int64)
    per_core = [[] for _ in range(NCORES)]
    for i in order:
        c = int(np.argmin(load))
        per_core[c].append(i)
        load[c] += chunks[i][1].shape[1]
    NCH = max(len(p) for p in per_core)
    NV = max(ch[1].shape[1] for ch in chunks)
    NV = (NV + 63) // 64 * 64
    maps, scatter = [], []
    for c in range(NCORES):
        wm = np.zeros((128, NCH, NV), bf16)
        ft = np.zeros((128, NCH, CIMG), bf16)
        sc = []
        for k, i in enumerate(per_core[c]):
            pix_ids, Wd, vids = chunks[i]
            npix, nv = Wd.shape
            wm[0:npix, k, 0:nv] = Wd
            ft[0:npix, k, :] = featflat_bf[pix_ids]
            sc.append((k, vids))
        maps.append(dict(wmat=wm, feats=ft))
        scatter.append(sc)
    return maps, scatter, NCH, NV


def _prep_c_inputs(inputs, pooled_t):
    """pooled_t: [CIMG, 360, 360] f32 -> per-core slabs + masks + weights."""
    NR1 = C_OUT_ROWS + 2
    NRP = 2 * NR1 + 1
    w1 = np.asarray(inputs["ds1_w"], np.float32)
    w2 = np.asarray(inputs["ds2_w"], np.float32)
    wd1 = np.stack([w1[:, :, ky, kx].T for ky in range(3) for kx in range(3)])
    wd2 = np.stack([w2[:, :, ky, kx].T for ky in range(3) for kx in range(3)])
    sb1 = np.stack([np.asarray(inputs["ds1_s"], np.float32),
                    np.asarray(inputs["ds1_t"], np.float32)], 1)
    sb2 = np.stack([np.asarray(inputs["ds2_s"], np.float32),
                    np.asarray(inputs["ds2_t"], np.float32)], 1)
    shared = dict(wd1=wd1.astype(bf16), wd2=wd2.astype(bf16), sb1=sb1, sb2=sb2)
    maps = []
    pt_bf = pooled_t.astype(bf16)
    for c in range(NCORES):
        o0g = C_OUT_ROWS * c
        p0 = 2 * o0g - 3
        slab = np.zeros((CIMG, NRP, 362), bf16)
        lo, hi = max(0, p0), min(NX, p0 + NRP)
        if hi > lo:
            slab[:, lo - p0:hi - p0, 1:361] = pt_bf[:, lo:hi, :]
        t1g = np.arange(NR1) + (o0g - 1)
        m1 = np.broadcast_to(((t1g >= 0) & (t1g < 180))[None, :],
                             (128, NR1)).astype(bf16)
        maps.append(dict(shared, slab=slab, m1=np.ascontiguousarray(m1)))
    return maps


def kernel(**inputs):
    inputs = {k: np.asarray(v) for k, v in inputs.items()}
    flat, kept = _host_geometry(inputs["cam2lidar_rots"],
                                inputs["cam2lidar_trans"], inputs["intrins"],
                                inputs["post_rots"], inputs["post_trans"])
    depth_rows, feat_rows = run_launch_a(inputs)
    featflat_bf = feat_rows.astype(bf16)

    chunks = _build_chunks(flat, kept, depth_rows)
    bmaps, scatter, NCH, NV = _prep_b_inputs(chunks, featflat_bf)
    key = ("B", NCH, NV)
    if key not in _CACHE:
        _CACHE[key] = build_launch_b(NCH, NV)
    res_b = run_bass_kernel_spmd(_CACHE[key], bmaps, list(range(NCORES)))

    allvox = np.concatenate([vids for c in range(NCORES)
                             for _, vids in scatter[c]])
    allval = np.concatenate(
        [res_b.results[c]["owin"][:, k, 0:len(vids)].T.astype(np.float32)
         for c in range(NCORES) for k, vids in scatter[c]])
    o = np.argsort(allvox, kind="stable")
    allvox, allval = allvox[o], allval[o]
    starts = np.flatnonzero(np.r_[True, allvox[1:] != allvox[:-1]])
    pooled = np.zeros((NX * NX, CIMG), np.float32)
    pooled[allvox[starts]] = np.add.reduceat(allval, starts, axis=0)
    pooled_t = np.ascontiguousarray(
        pooled.reshape(NX, NX, CIMG).transpose(2, 0, 1))

    if "C" not in _CACHE:
        _CACHE["C"] = build_launch_c()
    cmaps = _prep_c_inputs(inputs, pooled_t)
    res_c = run_bass_kernel_spmd(_CACHE["C"], cmaps, list(range(NCORES)))
    out = np.zeros((1, CIMG, 180, 180), np.float32)
    for c in range(NCORES):
        o0g = C_OUT_ROWS * c
        nr = min(C_OUT_ROWS, 180 - o0g)
        if nr > 0:
            out[0, :, o0g:o0g + nr, :] = res_c.results[c]["yout"][:, 0:nr, :]
    return out



# revision 7
# speedup vs baseline: 1.4949x; 1.0469x over previous
"""DepthLSSTransform Trainium kernel: 3 SPMD launches over 8 NeuronCores.

Launch A: per-camera conv pipeline (dtransform + depthnet + softmax) on
          24-row bands (one 16-row + one 8-row segment per core).
Launch B: bev_pool segment-sum via one-hot matmuls over a host-built
          virtual-window schedule (sorted-by-voxel points).
Launch C: BEV downsample convs, spatially sharded.
Host: geometry/voxel indices, scheduling, gathers, folds (orchestration).
"""
import numpy as np
import ml_dtypes

import concourse.bass as bass
import concourse.tile as tile
from concourse import bacc, mybir
from concourse.bass_utils import run_bass_kernel_spmd

dt = mybir.dt
bf16 = ml_dtypes.bfloat16

# ---- problem constants (hardcoded per contract) ----
B, N = 1, 6
CIN, CIMG, DD = 256, 80, 59
FH, FW, IH, IW = 32, 88, 256, 704
XY0, DXY, NX = -54.0, 0.3, 360
Z0, DZ, NZ = -10.0, 20.0, 1
NPTS = N * DD * FH * FW
NPIX = N * FH * FW
NCORES = 8
QV = 4                      # chunks of 128 points per virtual window

# per-core segments: (camera, h0) for seg A (16 rows) and seg B (8 rows)
SEG_A = [(0, 0), (1, 0), (1, 16), (2, 16), (3, 0), (4, 0), (4, 16), (5, 16)]
SEG_B = [(0, 16), (0, 24), (2, 0), (2, 8), (3, 16), (3, 24), (5, 0), (5, 8)]
# band pixel ranges in global row order (row = n*32 + h)
ROWS_OF_CORE = [[(SEG_A[c][0] * FH + SEG_A[c][1] + r) for r in range(16)] +
                [(SEG_B[c][0] * FH + SEG_B[c][1] + r) for r in range(8)]
                for c in range(NCORES)]

# segment geometry: rows16 segment: d rows [8h0-34, 8h0+158) (192), dt2 out
# rows [2h0-8, 2h0+39) (47), dt3 [h0-3, h0+19) (22), dn1 [h0-1, h0+17) (18)
SEGS = [dict(nout=16, nd=192, nq=48, nt2=47, nt3=22, nn1=18),
        dict(nout=8, nd=128, nq=32, nt2=31, nt3=14, nn1=10)]


def _seg_ranges(h0, S):
    return dict(d0=8 * h0 - 34, q0=2 * h0 - 8, t0=h0 - 3, r0=h0 - 1, o0=h0)


# ---------------------------------------------------------------- launch A
def build_launch_a(debug=False, psum_bufs=3, work_bufs=3, stages=9):
    nc = bacc.Bacc("TRN2", target_bir_lowering=False, debug=False,
                   num_devices=NCORES)
    AP = {}

    def inp(name, shape, dtype=dt.bfloat16):
        AP[name] = nc.dram_tensor(name, shape, dtype, kind="ExternalInput").ap()
        return AP[name]

    # per segment inputs (s = 0: 16-row, 1: 8-row)
    for s, S in enumerate(SEGS):
        inp(f"dph{s}", [128, S["nq"], 177])
        inp(f"masks{s}", [128, S["nq"] + S["nt2"] + S["nt3"] + S["nn1"]])
        inp(f"xseg{s}", [CIN, S["nt3"], FW])            # x_img slice (zeroed oob)
    # packed f32 constants: [alpha, beta, s_dt2, t_dt2, s_dt3, t_dt3,
    #  s_dn1(2), t_dn1(2), s_dn2(2), t_dn2(2), b_dn3(139)] -> [128, 153]
    inp("consts", [128, 153], dt.float32)
    # conv weights (host-prepped layouts)
    inp("w_dt2", [4, 128, 32])                          # groups (dky,dmx)
    inp("w_dt3", [9, 128, 64])
    inp("w_dn1", [9, 3, 128, 256])                      # tap, icchunk(128,128,64pad) -> 256
    inp("w_dn2", [9, 2, 128, 256])
    inp("w_dn3", [2, 128, 139])

    DBG = {}
    dbg_specs = [] if not debug else [("dbg_t1", [128, SEGS[0]["nq"], 177], dt.bfloat16),
                        ("dbg_dt2o", [32, SEGS[0]["nt2"] + 1, 180], dt.bfloat16),
                        ("dbg_dtc", [64, SEGS[0]["nt3"], 92], dt.bfloat16),
                        ("dbg_n1o", [128, SEGS[0]["nn1"], 92], dt.bfloat16),
                        ("dbg_n2o", [128, SEGS[0]["nout"], 88], dt.bfloat16)]
    for nm, sh, dty in dbg_specs:
        DBG[nm] = nc.dram_tensor(nm, sh, dty, kind="ExternalOutput").ap()
    out_depth = nc.dram_tensor("out_depth", [24 * FW, DD], dt.float32,
                               kind="ExternalOutput").ap()
    out_feat = nc.dram_tensor("out_feat", [24 * FW, CIMG], dt.bfloat16,
                              kind="ExternalOutput").ap()

    # HBM scratch
    scr = {}
    for s, S in enumerate(SEGS):
        scr[f"dt2o{s}"] = nc.dram_tensor(f"dt2o{s}", [32, S["nt2"] + 1, 2, 90], dt.bfloat16).ap()

    RELU = mybir.ActivationFunctionType.Relu
    with tile.TileContext(nc) as tc:
        with tc.tile_pool(name="const", bufs=1) as cpool, \
             tc.tile_pool(name="work", bufs=work_bufs) as wpool, \
             tc.tile_pool(name="big", bufs=1) as bpool, \
             tc.tile_pool(name="psum", bufs=psum_bufs, space="PSUM") as ppool:
            # ---- load packed constants in one DMA ----
            cts = cpool.tile([128, 153], dt.float32, name="cts")
            nc.sync.dma_start(out=cts[:], in_=AP["consts"])
            ct = {"dt1_alpha": cts[:, 0:1], "dt1_beta": cts[:, 1:2],
                  "s_dt2": cts[:, 2:3], "t_dt2": cts[:, 3:4],
                  "s_dt3": cts[:, 4:5], "t_dt3": cts[:, 5:6],
                  "s_dn1": cts[:, 6:8], "t_dn1": cts[:, 8:10],
                  "s_dn2": cts[:, 10:12], "t_dn2": cts[:, 12:14],
                  "b_dn3": cts[:, 14:153]}
            wt = {}
            for nm, pat in [("w_dt2", "g p o -> p g o"),
                            ("w_dt3", "g p o -> p g o"),
                            ("w_dn1", "t i p o -> p (t i) o"),
                            ("w_dn2", "t i p o -> p (t i) o"),
                            ("w_dn3", "g p o -> p g o")]:
                sh = list(AP[nm].shape)
                wt[nm] = cpool.tile([sh[-2], int(np.prod(sh[:-2])), sh[-1]],
                                    dt.bfloat16, tag=nm, name=f'wt_{nm}')
                nc.sync.dma_start(out=wt[nm][:], in_=AP[nm].rearrange(pat))

            feat_sb = {}
            depth_sb = {}
            for s, S in enumerate(SEGS):
                nq, nt2, nt3, nn1, nout = S["nq"], S["nt2"], S["nt3"], S["nn1"], S["nout"]
                # ======== dt1 : affine + relu + row-mask on host-phased d ====
                dph = bpool.tile([128, nq, 177], dt.bfloat16, tag=f"dph{s}")
                for qq in range(0, nq, nq // 4):
                    nqq = min(nq // 4, nq - qq)
                    nc.sync.dma_start(out=dph[:, qq:qq + nqq, :],
                                      in_=AP[f"dph{s}"][:, qq:qq + nqq, :])
                t1 = bpool.tile([128, nq, 177], dt.bfloat16, tag=f"t1{s}")
                mall = wpool.tile([128, nq + nt2 + nt3 + nn1], dt.bfloat16,
                                  tag=f"msk{s}", name="mall")
                nc.sync.dma_start(out=mall[:], in_=AP[f"masks{s}"])
                QCH = nq // 4
                for qq in range(0, nq, QCH):
                    nqq = min(QCH, nq - qq)
                    sl = (slice(None), slice(qq, qq + nqq), slice(None))
                    nc.vector.tensor_scalar(out=t1[sl], in0=dph[sl],
                                            scalar1=ct["dt1_alpha"][:, 0:1],
                                            scalar2=ct["dt1_beta"][:, 0:1],
                                            op0=mybir.AluOpType.mult,
                                            op1=mybir.AluOpType.add)
                    nc.vector.tensor_scalar(out=t1[sl], in0=t1[sl], scalar1=0.0,
                                            scalar2=None, op0=mybir.AluOpType.max)
                    mb = bass.AP(mall.tensor, mall.offset + qq,
                                 [mall.ap[0], [1, nqq], [0, 177]])
                    nc.vector.tensor_tensor(out=t1[sl], in0=t1[sl], in1=mb,
                                            op=mybir.AluOpType.mult)
                    nc.vector.memset(t1[:, qq:qq + nqq, 0:1], 0.0)
                if s == 0 and debug:
                    nc.sync.dma_start(out=DBG["dbg_t1"], in_=t1[:])

                if stages < 2:
                    continue
                # ======== dt2 ========
                o2 = bpool.tile([32, nt2 + 1, 180], dt.bfloat16, tag=f"o2{s}")
                nc.vector.memset(o2[:], 0.0)
                m2 = bass.AP(mall.tensor, mall.offset + nq, [mall.ap[0], [1, nt2]])
                RPP2 = 2
                for q0 in range(0, nt2, RPP2):
                    nr = min(RPP2, nt2 - q0)
                    ps = ppool.tile([32, nr, 176], dt.float32, tag=f"ps{s}", name="ps2")
                    gi = 0
                    for dky in range(2):
                        for dmx in range(2):
                            g = dky * 2 + dmx
                            rhs = bass.AP(
                                t1.tensor, t1.offset + (q0 + dky) * 177 + dmx,
                                [t1.ap[0], [177, nr], [1, 176]])
                            nc.tensor.matmul(ps[:], wt["w_dt2"][:, g, :], rhs,
                                             start=(gi == 0), stop=(gi == 3))
                            gi += 1
                    ev = wpool.tile([32, nr, 176], dt.bfloat16, tag=f"ev2{s}")
                    nc.scalar.activation(ev[:], ps[:], RELU,
                                         bias=ct["t_dt2"][0:32, 0:1],
                                         scale=ct["s_dt2"][0:32, 0:1])
                    mbb = bass.AP(m2.tensor, m2.offset + q0,
                                  [[m2.ap[0][0], 32], [1, nr], [0, 176]])
                    # write col c at (c%2)*90 + c//2 + 1  (phase-split layout)
                    o2dst = bass.AP(o2.tensor, o2.offset + q0 * 180 + 1,
                                    [[o2.ap[0][0], 32], [180, nr],
                                     [1, 88], [90, 2]])
                    nc.vector.tensor_tensor(out=o2dst, in0=ev[:], in1=mbb,
                                            op=mybir.AluOpType.mult)
                nc.sync.dma_start(out=scr[f"dt2o{s}"],
                                  in_=o2.rearrange("p q (b x) -> p q b x", b=2))
                if s == 0 and debug:
                    nc.sync.dma_start(out=DBG["dbg_dt2o"], in_=o2[:])

                if stages < 3:
                    continue
                # ======== dt3 ========
                nry3 = nt3 + 2
                ph3 = bpool.tile([128, nry3, 90], dt.bfloat16, tag=f"ph3{s}")
                sd2 = scr[f"dt2o{s}"]
                for a2 in range(2):
                    for b2 in range(2):
                        pap3 = bass.AP(sd2.tensor,
                                       sd2.offset + a2 * 180 + b2 * 90,
                                       [[(nt2 + 1) * 180, 32],
                                        [2 * 180, nry3], [1, 90]])
                        nc.sync.dma_start(
                            out=ph3[(a2 * 2 + b2) * 32:(a2 * 2 + b2 + 1) * 32],
                            in_=pap3)
                # concat input tile: [64 dt3 | pad] plus x_img tiles
                dtc = bpool.tile([64, nt3, 92], dt.bfloat16, tag=f"dtc{s}")
                nc.vector.memset(dtc[:], 0.0)
                m3 = bass.AP(mall.tensor, mall.offset + nq + nt2,
                             [mall.ap[0], [1, nt3]])
                RPP3 = 4
                for t0 in range(0, nt3, RPP3):
                    nr = min(RPP3, nt3 - t0)
                    ps = ppool.tile([64, nr, 88], dt.float32, tag=f"ps{s}")
                    gi = 0
                    for dky in range(3):
                        for dmx in range(3):
                            g = dky * 3 + dmx
                            rhs = bass.AP(ph3.tensor,
                                          ph3.offset + (t0 + dky) * 90 + dmx,
                                          [ph3.ap[0], [90, nr], [1, 88]])
                            nc.tensor.matmul(ps[:], wt["w_dt3"][:, g, :], rhs,
                                             start=(gi == 0), stop=(gi == 8))
                            gi += 1
                    ev = wpool.tile([64, nr, 88], dt.bfloat16, tag=f"ev3{s}")
                    nc.scalar.activation(ev[:], ps[:], RELU,
                                         bias=ct["t_dt3"][0:64, 0:1],
                                         scale=ct["s_dt3"][0:64, 0:1])
                    mbb = bass.AP(m3.tensor, m3.offset + t0,
                                  [m3.ap[0], [1, nr], [0, 88]])
                    nc.vector.tensor_tensor(out=dtc[:, t0:t0 + nr, 2:90],
                                            in0=ev[:], in1=mbb[0:64],
                                            op=mybir.AluOpType.mult)

                if s == 0 and debug:
                    nc.sync.dma_start(out=DBG["dbg_dtc"], in_=dtc[:])
                if stages < 4:
                    continue
                # ======== dn1 ========
                xs = []
                for g in range(2):
                    xt = bpool.tile([128, nt3, 92], dt.bfloat16, tag=f"x{g}_{s}",
                                     name=f"xseg_t{g}")
                    nc.vector.memset(xt[:], 0.0)
                    nc.sync.dma_start(
                        out=xt[:, :, 2:90],
                        in_=AP[f"xseg{s}"][g * 128:(g + 1) * 128])
                    xs.append(xt)
                mn1 = bass.AP(mall.tensor, mall.offset + nq + nt2 + nt3,
                              [mall.ap[0], [1, nn1]])
                n1o = []
                for g in range(2):
                    t = bpool.tile([128, nn1, 92], dt.bfloat16, tag=f"n1o{g}_{s}")
                    nc.vector.memset(t[:], 0.0)
                    n1o.append(t)
                RPP = 5
                for ocg in range(2):
                    for r0 in range(0, nn1, RPP):
                        nr = min(RPP, nn1 - r0)
                        ps = ppool.tile([128, nr, 88], dt.float32, tag=f"ps{s}")
                        gi = 0
                        for ky in range(3):
                            for kx in range(3):
                                tap = ky * 3 + kx
                                for icc, srcT in enumerate((xs[0], xs[1], dtc)):
                                    kk = 128 if icc < 2 else 64
                                    rhs = bass.AP(
                                        srcT.tensor,
                                        srcT.offset + (r0 + ky + 1) * 92 + kx + 1,
                                        [srcT.ap[0], [92, nr], [1, 88]])
                                    lhs = wt["w_dn1"][0:kk, tap * 3 + icc,
                                                      ocg * 128:(ocg + 1) * 128]
                                    nc.tensor.matmul(ps[:], lhs, rhs,
                                                     start=(gi == 0),
                                                     stop=(gi == 26))
                                    gi += 1
                        ev = wpool.tile([128, nr, 88], dt.bfloat16, tag=f"evn1{s}")
                        nc.scalar.activation(ev[:], ps[:], RELU,
                                             bias=ct["t_dn1"][:, ocg:ocg + 1],
                                             scale=ct["s_dn1"][:, ocg:ocg + 1])
                        mbb = bass.AP(mn1.tensor, mn1.offset + r0,
                                      [mn1.ap[0], [1, nr], [0, 88]])
                        nc.vector.tensor_tensor(
                            out=n1o[ocg][:, r0:r0 + nr, 2:90],
                            in0=ev[:], in1=mbb, op=mybir.AluOpType.mult)

                if s == 0 and debug:
                    nc.sync.dma_start(out=DBG["dbg_n1o"], in_=n1o[0][:])
                if stages < 5:
                    continue
                # ======== dn2 ========
                n2o = []
                for g in range(2):
                    n2o.append(bpool.tile([128, nout, 88], dt.bfloat16,
                                          tag=f"n2o{g}_{s}", name=f"n2o{g}"))
                for ocg in range(2):
                    for r0 in range(0, nout, RPP):
                        nr = min(RPP, nout - r0)
                        ps = ppool.tile([128, nr, 88], dt.float32, tag=f"ps{s}")
                        gi = 0
                        for ky in range(3):
                            for kx in range(3):
                                tap = ky * 3 + kx
                                for icc in range(2):
                                    rhs = bass.AP(
                                        n1o[icc].tensor,
                                        n1o[icc].offset + (r0 + ky) * 92 + kx + 1,
                                        [n1o[icc].ap[0], [92, nr], [1, 88]])
                                    lhs = wt["w_dn2"][:, tap * 2 + icc,
                                                      ocg * 128:(ocg + 1) * 128]
                                    nc.tensor.matmul(ps[:], lhs, rhs,
                                                     start=(gi == 0),
                                                     stop=(gi == 17))
                                    gi += 1
                        ev = wpool.tile([128, nr, 88], dt.bfloat16, tag=f"evn2{s}")
                        nc.scalar.activation(ev[:], ps[:], RELU,
                                             bias=ct["t_dn2"][:, ocg:ocg + 1],
                                             scale=ct["s_dn2"][:, ocg:ocg + 1])
                        nc.vector.tensor_copy(n2o[ocg][:, r0:r0 + nr, :], ev[:])

                if s == 0 and debug:
                    nc.sync.dma_start(out=DBG["dbg_n2o"], in_=n2o[0][:])
                if stages < 6:
                    continue
                # ======== dn3 + softmax + feat ========
                npix = nout * FW
                feat_sb[s] = bpool.tile([128, (npix + 127) // 128, CIMG],
                                        dt.bfloat16, tag=f"feat{s}", name=f"feat_sb{s}")
                depth_sb[s] = bpool.tile([128, (npix + 127) // 128, DD],
                                         dt.float32, tag=f"depth{s}", name=f"depth_sb{s}")
                n2f = [t.rearrange("p a b -> p (a b)") for t in n2o]
                for pc in range((npix + 127) // 128):
                    m = min(128, npix - pc * 128)
                    ps = ppool.tile([m, 139], dt.float32, tag=f"ps{s}")
                    for icc in range(2):
                        nc.tensor.matmul(ps[:], n2f[icc][:, pc * 128:pc * 128 + m],
                                         wt["w_dn3"][:, icc, :],
                                         start=(icc == 0), stop=(icc == 1))
                    # add bias via vector then softmax over first 59
                    lg = wpool.tile([m, 139], dt.float32, tag=f"lg{s}")
                    nc.vector.tensor_tensor(out=lg[:], in0=ps[:],
                                            in1=ct["b_dn3"][0:m],
                                            op=mybir.AluOpType.add)
                    mx = wpool.tile([m, 1], dt.float32, tag=f"mx{s}")
                    nc.vector.reduce_max(mx[:], lg[:, 0:DD],
                                         axis=mybir.AxisListType.X, negate=True)
                    ex = wpool.tile([m, DD], dt.float32, tag=f"ex{s}")
                    nc.scalar.activation(ex[:], lg[:, 0:DD],
                                         mybir.ActivationFunctionType.Exp,
                                         bias=mx[:, 0:1], scale=1.0)
                    sm = wpool.tile([m, 1], dt.float32, tag=f"sm{s}")
                    nc.vector.reduce_sum(sm[:], ex[:], axis=mybir.AxisListType.X)
                    rc = wpool.tile([m, 1], dt.float32, tag=f"rc{s}")
                    nc.vector.reciprocal(rc[:], sm[:])
                    nc.vector.tensor_scalar(out=depth_sb[s][0:m, pc, :], in0=ex[:],
                                            scalar1=rc[:, 0:1], scalar2=None,
                                            op0=mybir.AluOpType.mult)
                    nc.vector.tensor_copy(feat_sb[s][0:m, pc, :],
                                          lg[:, DD:DD + CIMG])

            # DMA outputs: global pix index = seg-A pix then seg-B pix
            for s, S in (enumerate(SEGS) if stages >= 6 else []):
                npix = S["nout"] * FW
                base = 0 if s == 0 else 16 * FW
                nfull = npix // 128
                dsl = out_depth[base:base + nfull * 128].rearrange(
                    "(a p) d -> p a d", p=128)
                nc.sync.dma_start(out=dsl, in_=depth_sb[s][:, 0:nfull, :])
                fsl = out_feat[base:base + nfull * 128].rearrange(
                    "(a p) d -> p a d", p=128)
                nc.sync.dma_start(out=fsl, in_=feat_sb[s][:, 0:nfull, :])
                rem = npix - nfull * 128
                if rem:
                    nc.sync.dma_start(
                        out=out_depth[base + nfull * 128:base + npix],
                        in_=depth_sb[s][0:rem, nfull, :])
                    nc.sync.dma_start(
                        out=out_feat[base + nfull * 128:base + npix],
                        in_=feat_sb[s][0:rem, nfull, :])
    nc.compile()
    return nc


# ------------------------------------------------------------ host helpers
def _host_geometry(rots, trans, intr, post_rots, post_trans):
    import jax
    import jax.numpy as jnp
    with jax.default_device(jax.devices("cpu")[0]):
        f32 = jnp.float32
        ds = jnp.arange(1.0, 60.0, 1.0, dtype=f32)
        xs = jnp.linspace(0.0, IW - 1.0, FW, dtype=f32)
        ys = jnp.linspace(0.0, IH - 1.0, FH, dtype=f32)
        dm = jnp.broadcast_to(ds[:, None, None], (DD, FH, FW))
        xm = jnp.broadcast_to(xs[None, None, :], (DD, FH, FW))
        ym = jnp.broadcast_to(ys[None, :, None], (DD, FH, FW))
        fr = jnp.stack([xm, ym, dm], -1)
        pts = fr[None, None] - jnp.asarray(post_trans)[:, :, None, None, None, :]
        pts = jnp.einsum("bnij,bndhwj->bndhwi",
                         jnp.linalg.inv(jnp.asarray(post_rots)), pts)
        pts = jnp.concatenate([pts[..., :2] * pts[..., 2:3], pts[..., 2:3]], -1)
        comb = jnp.einsum("bnij,bnjk->bnik", jnp.asarray(rots),
                          jnp.linalg.inv(jnp.asarray(intr)))
        pts = jnp.einsum("bnij,bndhwj->bndhwi", comb, pts) \
            + jnp.asarray(trans)[:, :, None, None, None, :]
        lo = jnp.array([XY0, XY0, Z0], dtype=f32)
        dxv = jnp.array([DXY, DXY, DZ], dtype=f32)
        g = ((pts - lo) / dxv).astype(jnp.int32).reshape(-1, 3)
        kept = ((g[:, 0] >= 0) & (g[:, 0] < NX) & (g[:, 1] >= 0) & (g[:, 1] < NX)
                & (g[:, 2] >= 0) & (g[:, 2] < NZ))
        flat = (g[:, 2] * NX + g[:, 0]) * NX + g[:, 1]
        return np.asarray(flat, np.int64), np.asarray(kept)


def _prep_a_inputs(inputs):
    """Build per-core input maps for launch A."""
    d = np.asarray(inputs["d"], np.float32).reshape(N, IH, IW)
    x_img = np.asarray(inputs["x_img"], np.float32)

    # dt1 folded affine: relu(alpha*d + beta), alpha = s*w, beta = s*b + t
    a1 = (inputs["dt1_s"] * inputs["dt1_w"][:, 0, 0, 0]).astype(np.float32)
    b1 = (inputs["dt1_s"] * inputs["dt1_b"] + inputs["dt1_t"]).astype(np.float32)
    cab = np.arange(128)
    dt1_alpha = a1[cab // 16][:, None]
    dt1_beta = b1[cab // 16][:, None]

    def wprep_dt2():
        w = np.asarray(inputs["dt2_w"], np.float32)      # [32,8,5,5]
        out = np.zeros((4, 128, 32), np.float32)
        for ky in range(5):
            for kx in range(5):
                a, dky = ky % 4, ky // 4
                bph, dmx = (kx + 2) % 4, (kx + 2) // 4
                g = dky * 2 + dmx
                rows = (np.arange(8)) * 16 + a * 4 + bph
                out[g, rows, :] = w[:, :, ky, kx].T
        return out.astype(bf16)

    def wprep_dt3():
        w = np.asarray(inputs["dt3_w"], np.float32)      # [64,32,5,5]
        out = np.zeros((9, 128, 64), np.float32)
        for ky in range(5):
            for kx in range(5):
                a, dky = ky % 2, ky // 2
                bph, dmx = kx % 2, (kx + 2) // 2 - 1
                g = dky * 3 + dmx
                rows = (a * 2 + bph) * 32 + np.arange(32)
                out[g, rows, :] = w[:, :, ky, kx].T
        return out.astype(bf16)

    def wprep_3x3(w, icc_sizes):
        O, I = w.shape[0], w.shape[1]
        nic = len(icc_sizes)
        out = np.zeros((9, nic, 128, O), np.float32)
        for ky in range(3):
            for kx in range(3):
                tap = ky * 3 + kx
                ic0 = 0
                for icc, sz in enumerate(icc_sizes):
                    out[tap, icc, 0:sz, :] = w[:, ic0:ic0 + sz, ky, kx].T
                    ic0 += sz
        return out.astype(bf16)

    # NOTE: dn1 input concat order is [dt3(64) | x_img(256)] in the reference;
    # our matmul chunks are (x0:128, x1:128, dt3:64) -> weight cols must match:
    w_dn1_full = np.asarray(inputs["dn1_w"], np.float32)
    w_dn1 = np.zeros((9, 3, 128, 256), np.float32)
    for ky in range(3):
        for kx in range(3):
            tap = ky * 3 + kx
            w_dn1[tap, 0, :, :] = w_dn1_full[:, 64:192, ky, kx].T
            w_dn1[tap, 1, :, :] = w_dn1_full[:, 192:320, ky, kx].T
            w_dn1[tap, 2, 0:64, :] = w_dn1_full[:, 0:64, ky, kx].T
    w_dn1 = w_dn1.astype(bf16)
    w_dn2 = wprep_3x3(np.asarray(inputs["dn2_w"], np.float32), [128, 128])
    w_dn3 = np.asarray(inputs["dn3_w"], np.float32)[:, :, 0, 0]  # [139, 256]
    w_dn3p = np.zeros((2, 128, 139), np.float32)
    w_dn3p[0] = w_dn3[:, 0:128].T
    w_dn3p[1] = w_dn3[:, 128:256].T

    def fold_bias(b, s, t):
        # conv bias b then bn scale/shift: relu(s*(x+b) + t) = relu(s*x + (s*b+t))
        return np.asarray(s, np.float32), np.asarray(s * b + t, np.float32)

    s2, t2 = fold_bias(inputs["dt2_b"], inputs["dt2_s"], inputs["dt2_t"])
    s3, t3 = fold_bias(inputs["dt3_b"], inputs["dt3_s"], inputs["dt3_t"])
    sn1, tn1 = fold_bias(inputs["dn1_b"], inputs["dn1_s"], inputs["dn1_t"])
    sn2, tn2 = fold_bias(inputs["dn2_b"], inputs["dn2_s"], inputs["dn2_t"])
    b_dn3 = np.broadcast_to(np.asarray(inputs["dn3_b"], np.float32)[None, :],
                            (128, 139)).copy()

    consts = np.zeros((128, 153), np.float32)
    consts[:, 0] = dt1_alpha[:, 0]
    consts[:, 1] = dt1_beta[:, 0]
    consts[:, 2] = np.tile(s2, 4)
    consts[:, 3] = np.tile(t2, 4)
    consts[:, 4] = np.tile(s3, 2)
    consts[:, 5] = np.tile(t3, 2)
    consts[:, 6:8] = sn1.reshape(2, 128).T
    consts[:, 8:10] = tn1.reshape(2, 128).T
    consts[:, 10:12] = sn2.reshape(2, 128).T
    consts[:, 12:14] = tn2.reshape(2, 128).T
    consts[:, 14:153] = b_dn3
    shared = dict(
        consts=consts,
        w_dt2=wprep_dt2(), w_dt3=wprep_dt3(), w_dn1=w_dn1, w_dn2=w_dn2,
        w_dn3=w_dn3p.astype(bf16),
    )

    maps = []
    for c in range(NCORES):
        m = dict(shared)
        for s, (cam, h0) in enumerate([SEG_A[c], SEG_B[c]]):
            S = SEGS[s]
            d0 = 8 * h0 - 34
            dseg = np.zeros((S["nd"], 712), np.float32)
            lo, hi = max(0, d0), min(IH, d0 + S["nd"])
            if hi > lo:
                dseg[lo - d0:hi - d0, 4:708] = d[cam, lo:hi]
            nq = S["nq"]
            ph = dseg.reshape(nq, 4, 178, 4)[:, :, :177, :]     # ry a rx b
            ph = ph.transpose(1, 3, 0, 2)                        # a b ry rx
            m[f"dph{s}"] = np.broadcast_to(
                ph[None], (8, 4, 4, nq, 177)).reshape(128, nq, 177).astype(bf16)
            # dmask: partition (c8,a,b) x ry -> valid(4ry+a)
            ry = np.arange(S["nd"] // 4)
            rows = 4 * ry[None, :] + (cab[:, None] // 4) % 4
            dmask = (((rows + d0) >= 0) & ((rows + d0) < IH))
            q0, t0, r0 = 2 * h0 - 8, h0 - 3, h0 - 1
            qr = np.arange(S["nt2"]) + q0
            m2m = np.broadcast_to(((qr >= 0) & (qr < 64))[None, :],
                                  (128, S["nt2"]))
            tr = np.arange(S["nt3"]) + t0
            m3m = np.broadcast_to(((tr >= 0) & (tr < FH))[None, :],
                                  (128, S["nt3"]))
            rr = np.arange(S["nn1"]) + r0
            mn1m = np.broadcast_to(((rr >= 0) & (rr < FH))[None, :],
                                   (128, S["nn1"]))
            m[f"masks{s}"] = np.concatenate(
                [dmask, m2m, m3m, mn1m], axis=1).astype(bf16)
            xseg = np.zeros((CIN, S["nt3"], FW), np.float32)
            lo2, hi2 = max(0, t0), min(FH, t0 + S["nt3"])
            if hi2 > lo2:
                xseg[:, lo2 - t0:hi2 - t0, :] = x_img[cam, :, lo2:hi2, :]
            m[f"xseg{s}"] = xseg.astype(bf16)
        maps.append(m)
    return maps


# ---------------------------------------------------------------- launch B
def build_launch_b(sizes):
    """Per chunk k: [128pix x 80ch] stationary feat tile x host-built
    [128pix x sizes[k] voxel-slot] depth-weight matrix -> [80, nv] window
    sums. W and out use packed (variable-size) layouts; W loads in a few
    batched DMAs, out in one."""
    nc = bacc.Bacc("TRN2", target_bir_lowering=False, debug=False,
                   num_devices=NCORES)
    NCH = len(sizes)
    offs = np.concatenate([[0], np.cumsum(sizes)]).astype(int)
    S = int(offs[-1])
    wmat = nc.dram_tensor("wmat", [128, S], dt.bfloat16,
                          kind="ExternalInput").ap()
    feats = nc.dram_tensor("feats", [128, NCH, CIMG], dt.bfloat16,
                           kind="ExternalInput").ap()
    owin = nc.dram_tensor("owin", [CIMG, S], dt.bfloat16,
                          kind="ExternalOutput").ap()
    NB = 4                                   # W DMA batches
    bnd = [int(round(NCH * i / NB)) for i in range(NB + 1)]
    with tile.TileContext(nc) as tc:
        with tc.tile_pool(name="const", bufs=1) as cpool, \
             tc.tile_pool(name="ps", bufs=4, space="PSUM") as pp:
            ft = cpool.tile([128, NCH, CIMG], dt.bfloat16, name="ft")
            nc.sync.dma_start(out=ft[:], in_=feats)
            wt = cpool.tile([128, S], dt.bfloat16, name="wt")
            for b in range(NB):
                lo, hi = offs[bnd[b]], offs[bnd[b + 1]]
                if hi > lo:
                    nc.sync.dma_start(out=wt[:, lo:hi], in_=wmat[:, lo:hi])
            ot = cpool.tile([CIMG, S], dt.bfloat16, name="ot")
            for k in range(NCH):
                nv, o0 = int(sizes[k]), int(offs[k])
                ps = pp.tile([CIMG, 512], dt.float32, tag="ps", name="ps")
                nc.tensor.matmul(ps[:, 0:nv], ft[:, k, :], wt[:, o0:o0 + nv],
                                 start=True, stop=True)
                if k % 2 == 0:
                    nc.scalar.activation(ot[:, o0:o0 + nv], ps[:, 0:nv],
                                         mybir.ActivationFunctionType.Copy)
                else:
                    nc.vector.tensor_copy(ot[:, o0:o0 + nv], ps[:, 0:nv])
            nc.sync.dma_start(out=owin, in_=ot[:])
    nc.compile()
    return nc


# ---------------------------------------------------------------- launch C
C_OUT_ROWS = 23              # ds2-out rows per core (8*23 = 184 >= 180)


def build_launch_c():
    nc = bacc.Bacc("TRN2", target_bir_lowering=False, debug=False,
                   num_devices=NCORES)
    NR1 = C_OUT_ROWS + 2                         # ds1-out rows incl halo (25)
    NRP = 2 * NR1 + 1                            # pooled rows needed (51)
    slab = nc.dram_tensor("slab", [CIMG, NRP, 362], dt.bfloat16,
                          kind="ExternalInput").ap()
    m1 = nc.dram_tensor("m1", [128, NR1], dt.bfloat16, kind="ExternalInput").ap()
    wd1 = nc.dram_tensor("wd1", [9, CIMG, CIMG], dt.bfloat16,
                         kind="ExternalInput").ap()
    wd2 = nc.dram_tensor("wd2", [9, CIMG, CIMG], dt.bfloat16,
                         kind="ExternalInput").ap()
    sb1 = nc.dram_tensor("sb1", [CIMG, 2], dt.float32, kind="ExternalInput").ap()
    sb2 = nc.dram_tensor("sb2", [CIMG, 2], dt.float32, kind="ExternalInput").ap()
    yout = nc.dram_tensor("yout", [CIMG, C_OUT_ROWS, 180], dt.float32,
                          kind="ExternalOutput").ap()
    RELU = mybir.ActivationFunctionType.Relu
    with tile.TileContext(nc) as tc:
        with tc.tile_pool(name="const", bufs=1) as cpool,              tc.tile_pool(name="work", bufs=2) as wp,              tc.tile_pool(name="big", bufs=1) as bp,              tc.tile_pool(name="ps", bufs=3, space="PSUM") as pp:
            slabt = bp.tile([CIMG, NRP, 362], dt.bfloat16, name="slabt")
            for rr in range(0, NRP, 13):
                nrr = min(13, NRP - rr)
                nc.sync.dma_start(out=slabt[:, rr:rr + nrr, :],
                                  in_=slab[:, rr:rr + nrr, :])
            w1 = cpool.tile([CIMG, 9, CIMG], dt.bfloat16, name="w1")
            nc.sync.dma_start(out=w1[:], in_=wd1.rearrange("t p o -> p t o"))
            w2 = cpool.tile([CIMG, 9, CIMG], dt.bfloat16, name="w2")
            nc.sync.dma_start(out=w2[:], in_=wd2.rearrange("t p o -> p t o"))
            sb1t = cpool.tile([CIMG, 2], dt.float32, name="sb1t")
            nc.sync.dma_start(out=sb1t[:], in_=sb1)
            sb2t = cpool.tile([CIMG, 2], dt.float32, name="sb2t")
            nc.sync.dma_start(out=sb2t[:], in_=sb2)
            m1t = wp.tile([128, NR1], dt.bfloat16, name="m1t")
            nc.sync.dma_start(out=m1t[:], in_=m1)
            h1 = bp.tile([CIMG, NR1, 182], dt.bfloat16, name="h1")
            nc.vector.memset(h1[:, :, 0:1], 0.0)
            nc.vector.memset(h1[:, :, 181:182], 0.0)
            # ds1: stride-2 3x3; out row t reads slab rows 2t..2t+2 (slab row 0
            # = pooled row 2o0-3, so out row t (global o0-1+t) reads
            # 2(o0-1+t)-1..+1 - (2o0-3) = 2t..2t+2); col c reads 2c..2c+2
            RP = 2
            for t0 in range(0, NR1, RP):
                nr = min(RP, NR1 - t0)
                ps = pp.tile([CIMG, nr, 180], dt.float32, tag="ps1", name="ps")
                gi = 0
                for ky in range(3):
                    for kx in range(3):
                        rhs = bass.AP(slabt.tensor,
                                      slabt.offset + (2 * t0 + ky) * 362 + kx,
                                      [slabt.ap[0], [2 * 362, nr], [2, 180]])
                        nc.tensor.matmul(ps[:], w1[:, ky * 3 + kx, :], rhs,
                                         start=(gi == 0), stop=(gi == 8))
                        gi += 1
                ev = wp.tile([CIMG, nr, 180], dt.bfloat16, tag="ev", name="ev")
                nc.scalar.activation(ev[:], ps[:], RELU, bias=sb1t[:, 1:2],
                                     scale=sb1t[:, 0:1])
                mbb = bass.AP(m1t.tensor, m1t.offset + t0,
                              [[m1t.ap[0][0], CIMG], [1, nr], [0, 180]])
                nc.vector.tensor_tensor(out=h1[:, t0:t0 + nr, 1:181],
                                        in0=ev[:], in1=mbb,
                                        op=mybir.AluOpType.mult)
            # ds2: 3x3 pad 1: out row o reads h1 rows o..o+2, col c: c..c+2
            yo = bp.tile([CIMG, C_OUT_ROWS, 180], dt.float32, name="yo")
            for o0 in range(0, C_OUT_ROWS, RP):
                nr = min(RP, C_OUT_ROWS - o0)
                ps = pp.tile([CIMG, nr, 180], dt.float32, tag="ps2", name="ps")
                gi = 0
                for ky in range(3):
                    for kx in range(3):
                        rhs = bass.AP(h1.tensor,
                                      h1.offset + (o0 + ky) * 182 + kx,
                                      [h1.ap[0], [182, nr], [1, 180]])
                        nc.tensor.matmul(ps[:], w2[:, ky * 3 + kx, :], rhs,
                                         start=(gi == 0), stop=(gi == 8))
                        gi += 1
                nc.scalar.activation(yo[:, o0:o0 + nr, :], ps[:], RELU,
                                     bias=sb2t[:, 1:2], scale=sb2t[:, 0:1])
                nc.sync.dma_start(out=yout[:, o0:o0 + nr, :],
                                  in_=yo[:, o0:o0 + nr, :])
    nc.compile()
    return nc


_CACHE = {}


def run_launch_a(inputs):
    if "A" not in _CACHE:
        _CACHE["A"] = build_launch_a()
    nc = _CACHE["A"]
    maps = _prep_a_inputs(inputs)
    res = run_bass_kernel_spmd(nc, maps, list(range(NCORES)))
    depth = np.zeros((NPIX, DD), np.float32)
    feat = np.zeros((NPIX, CIMG), np.float32)
    for c in range(NCORES):
        r = res.results[c]
        for s, (cam, h0) in enumerate([SEG_A[c], SEG_B[c]]):
            S = SEGS[s]
            npix = S["nout"] * FW
            base = (cam * FH + h0) * FW
            off = 0 if s == 0 else 16 * FW
            depth[base:base + npix] = r["out_depth"][off:off + npix]
            feat[base:base + npix] = r["out_feat"][off:off + npix].astype(np.float32)
    return depth, feat


def _build_chunks(flat, kept, depth_rows):
    """Group points by (camera, column-block); per group build the
    [pix, voxel-slot] depth-weight matrix over the group's voxel union.
    Splits column blocks whose union exceeds the PSUM window (512)."""
    fl = flat.reshape(N, DD, FH, FW)
    kp = kept.reshape(N, DD, FH, FW)
    chunks = []                      # (pix_ids, Wdense[npix, nv], vox_ids)

    def add_group(n, w0, w1):
        nw = w1 - w0
        f = fl[n, :, :, w0:w1]                       # [DD, FH, nw]
        k = kp[n, :, :, w0:w1]
        vids = np.unique(f[k])
        if len(vids) > 512 and nw > 1:
            mid = w0 + nw // 2
            add_group(n, w0, mid)
            add_group(n, mid, w1)
            return
        nv = max(len(vids), 1)
        # pixel local idx = (w - w0) * FH + h; point (d, h, w)
        slot = np.searchsorted(vids, f[k]) if len(vids) else np.zeros(0, np.int64)
        dd, hh, ww = np.nonzero(k)
        pix_loc = ww * FH + hh
        pixcol = n * FH * FW + hh * FW + (ww + w0)
        dep = depth_rows[pixcol, dd]
        Wd = np.bincount(pix_loc * nv + slot, weights=dep,
                         minlength=nw * FH * nv).reshape(nw * FH, nv)
        pix_ids = (n * FH * FW + np.arange(FH)[None, :] * FW
                   + (w0 + np.arange(nw))[:, None]).reshape(-1)
        chunks.append((pix_ids, Wd, vids))

    for n in range(N):
        for w0 in range(0, FW, 4):
            add_group(n, w0, w0 + 4)
    return chunks


def _prep_b_inputs(chunks, featflat_bf):
    """Balance chunks across cores by window size; build per-core maps with
    the packed per-slot layout (chunk k size = max over cores, desc-sorted)."""
    order = sorted(range(len(chunks)), key=lambda i: -chunks[i][1].shape[1])
    load = np.zeros(NCORES, np.int64)
    per_core = [[] for _ in range(NCORES)]
    for i in order:
        c = int(np.argmin(load))
        per_core[c].append(i)
        load[c] += chunks[i][1].shape[1]
    NCH = max(len(p) for p in per_core)
    sizes = np.zeros(NCH, np.int64)
    for p in per_core:
        for k, i in enumerate(p):
            sizes[k] = max(sizes[k], chunks[i][1].shape[1])
    sizes = (sizes + 15) // 16 * 16
    offs = np.concatenate([[0], np.cumsum(sizes)]).astype(int)
    S = int(offs[-1])
    maps, scatter = [], []
    for c in range(NCORES):
        wm = np.zeros((128, S), bf16)
        ft = np.zeros((128, NCH, CIMG), bf16)
        sc = []
        for k, i in enumerate(per_core[c]):
            pix_ids, Wd, vids = chunks[i]
            npix, nv = Wd.shape
            wm[0:npix, offs[k]:offs[k] + nv] = Wd
            ft[0:npix, k, :] = featflat_bf[pix_ids]
            sc.append((int(offs[k]), vids))
        maps.append(dict(wmat=wm, feats=ft))
        scatter.append(sc)
    return maps, scatter, tuple(int(s) for s in sizes)


def _prep_c_inputs(inputs, pooled_t):
    """pooled_t: [CIMG, 360, 360] f32 -> per-core slabs + masks + weights."""
    NR1 = C_OUT_ROWS + 2
    NRP = 2 * NR1 + 1
    w1 = np.asarray(inputs["ds1_w"], np.float32)
    w2 = np.asarray(inputs["ds2_w"], np.float32)
    wd1 = np.stack([w1[:, :, ky, kx].T for ky in range(3) for kx in range(3)])
    wd2 = np.stack([w2[:, :, ky, kx].T for ky in range(3) for kx in range(3)])
    sb1 = np.stack([np.asarray(inputs["ds1_s"], np.float32),
                    np.asarray(inputs["ds1_t"], np.float32)], 1)
    sb2 = np.stack([np.asarray(inputs["ds2_s"], np.float32),
                    np.asarray(inputs["ds2_t"], np.float32)], 1)
    shared = dict(wd1=wd1.astype(bf16), wd2=wd2.astype(bf16), sb1=sb1, sb2=sb2)
    maps = []
    pt_bf = pooled_t.astype(bf16)
    for c in range(NCORES):
        o0g = C_OUT_ROWS * c
        p0 = 2 * o0g - 3
        slab = np.zeros((CIMG, NRP, 362), bf16)
        lo, hi = max(0, p0), min(NX, p0 + NRP)
        if hi > lo:
            slab[:, lo - p0:hi - p0, 1:361] = pt_bf[:, lo:hi, :]
        t1g = np.arange(NR1) + (o0g - 1)
        m1 = np.broadcast_to(((t1g >= 0) & (t1g < 180))[None, :],
                             (128, NR1)).astype(bf16)
        maps.append(dict(shared, slab=slab, m1=np.ascontiguousarray(m1)))
    return maps


def kernel(**inputs):
    inputs = {k: np.asarray(v) for k, v in inputs.items()}
    flat, kept = _host_geometry(inputs["cam2lidar_rots"],
                                inputs["cam2lidar_trans"], inputs["intrins"],
                                inputs["post_rots"], inputs["post_trans"])
    depth_rows, feat_rows = run_launch_a(inputs)
    featflat_bf = feat_rows.astype(bf16)

    chunks = _build_chunks(flat, kept, depth_rows)
    bmaps, scatter, sizes = _prep_b_inputs(chunks, featflat_bf)
    key = ("B", sizes)
    if key not in _CACHE:
        _CACHE[key] = build_launch_b(sizes)
    res_b = run_bass_kernel_spmd(_CACHE[key], bmaps, list(range(NCORES)))

    allvox = np.concatenate([vids for c in range(NCORES)
                             for _, vids in scatter[c]])
    allval = np.concatenate(
        [res_b.results[c]["owin"][:, o0:o0 + len(vids)].T.astype(np.float32)
         for c in range(NCORES) for o0, vids in scatter[c]])
    o = np.argsort(allvox, kind="stable")
    allvox, allval = allvox[o], allval[o]
    starts = np.flatnonzero(np.r_[True, allvox[1:] != allvox[:-1]])
    pooled = np.zeros((NX * NX, CIMG), np.float32)
    pooled[allvox[starts]] = np.add.reduceat(allval, starts, axis=0)
    pooled_t = np.ascontiguousarray(
        pooled.reshape(NX, NX, CIMG).transpose(2, 0, 1))

    if "C" not in _CACHE:
        _CACHE["C"] = build_launch_c()
    cmaps = _prep_c_inputs(inputs, pooled_t)
    res_c = run_bass_kernel_spmd(_CACHE["C"], cmaps, list(range(NCORES)))
    out = np.zeros((1, CIMG, 180, 180), np.float32)
    for c in range(NCORES):
        o0g = C_OUT_ROWS * c
        nr = min(C_OUT_ROWS, 180 - o0g)
        if nr > 0:
            out[0, :, o0g:o0g + nr, :] = res_c.results[c]["yout"][:, 0:nr, :]
    return out



# revision 14
# speedup vs baseline: 1.6202x; 1.0838x over previous
"""DepthLSSTransform Trainium kernel: 3 SPMD launches over 8 NeuronCores.

Launch A: per-camera conv pipeline (dtransform + depthnet + softmax) on
          24-row bands (one 16-row + one 8-row segment per core).
Launch B: bev_pool segment-sum via one-hot matmuls over a host-built
          virtual-window schedule (sorted-by-voxel points).
Launch C: BEV downsample convs, spatially sharded.
Host: geometry/voxel indices, scheduling, gathers, folds (orchestration).
"""
import numpy as np
import ml_dtypes

import concourse.bass as bass
import concourse.tile as tile
from concourse import bacc, mybir
from concourse.bass_utils import run_bass_kernel_spmd

dt = mybir.dt
bf16 = ml_dtypes.bfloat16

# ---- problem constants (hardcoded per contract) ----
B, N = 1, 6
CIN, CIMG, DD = 256, 80, 59
FH, FW, IH, IW = 32, 88, 256, 704
XY0, DXY, NX = -54.0, 0.3, 360
Z0, DZ, NZ = -10.0, 20.0, 1
NPTS = N * DD * FH * FW
NPIX = N * FH * FW
NCORES = 8
QV = 4                      # chunks of 128 points per virtual window

# per-core segments: (camera, h0) for seg A (16 rows) and seg B (8 rows)
SEG_A = [(0, 0), (1, 0), (1, 16), (2, 16), (3, 0), (4, 0), (4, 16), (5, 16)]
SEG_B = [(0, 16), (0, 24), (2, 0), (2, 8), (3, 16), (3, 24), (5, 0), (5, 8)]
# band pixel ranges in global row order (row = n*32 + h)
ROWS_OF_CORE = [[(SEG_A[c][0] * FH + SEG_A[c][1] + r) for r in range(16)] +
                [(SEG_B[c][0] * FH + SEG_B[c][1] + r) for r in range(8)]
                for c in range(NCORES)]

# segment geometry: rows16 segment: d rows [8h0-34, 8h0+158) (192), dt2 out
# rows [2h0-8, 2h0+39) (47), dt3 [h0-3, h0+19) (22), dn1 [h0-1, h0+17) (18)
SEGS = [dict(nout=16, nd=192, nq=48, nt2=47, nt3=22, nn1=18),
        dict(nout=8, nd=128, nq=32, nt2=31, nt3=14, nn1=10)]


def _seg_ranges(h0, S):
    return dict(d0=8 * h0 - 34, q0=2 * h0 - 8, t0=h0 - 3, r0=h0 - 1, o0=h0)


# ---------------------------------------------------------------- launch A
def build_launch_a(debug=False, psum_bufs=3, work_bufs=3, stages=9):
    nc = bacc.Bacc("TRN2", target_bir_lowering=False, debug=False,
                   num_devices=NCORES)
    AP = {}

    def inp(name, shape, dtype=dt.bfloat16):
        AP[name] = nc.dram_tensor(name, shape, dtype, kind="ExternalInput").ap()
        return AP[name]

    # per segment inputs (s = 0: 16-row, 1: 8-row)
    for s, S in enumerate(SEGS):
        inp(f"dph{s}", [128, S["nq"], 177])
        inp(f"masks{s}", [128, S["nt2"] + S["nt3"] + S["nn1"]])
        inp(f"xseg{s}", [2, 128, S["nt3"], 92])         # x_img slice (padded)
    # packed f32 constants: [alpha, beta, s_dt2, t_dt2, s_dt3, t_dt3,
    #  s_dn1(2), t_dn1(2), s_dn2(2), t_dn2(2), b_dn3(139)] -> [128, 153]
    inp("consts", [128, 153], dt.float32)
    # conv weights (host-prepped layouts)
    inp("w_dt2", [4, 128, 32])                          # groups (dky,dmx)
    inp("w_dt3", [9, 128, 64])
    inp("w_dn1", [9, 3, 128, 256])                      # tap, icchunk(128,128,64pad) -> 256
    inp("w_dn2", [9, 2, 128, 256])
    inp("w_dn3", [2, 128, 139])

    DBG = {}
    dbg_specs = [] if not debug else [("dbg_t1", [128, SEGS[0]["nq"], 177], dt.bfloat16),
                        ("dbg_dt2o", [32, SEGS[0]["nt2"] + 1, 180], dt.bfloat16),
                        ("dbg_dtc", [64, SEGS[0]["nt3"], 92], dt.bfloat16),
                        ("dbg_n1o", [128, SEGS[0]["nn1"], 92], dt.bfloat16),
                        ("dbg_n2o", [128, SEGS[0]["nout"], 88], dt.bfloat16)]
    for nm, sh, dty in dbg_specs:
        DBG[nm] = nc.dram_tensor(nm, sh, dty, kind="ExternalOutput").ap()
    out_depth = nc.dram_tensor("out_depth", [24 * FW, DD], dt.float32,
                               kind="ExternalOutput").ap()
    out_feat = nc.dram_tensor("out_feat", [24 * FW, CIMG], dt.bfloat16,
                              kind="ExternalOutput").ap()

    # HBM scratch
    scr = {}
    for s, S in enumerate(SEGS):
        scr[f"dt2o{s}"] = nc.dram_tensor(f"dt2o{s}", [32, S["nt2"] + 1, 2, 90], dt.bfloat16).ap()

    RELU = mybir.ActivationFunctionType.Relu
    with tile.TileContext(nc) as tc:
        with tc.tile_pool(name="const", bufs=1) as cpool, \
             tc.tile_pool(name="work", bufs=work_bufs) as wpool, \
             tc.tile_pool(name="big", bufs=1) as bpool, \
             tc.tile_pool(name="psum", bufs=psum_bufs, space="PSUM") as ppool:
            # ---- load packed constants in one DMA ----
            cts = cpool.tile([128, 153], dt.float32, name="cts")
            nc.sync.dma_start(out=cts[:], in_=AP["consts"])
            ct = {"dt1_alpha": cts[:, 0:1], "dt1_beta": cts[:, 1:2],
                  "s_dt2": cts[:, 2:3], "t_dt2": cts[:, 3:4],
                  "s_dt3": cts[:, 4:5], "t_dt3": cts[:, 5:6],
                  "s_dn1": cts[:, 6:8], "t_dn1": cts[:, 8:10],
                  "s_dn2": cts[:, 10:12], "t_dn2": cts[:, 12:14],
                  "b_dn3": cts[:, 14:153]}
            wt = {}
            for nm, pat in [("w_dt2", "g p o -> p g o"),
                            ("w_dt3", "g p o -> p g o"),
                            ("w_dn1", "t i p o -> p (t i) o"),
                            ("w_dn2", "t i p o -> p (t i) o"),
                            ("w_dn3", "g p o -> p g o")]:
                sh = list(AP[nm].shape)
                wt[nm] = cpool.tile([sh[-2], int(np.prod(sh[:-2])), sh[-1]],
                                    dt.bfloat16, tag=nm, name=f'wt_{nm}')
                nc.sync.dma_start(out=wt[nm][:], in_=AP[nm].rearrange(pat))

            feat_sb = {}
            depth_sb = {}
            for s, S in enumerate(SEGS):
                nq, nt2, nt3, nn1, nout = S["nq"], S["nt2"], S["nt3"], S["nn1"], S["nout"]
                # ======== dt1: relu(alpha*d + beta) on Act; host bakes pad
                # values into dph so relu zeroes out-of-image positions ====
                dph = bpool.tile([128, nq, 177], dt.bfloat16, tag=f"dph{s}")
                for qq in range(0, nq, nq // 4):
                    nqq = min(nq // 4, nq - qq)
                    nc.sync.dma_start(out=dph[:, qq:qq + nqq, :],
                                      in_=AP[f"dph{s}"][:, qq:qq + nqq, :])
                t1 = bpool.tile([128, nq, 177], dt.bfloat16, tag=f"t1{s}")
                mall = wpool.tile([128, nt2 + nt3 + nn1], dt.bfloat16,
                                  tag=f"msk{s}", name="mall")
                nc.sync.dma_start(out=mall[:], in_=AP[f"masks{s}"])
                QCH = nq // 4
                for qq in range(0, nq, QCH):
                    nqq = min(QCH, nq - qq)
                    sl = (slice(None), slice(qq, qq + nqq), slice(None))
                    nc.scalar.activation(t1[sl], dph[sl], RELU,
                                         bias=ct["dt1_beta"][:, 0:1],
                                         scale=ct["dt1_alpha"][:, 0:1])
                if s == 0 and debug:
                    nc.sync.dma_start(out=DBG["dbg_t1"], in_=t1[:])

                if stages < 2:
                    continue
                # ======== dt2 ========
                o2 = bpool.tile([32, nt2 + 1, 180], dt.bfloat16, tag=f"o2{s}")
                # border strips only: cols 0, 89-90, 179 and the pad row nt2
                nc.vector.memset(o2[:, :, 0:1], 0.0)
                nc.vector.memset(o2[:, :, 89:91], 0.0)
                nc.vector.memset(o2[:, :, 179:180], 0.0)
                nc.vector.memset(o2[:, nt2:nt2 + 1, :], 0.0)
                m2 = bass.AP(mall.tensor, mall.offset, [mall.ap[0], [1, nt2]])
                RPP2 = 2
                for q0 in range(0, nt2, RPP2):
                    nr = min(RPP2, nt2 - q0)
                    ps = ppool.tile([32, nr, 176], dt.float32, tag=f"ps{s}", name="ps2")
                    gi = 0
                    for dky in range(2):
                        for dmx in range(2):
                            g = dky * 2 + dmx
                            rhs = bass.AP(
                                t1.tensor, t1.offset + (q0 + dky) * 177 + dmx,
                                [t1.ap[0], [177, nr], [1, 176]])
                            nc.tensor.matmul(ps[:], wt["w_dt2"][:, g, :], rhs,
                                             start=(gi == 0), stop=(gi == 3))
                            gi += 1
                    ev = wpool.tile([32, nr, 176], dt.bfloat16, tag=f"ev2{s}")
                    nc.scalar.activation(ev[:], ps[:], RELU,
                                         bias=ct["t_dt2"][0:32, 0:1],
                                         scale=ct["s_dt2"][0:32, 0:1])
                    mbb = bass.AP(m2.tensor, m2.offset + q0,
                                  [[m2.ap[0][0], 32], [1, nr], [0, 176]])
                    # write col c at (c%2)*90 + c//2 + 1  (phase-split layout)
                    o2dst = bass.AP(o2.tensor, o2.offset + q0 * 180 + 1,
                                    [[o2.ap[0][0], 32], [180, nr],
                                     [1, 88], [90, 2]])
                    nc.vector.tensor_tensor(out=o2dst, in0=ev[:], in1=mbb,
                                            op=mybir.AluOpType.mult)
                nc.sync.dma_start(out=scr[f"dt2o{s}"],
                                  in_=o2.rearrange("p q (b x) -> p q b x", b=2))
                if s == 0 and debug:
                    nc.sync.dma_start(out=DBG["dbg_dt2o"], in_=o2[:])

                if stages < 3:
                    continue
                # ======== dt3 ========
                nry3 = nt3 + 2
                ph3 = bpool.tile([128, nry3, 90], dt.bfloat16, tag=f"ph3{s}")
                sd2 = scr[f"dt2o{s}"]
                for a2 in range(2):
                    for b2 in range(2):
                        pap3 = bass.AP(sd2.tensor,
                                       sd2.offset + a2 * 180 + b2 * 90,
                                       [[(nt2 + 1) * 180, 32],
                                        [2 * 180, nry3], [1, 90]])
                        nc.sync.dma_start(
                            out=ph3[(a2 * 2 + b2) * 32:(a2 * 2 + b2 + 1) * 32],
                            in_=pap3)
                # concat input tile: [64 dt3 | pad] plus x_img tiles
                dtc = bpool.tile([64, nt3, 92], dt.bfloat16, tag=f"dtc{s}")
                nc.vector.memset(dtc[:, :, 0:2], 0.0)
                nc.vector.memset(dtc[:, :, 90:92], 0.0)
                m3 = bass.AP(mall.tensor, mall.offset + nt2,
                             [mall.ap[0], [1, nt3]])
                RPP3 = 4
                for t0 in range(0, nt3, RPP3):
                    nr = min(RPP3, nt3 - t0)
                    ps = ppool.tile([64, nr, 88], dt.float32, tag=f"ps{s}")
                    gi = 0
                    for dky in range(3):
                        for dmx in range(3):
                            g = dky * 3 + dmx
                            rhs = bass.AP(ph3.tensor,
                                          ph3.offset + (t0 + dky) * 90 + dmx,
                                          [ph3.ap[0], [90, nr], [1, 88]])
                            nc.tensor.matmul(ps[:], wt["w_dt3"][:, g, :], rhs,
                                             start=(gi == 0), stop=(gi == 8))
                            gi += 1
                    ev = wpool.tile([64, nr, 88], dt.bfloat16, tag=f"ev3{s}")
                    nc.scalar.activation(ev[:], ps[:], RELU,
                                         bias=ct["t_dt3"][0:64, 0:1],
                                         scale=ct["s_dt3"][0:64, 0:1])
                    mbb = bass.AP(m3.tensor, m3.offset + t0,
                                  [m3.ap[0], [1, nr], [0, 88]])
                    nc.vector.tensor_tensor(out=dtc[:, t0:t0 + nr, 2:90],
                                            in0=ev[:], in1=mbb[0:64],
                                            op=mybir.AluOpType.mult)

                if s == 0 and debug:
                    nc.sync.dma_start(out=DBG["dbg_dtc"], in_=dtc[:])
                if stages < 4:
                    continue
                # ======== dn1 ========
                xs = []
                for g in range(2):
                    xt = bpool.tile([128, nt3, 92], dt.bfloat16, tag=f"x{g}_{s}",
                                     name=f"xseg_t{g}")
                    nc.sync.dma_start(out=xt[:], in_=AP[f"xseg{s}"][g])
                    xs.append(xt)
                mn1 = bass.AP(mall.tensor, mall.offset + nt2 + nt3,
                              [mall.ap[0], [1, nn1]])
                n1o = []
                for g in range(2):
                    t = bpool.tile([128, nn1, 92], dt.bfloat16, tag=f"n1o{g}_{s}")
                    nc.vector.memset(t[:, :, 0:2], 0.0)
                    nc.vector.memset(t[:, :, 90:92], 0.0)
                    n1o.append(t)
                RPP = 5
                for ocg in range(2):
                    for r0 in range(0, nn1, RPP):
                        nr = min(RPP, nn1 - r0)
                        ps = ppool.tile([128, nr, 88], dt.float32, tag=f"ps{s}")
                        gi = 0
                        for ky in range(3):
                            for kx in range(3):
                                tap = ky * 3 + kx
                                for icc, srcT in enumerate((xs[0], xs[1], dtc)):
                                    kk = 128 if icc < 2 else 64
                                    rhs = bass.AP(
                                        srcT.tensor,
                                        srcT.offset + (r0 + ky + 1) * 92 + kx + 1,
                                        [srcT.ap[0], [92, nr], [1, 88]])
                                    lhs = wt["w_dn1"][0:kk, tap * 3 + icc,
                                                      ocg * 128:(ocg + 1) * 128]
                                    nc.tensor.matmul(ps[:], lhs, rhs,
                                                     start=(gi == 0),
                                                     stop=(gi == 26))
                                    gi += 1
                        ev = wpool.tile([128, nr, 88], dt.bfloat16, tag=f"evn1{s}")
                        nc.scalar.activation(ev[:], ps[:], RELU,
                                             bias=ct["t_dn1"][:, ocg:ocg + 1],
                                             scale=ct["s_dn1"][:, ocg:ocg + 1])
                        mbb = bass.AP(mn1.tensor, mn1.offset + r0,
                                      [mn1.ap[0], [1, nr], [0, 88]])
                        nc.vector.tensor_tensor(
                            out=n1o[ocg][:, r0:r0 + nr, 2:90],
                            in0=ev[:], in1=mbb, op=mybir.AluOpType.mult)

                if s == 0 and debug:
                    nc.sync.dma_start(out=DBG["dbg_n1o"], in_=n1o[0][:])
                if stages < 5:
                    continue
                # ======== dn2 ========
                n2o = []
                for g in range(2):
                    n2o.append(bpool.tile([128, nout, 88], dt.bfloat16,
                                          tag=f"n2o{g}_{s}", name=f"n2o{g}"))
                for ocg in range(2):
                    for r0 in range(0, nout, RPP):
                        nr = min(RPP, nout - r0)
                        ps = ppool.tile([128, nr, 88], dt.float32, tag=f"ps{s}")
                        gi = 0
                        for ky in range(3):
                            for kx in range(3):
                                tap = ky * 3 + kx
                                for icc in range(2):
                                    rhs = bass.AP(
                                        n1o[icc].tensor,
                                        n1o[icc].offset + (r0 + ky) * 92 + kx + 1,
                                        [n1o[icc].ap[0], [92, nr], [1, 88]])
                                    lhs = wt["w_dn2"][:, tap * 2 + icc,
                                                      ocg * 128:(ocg + 1) * 128]
                                    nc.tensor.matmul(ps[:], lhs, rhs,
                                                     start=(gi == 0),
                                                     stop=(gi == 17))
                                    gi += 1
                        ev = wpool.tile([128, nr, 88], dt.bfloat16, tag=f"evn2{s}")
                        nc.scalar.activation(ev[:], ps[:], RELU,
                                             bias=ct["t_dn2"][:, ocg:ocg + 1],
                                             scale=ct["s_dn2"][:, ocg:ocg + 1])
                        nc.vector.tensor_copy(n2o[ocg][:, r0:r0 + nr, :], ev[:])

                if s == 0 and debug:
                    nc.sync.dma_start(out=DBG["dbg_n2o"], in_=n2o[0][:])
                if stages < 6:
                    continue
                # ======== dn3 + softmax + feat ========
                npix = nout * FW
                feat_sb[s] = bpool.tile([128, (npix + 127) // 128, CIMG],
                                        dt.bfloat16, tag=f"feat{s}", name=f"feat_sb{s}")
                depth_sb[s] = bpool.tile([128, (npix + 127) // 128, DD],
                                         dt.float32, tag=f"depth{s}", name=f"depth_sb{s}")
                n2f = [t.rearrange("p a b -> p (a b)") for t in n2o]
                for pc in range((npix + 127) // 128):
                    m = min(128, npix - pc * 128)
                    ps = ppool.tile([m, 139], dt.float32, tag=f"ps{s}")
                    for icc in range(2):
                        nc.tensor.matmul(ps[:], n2f[icc][:, pc * 128:pc * 128 + m],
                                         wt["w_dn3"][:, icc, :],
                                         start=(icc == 0), stop=(icc == 1))
                    # add bias via vector then softmax over first 59
                    lg = wpool.tile([m, 139], dt.float32, tag=f"lg{s}")
                    nc.vector.tensor_tensor(out=lg[:], in0=ps[:],
                                            in1=ct["b_dn3"][0:m],
                                            op=mybir.AluOpType.add)
                    mx = wpool.tile([m, 1], dt.float32, tag=f"mx{s}")
                    nc.vector.reduce_max(mx[:], lg[:, 0:DD],
                                         axis=mybir.AxisListType.X, negate=True)
                    ex = wpool.tile([m, DD], dt.float32, tag=f"ex{s}")
                    nc.scalar.activation(ex[:], lg[:, 0:DD],
                                         mybir.ActivationFunctionType.Exp,
                                         bias=mx[:, 0:1], scale=1.0)
                    sm = wpool.tile([m, 1], dt.float32, tag=f"sm{s}")
                    nc.vector.reduce_sum(sm[:], ex[:], axis=mybir.AxisListType.X)
                    rc = wpool.tile([m, 1], dt.float32, tag=f"rc{s}")
                    nc.vector.reciprocal(rc[:], sm[:])
                    nc.vector.tensor_scalar(out=depth_sb[s][0:m, pc, :], in0=ex[:],
                                            scalar1=rc[:, 0:1], scalar2=None,
                                            op0=mybir.AluOpType.mult)
                    nc.vector.tensor_copy(feat_sb[s][0:m, pc, :],
                                          lg[:, DD:DD + CIMG])

            # DMA outputs: global pix index = seg-A pix then seg-B pix
            for s, S in (enumerate(SEGS) if stages >= 6 else []):
                npix = S["nout"] * FW
                base = 0 if s == 0 else 16 * FW
                nfull = npix // 128
                dsl = out_depth[base:base + nfull * 128].rearrange(
                    "(a p) d -> p a d", p=128)
                nc.sync.dma_start(out=dsl, in_=depth_sb[s][:, 0:nfull, :])
                fsl = out_feat[base:base + nfull * 128].rearrange(
                    "(a p) d -> p a d", p=128)
                nc.sync.dma_start(out=fsl, in_=feat_sb[s][:, 0:nfull, :])
                rem = npix - nfull * 128
                if rem:
                    nc.sync.dma_start(
                        out=out_depth[base + nfull * 128:base + npix],
                        in_=depth_sb[s][0:rem, nfull, :])
                    nc.sync.dma_start(
                        out=out_feat[base + nfull * 128:base + npix],
                        in_=feat_sb[s][0:rem, nfull, :])
    nc.compile()
    return nc


# ------------------------------------------------------------ host helpers
def _host_geometry(rots, trans, intr, post_rots, post_trans):
    import jax
    import jax.numpy as jnp
    with jax.default_device(jax.devices("cpu")[0]):
        f32 = jnp.float32
        ds = jnp.arange(1.0, 60.0, 1.0, dtype=f32)
        xs = jnp.linspace(0.0, IW - 1.0, FW, dtype=f32)
        ys = jnp.linspace(0.0, IH - 1.0, FH, dtype=f32)
        dm = jnp.broadcast_to(ds[:, None, None], (DD, FH, FW))
        xm = jnp.broadcast_to(xs[None, None, :], (DD, FH, FW))
        ym = jnp.broadcast_to(ys[None, :, None], (DD, FH, FW))
        fr = jnp.stack([xm, ym, dm], -1)
        pts = fr[None, None] - jnp.asarray(post_trans)[:, :, None, None, None, :]
        pts = jnp.einsum("bnij,bndhwj->bndhwi",
                         jnp.linalg.inv(jnp.asarray(post_rots)), pts)
        pts = jnp.concatenate([pts[..., :2] * pts[..., 2:3], pts[..., 2:3]], -1)
        comb = jnp.einsum("bnij,bnjk->bnik", jnp.asarray(rots),
                          jnp.linalg.inv(jnp.asarray(intr)))
        pts = jnp.einsum("bnij,bndhwj->bndhwi", comb, pts) \
            + jnp.asarray(trans)[:, :, None, None, None, :]
        lo = jnp.array([XY0, XY0, Z0], dtype=f32)
        dxv = jnp.array([DXY, DXY, DZ], dtype=f32)
        g = ((pts - lo) / dxv).astype(jnp.int32).reshape(-1, 3)
        kept = ((g[:, 0] >= 0) & (g[:, 0] < NX) & (g[:, 1] >= 0) & (g[:, 1] < NX)
                & (g[:, 2] >= 0) & (g[:, 2] < NZ))
        flat = (g[:, 2] * NX + g[:, 0]) * NX + g[:, 1]
        return np.asarray(flat, np.int64), np.asarray(kept)


def _prep_a_inputs(inputs):
    """Build per-core input maps for launch A."""
    d = np.asarray(inputs["d"], np.float32).reshape(N, IH, IW)
    x_img = np.asarray(inputs["x_img"], np.float32)

    # dt1 folded affine: relu(alpha*d + beta), alpha = s*w, beta = s*b + t
    a1 = (inputs["dt1_s"] * inputs["dt1_w"][:, 0, 0, 0]).astype(np.float32)
    b1 = (inputs["dt1_s"] * inputs["dt1_b"] + inputs["dt1_t"]).astype(np.float32)
    cab = np.arange(128)
    dt1_alpha = a1[cab // 16][:, None]
    dt1_beta = b1[cab // 16][:, None]

    def wprep_dt2():
        w = np.asarray(inputs["dt2_w"], np.float32)      # [32,8,5,5]
        out = np.zeros((4, 128, 32), np.float32)
        for ky in range(5):
            for kx in range(5):
                a, dky = ky % 4, ky // 4
                bph, dmx = (kx + 2) % 4, (kx + 2) // 4
                g = dky * 2 + dmx
                rows = (np.arange(8)) * 16 + a * 4 + bph
                out[g, rows, :] = w[:, :, ky, kx].T
        return out.astype(bf16)

    def wprep_dt3():
        w = np.asarray(inputs["dt3_w"], np.float32)      # [64,32,5,5]
        out = np.zeros((9, 128, 64), np.float32)
        for ky in range(5):
            for kx in range(5):
                a, dky = ky % 2, ky // 2
                bph, dmx = kx % 2, (kx + 2) // 2 - 1
                g = dky * 3 + dmx
                rows = (a * 2 + bph) * 32 + np.arange(32)
                out[g, rows, :] = w[:, :, ky, kx].T
        return out.astype(bf16)

    def wprep_3x3(w, icc_sizes):
        O, I = w.shape[0], w.shape[1]
        nic = len(icc_sizes)
        out = np.zeros((9, nic, 128, O), np.float32)
        for ky in range(3):
            for kx in range(3):
                tap = ky * 3 + kx
                ic0 = 0
                for icc, sz in enumerate(icc_sizes):
                    out[tap, icc, 0:sz, :] = w[:, ic0:ic0 + sz, ky, kx].T
                    ic0 += sz
        return out.astype(bf16)

    # NOTE: dn1 input concat order is [dt3(64) | x_img(256)] in the reference;
    # our matmul chunks are (x0:128, x1:128, dt3:64) -> weight cols must match:
    w_dn1_full = np.asarray(inputs["dn1_w"], np.float32)
    w_dn1 = np.zeros((9, 3, 128, 256), np.float32)
    for ky in range(3):
        for kx in range(3):
            tap = ky * 3 + kx
            w_dn1[tap, 0, :, :] = w_dn1_full[:, 64:192, ky, kx].T
            w_dn1[tap, 1, :, :] = w_dn1_full[:, 192:320, ky, kx].T
            w_dn1[tap, 2, 0:64, :] = w_dn1_full[:, 0:64, ky, kx].T
    w_dn1 = w_dn1.astype(bf16)
    w_dn2 = wprep_3x3(np.asarray(inputs["dn2_w"], np.float32), [128, 128])
    w_dn3 = np.asarray(inputs["dn3_w"], np.float32)[:, :, 0, 0]  # [139, 256]
    w_dn3p = np.zeros((2, 128, 139), np.float32)
    w_dn3p[0] = w_dn3[:, 0:128].T
    w_dn3p[1] = w_dn3[:, 128:256].T

    def fold_bias(b, s, t):
        # conv bias b then bn scale/shift: relu(s*(x+b) + t) = relu(s*x + (s*b+t))
        return np.asarray(s, np.float32), np.asarray(s * b + t, np.float32)

    s2, t2 = fold_bias(inputs["dt2_b"], inputs["dt2_s"], inputs["dt2_t"])
    s3, t3 = fold_bias(inputs["dt3_b"], inputs["dt3_s"], inputs["dt3_t"])
    sn1, tn1 = fold_bias(inputs["dn1_b"], inputs["dn1_s"], inputs["dn1_t"])
    sn2, tn2 = fold_bias(inputs["dn2_b"], inputs["dn2_s"], inputs["dn2_t"])
    b_dn3 = np.broadcast_to(np.asarray(inputs["dn3_b"], np.float32)[None, :],
                            (128, 139)).copy()

    consts = np.zeros((128, 153), np.float32)
    consts[:, 0] = dt1_alpha[:, 0]
    consts[:, 1] = dt1_beta[:, 0]
    consts[:, 2] = np.tile(s2, 4)
    consts[:, 3] = np.tile(t2, 4)
    consts[:, 4] = np.tile(s3, 2)
    consts[:, 5] = np.tile(t3, 2)
    consts[:, 6:8] = sn1.reshape(2, 128).T
    consts[:, 8:10] = tn1.reshape(2, 128).T
    consts[:, 10:12] = sn2.reshape(2, 128).T
    consts[:, 12:14] = tn2.reshape(2, 128).T
    consts[:, 14:153] = b_dn3
    shared = dict(
        consts=consts,
        w_dt2=wprep_dt2(), w_dt3=wprep_dt3(), w_dn1=w_dn1, w_dn2=w_dn2,
        w_dn3=w_dn3p.astype(bf16),
    )

    # per-channel pad value: alpha*v + beta <= -|alpha|*1e8 < 0 -> relu -> 0
    assert np.abs(a1).min() > 1e-5, "dt1 alpha too small for pad-value trick"
    vpad = (-np.sign(a1) * 1e8).astype(np.float32)           # [8] per channel

    maps = []
    for c in range(NCORES):
        m = dict(shared)
        for s, (cam, h0) in enumerate([SEG_A[c], SEG_B[c]]):
            S = SEGS[s]
            d0 = 8 * h0 - 34
            dseg = np.zeros((S["nd"], 712), np.float32)
            vseg = np.zeros((S["nd"], 712), bool)
            lo, hi = max(0, d0), min(IH, d0 + S["nd"])
            if hi > lo:
                dseg[lo - d0:hi - d0, 4:708] = d[cam, lo:hi]
                vseg[lo - d0:hi - d0, 4:708] = True
            nq = S["nq"]
            ph = dseg.reshape(nq, 4, 178, 4)[:, :, :177, :]     # ry a rx b
            ph = ph.transpose(1, 3, 0, 2)                        # a b ry rx
            vph = vseg.reshape(nq, 4, 178, 4)[:, :, :177, :].transpose(1, 3, 0, 2)
            dphc = np.where(vph[None], ph[None],
                            vpad[:, None, None, None, None])    # [8,4,4,nq,177]
            m[f"dph{s}"] = dphc.reshape(128, nq, 177).astype(bf16)
            q0, t0, r0 = 2 * h0 - 8, h0 - 3, h0 - 1
            qr = np.arange(S["nt2"]) + q0
            m2m = np.broadcast_to(((qr >= 0) & (qr < 64))[None, :],
                                  (128, S["nt2"]))
            tr = np.arange(S["nt3"]) + t0
            m3m = np.broadcast_to(((tr >= 0) & (tr < FH))[None, :],
                                  (128, S["nt3"]))
            rr = np.arange(S["nn1"]) + r0
            mn1m = np.broadcast_to(((rr >= 0) & (rr < FH))[None, :],
                                   (128, S["nn1"]))
            m[f"masks{s}"] = np.concatenate(
                [m2m, m3m, mn1m], axis=1).astype(bf16)
            xseg = np.zeros((2, 128, S["nt3"], 92), np.float32)
            lo2, hi2 = max(0, t0), min(FH, t0 + S["nt3"])
            if hi2 > lo2:
                xseg[:, :, lo2 - t0:hi2 - t0, 2:90] = \
                    x_img[cam, :, lo2:hi2, :].reshape(2, 128, hi2 - lo2, FW)
            m[f"xseg{s}"] = xseg.astype(bf16)
        maps.append(m)
    return maps


# ---------------------------------------------------------------- launch B
def build_launch_b(sizes):
    """Per chunk k: [128pix x 80ch] stationary feat tile x host-built
    [128pix x sizes[k] voxel-slot] depth-weight matrix -> [80, nv] window
    sums. W and out use packed (variable-size) layouts; W loads in a few
    batched DMAs, out in one."""
    nc = bacc.Bacc("TRN2", target_bir_lowering=False, debug=False,
                   num_devices=NCORES)
    NCH = len(sizes)
    offs = np.concatenate([[0], np.cumsum(sizes)]).astype(int)
    S = int(offs[-1])
    wmat = nc.dram_tensor("wmat", [128, S], dt.bfloat16,
                          kind="ExternalInput").ap()
    feats = nc.dram_tensor("feats", [128, NCH, CIMG], dt.bfloat16,
                           kind="ExternalInput").ap()
    owin = nc.dram_tensor("owin", [CIMG, S], dt.bfloat16,
                          kind="ExternalOutput").ap()
    NB = 4                                   # W DMA batches
    bnd = [int(round(NCH * i / NB)) for i in range(NB + 1)]
    with tile.TileContext(nc) as tc:
        with tc.tile_pool(name="const", bufs=1) as cpool, \
             tc.tile_pool(name="ps", bufs=4, space="PSUM") as pp:
            ft = cpool.tile([128, NCH, CIMG], dt.bfloat16, name="ft")
            nc.sync.dma_start(out=ft[:], in_=feats)
            wt = cpool.tile([128, S], dt.bfloat16, name="wt")
            for b in range(NB):
                lo, hi = offs[bnd[b]], offs[bnd[b + 1]]
                if hi > lo:
                    nc.sync.dma_start(out=wt[:, lo:hi], in_=wmat[:, lo:hi])
            ot = cpool.tile([CIMG, S], dt.bfloat16, name="ot")
            for k in range(NCH):
                nv, o0 = int(sizes[k]), int(offs[k])
                ps = pp.tile([CIMG, 512], dt.float32, tag="ps", name="ps")
                nc.tensor.matmul(ps[:, 0:nv], ft[:, k, :], wt[:, o0:o0 + nv],
                                 start=True, stop=True)
                if k % 2 == 0:
                    nc.scalar.activation(ot[:, o0:o0 + nv], ps[:, 0:nv],
                                         mybir.ActivationFunctionType.Copy)
                else:
                    nc.vector.tensor_copy(ot[:, o0:o0 + nv], ps[:, 0:nv])
            nc.sync.dma_start(out=owin, in_=ot[:])
    nc.compile()
    return nc


# ---------------------------------------------------------------- launch C
C_OUT_ROWS = 23              # ds2-out rows per core (8*23 = 184 >= 180)


def build_launch_c():
    nc = bacc.Bacc("TRN2", target_bir_lowering=False, debug=False,
                   num_devices=NCORES)
    NR1 = C_OUT_ROWS + 2                         # ds1-out rows incl halo (25)
    NRP = 2 * NR1 + 1                            # pooled rows needed (51)
    slab = nc.dram_tensor("slab", [CIMG, NRP, 362], dt.bfloat16,
                          kind="ExternalInput").ap()
    m1 = nc.dram_tensor("m1", [128, NR1], dt.bfloat16, kind="ExternalInput").ap()
    wd1 = nc.dram_tensor("wd1", [9, CIMG, CIMG], dt.bfloat16,
                         kind="ExternalInput").ap()
    wd2 = nc.dram_tensor("wd2", [9, CIMG, CIMG], dt.bfloat16,
                         kind="ExternalInput").ap()
    sb1 = nc.dram_tensor("sb1", [CIMG, 2], dt.float32, kind="ExternalInput").ap()
    sb2 = nc.dram_tensor("sb2", [CIMG, 2], dt.float32, kind="ExternalInput").ap()
    yout = nc.dram_tensor("yout", [CIMG, C_OUT_ROWS, 180], dt.float32,
                          kind="ExternalOutput").ap()
    RELU = mybir.ActivationFunctionType.Relu
    with tile.TileContext(nc) as tc:
        with tc.tile_pool(name="const", bufs=1) as cpool,              tc.tile_pool(name="work", bufs=2) as wp,              tc.tile_pool(name="big", bufs=1) as bp,              tc.tile_pool(name="ps", bufs=3, space="PSUM") as pp:
            slabt = bp.tile([CIMG, NRP, 362], dt.bfloat16, name="slabt")
            for rr in range(0, NRP, 13):
                nrr = min(13, NRP - rr)
                nc.sync.dma_start(out=slabt[:, rr:rr + nrr, :],
                                  in_=slab[:, rr:rr + nrr, :])
            w1 = cpool.tile([CIMG, 9, CIMG], dt.bfloat16, name="w1")
            nc.sync.dma_start(out=w1[:], in_=wd1.rearrange("t p o -> p t o"))
            w2 = cpool.tile([CIMG, 9, CIMG], dt.bfloat16, name="w2")
            nc.sync.dma_start(out=w2[:], in_=wd2.rearrange("t p o -> p t o"))
            sb1t = cpool.tile([CIMG, 2], dt.float32, name="sb1t")
            nc.sync.dma_start(out=sb1t[:], in_=sb1)
            sb2t = cpool.tile([CIMG, 2], dt.float32, name="sb2t")
            nc.sync.dma_start(out=sb2t[:], in_=sb2)
            m1t = wp.tile([128, NR1], dt.bfloat16, name="m1t")
            nc.sync.dma_start(out=m1t[:], in_=m1)
            h1 = bp.tile([CIMG, NR1, 182], dt.bfloat16, name="h1")
            nc.vector.memset(h1[:, :, 0:1], 0.0)
            nc.vector.memset(h1[:, :, 181:182], 0.0)
            # ds1: stride-2 3x3; out row t reads slab rows 2t..2t+2 (slab row 0
            # = pooled row 2o0-3, so out row t (global o0-1+t) reads
            # 2(o0-1+t)-1..+1 - (2o0-3) = 2t..2t+2); col c reads 2c..2c+2
            RP = 2
            for t0 in range(0, NR1, RP):
                nr = min(RP, NR1 - t0)
                ps = pp.tile([CIMG, nr, 180], dt.float32, tag="ps1", name="ps")
                gi = 0
                for ky in range(3):
                    for kx in range(3):
                        rhs = bass.AP(slabt.tensor,
                                      slabt.offset + (2 * t0 + ky) * 362 + kx,
                                      [slabt.ap[0], [2 * 362, nr], [2, 180]])
                        nc.tensor.matmul(ps[:], w1[:, ky * 3 + kx, :], rhs,
                                         start=(gi == 0), stop=(gi == 8))
                        gi += 1
                ev = wp.tile([CIMG, nr, 180], dt.bfloat16, tag="ev", name="ev")
                nc.scalar.activation(ev[:], ps[:], RELU, bias=sb1t[:, 1:2],
                                     scale=sb1t[:, 0:1])
                mbb = bass.AP(m1t.tensor, m1t.offset + t0,
                              [[m1t.ap[0][0], CIMG], [1, nr], [0, 180]])
                nc.vector.tensor_tensor(out=h1[:, t0:t0 + nr, 1:181],
                                        in0=ev[:], in1=mbb,
                                        op=mybir.AluOpType.mult)
            # ds2: 3x3 pad 1: out row o reads h1 rows o..o+2, col c: c..c+2
            yo = bp.tile([CIMG, C_OUT_ROWS, 180], dt.float32, name="yo")
            for o0 in range(0, C_OUT_ROWS, RP):
                nr = min(RP, C_OUT_ROWS - o0)
                ps = pp.tile([CIMG, nr, 180], dt.float32, tag="ps2", name="ps")
                gi = 0
                for ky in range(3):
                    for kx in range(3):
                        rhs = bass.AP(h1.tensor,
                                      h1.offset + (o0 + ky) * 182 + kx,
                                      [h1.ap[0], [182, nr], [1, 180]])
                        nc.tensor.matmul(ps[:], w2[:, ky * 3 + kx, :], rhs,
                                         start=(gi == 0), stop=(gi == 8))
                        gi += 1
                nc.scalar.activation(yo[:, o0:o0 + nr, :], ps[:], RELU,
                                     bias=sb2t[:, 1:2], scale=sb2t[:, 0:1])
                nc.sync.dma_start(out=yout[:, o0:o0 + nr, :],
                                  in_=yo[:, o0:o0 + nr, :])
    nc.compile()
    return nc


_CACHE = {}


def run_launch_a(inputs):
    if "A" not in _CACHE:
        _CACHE["A"] = build_launch_a()
    nc = _CACHE["A"]
    maps = _prep_a_inputs(inputs)
    res = run_bass_kernel_spmd(nc, maps, list(range(NCORES)))
    depth = np.zeros((NPIX, DD), np.float32)
    feat = np.zeros((NPIX, CIMG), np.float32)
    for c in range(NCORES):
        r = res.results[c]
        for s, (cam, h0) in enumerate([SEG_A[c], SEG_B[c]]):
            S = SEGS[s]
            npix = S["nout"] * FW
            base = (cam * FH + h0) * FW
            off = 0 if s == 0 else 16 * FW
            depth[base:base + npix] = r["out_depth"][off:off + npix]
            feat[base:base + npix] = r["out_feat"][off:off + npix].astype(np.float32)
    return depth, feat


def _build_chunks(flat, kept, depth_rows):
    """Group points by (camera, column-block); per group build the
    [pix, voxel-slot] depth-weight matrix over the group's voxel union.
    Splits column blocks whose union exceeds the PSUM window (512)."""
    fl = flat.reshape(N, DD, FH, FW)
    kp = kept.reshape(N, DD, FH, FW)
    chunks = []                      # (pix_ids, Wdense[npix, nv], vox_ids)

    def add_group(n, w0, w1):
        nw = w1 - w0
        f = fl[n, :, :, w0:w1]                       # [DD, FH, nw]
        k = kp[n, :, :, w0:w1]
        vids = np.unique(f[k])
        if len(vids) > 512 and nw > 1:
            mid = w0 + nw // 2
            add_group(n, w0, mid)
            add_group(n, mid, w1)
            return
        nv = max(len(vids), 1)
        # pixel local idx = (w - w0) * FH + h; point (d, h, w)
        slot = np.searchsorted(vids, f[k]) if len(vids) else np.zeros(0, np.int64)
        dd, hh, ww = np.nonzero(k)
        pix_loc = ww * FH + hh
        pixcol = n * FH * FW + hh * FW + (ww + w0)
        dep = depth_rows[pixcol, dd]
        Wd = np.bincount(pix_loc * nv + slot, weights=dep,
                         minlength=nw * FH * nv).reshape(nw * FH, nv)
        pix_ids = (n * FH * FW + np.arange(FH)[None, :] * FW
                   + (w0 + np.arange(nw))[:, None]).reshape(-1)
        chunks.append((pix_ids, Wd, vids))

    for n in range(N):
        for w0 in range(0, FW, 4):
            add_group(n, w0, w0 + 4)
    return chunks


def _prep_b_inputs(chunks, featflat_bf):
    """Balance chunks across cores by window size; build per-core maps with
    the packed per-slot layout (chunk k size = max over cores, desc-sorted)."""
    order = sorted(range(len(chunks)), key=lambda i: -chunks[i][1].shape[1])
    load = np.zeros(NCORES, np.int64)
    per_core = [[] for _ in range(NCORES)]
    for i in order:
        c = int(np.argmin(load))
        per_core[c].append(i)
        load[c] += chunks[i][1].shape[1]
    NCH = max(len(p) for p in per_core)
    sizes = np.zeros(NCH, np.int64)
    for p in per_core:
        for k, i in enumerate(p):
            sizes[k] = max(sizes[k], chunks[i][1].shape[1])
    sizes = (sizes + 15) // 16 * 16
    offs = np.concatenate([[0], np.cumsum(sizes)]).astype(int)
    S = int(offs[-1])
    maps, scatter = [], []
    for c in range(NCORES):
        wm = np.zeros((128, S), bf16)
        ft = np.zeros((128, NCH, CIMG), bf16)
        sc = []
        for k, i in enumerate(per_core[c]):
            pix_ids, Wd, vids = chunks[i]
            npix, nv = Wd.shape
            wm[0:npix, offs[k]:offs[k] + nv] = Wd
            ft[0:npix, k, :] = featflat_bf[pix_ids]
            sc.append((int(offs[k]), vids))
        maps.append(dict(wmat=wm, feats=ft))
        scatter.append(sc)
    return maps, scatter, tuple(int(s) for s in sizes)


def _prep_c_inputs(inputs, pooled_t):
    """pooled_t: [CIMG, 360, 360] f32 -> per-core slabs + masks + weights."""
    NR1 = C_OUT_ROWS + 2
    NRP = 2 * NR1 + 1
    w1 = np.asarray(inputs["ds1_w"], np.float32)
    w2 = np.asarray(inputs["ds2_w"], np.float32)
    wd1 = np.stack([w1[:, :, ky, kx].T for ky in range(3) for kx in range(3)])
    wd2 = np.stack([w2[:, :, ky, kx].T for ky in range(3) for kx in range(3)])
    sb1 = np.stack([np.asarray(inputs["ds1_s"], np.float32),
                    np.asarray(inputs["ds1_t"], np.float32)], 1)
    sb2 = np.stack([np.asarray(inputs["ds2_s"], np.float32),
                    np.asarray(inputs["ds2_t"], np.float32)], 1)
    shared = dict(wd1=wd1.astype(bf16), wd2=wd2.astype(bf16), sb1=sb1, sb2=sb2)
    maps = []
    pt_bf = pooled_t.astype(bf16)
    for c in range(NCORES):
        o0g = C_OUT_ROWS * c
        p0 = 2 * o0g - 3
        slab = np.zeros((CIMG, NRP, 362), bf16)
        lo, hi = max(0, p0), min(NX, p0 + NRP)
        if hi > lo:
            slab[:, lo - p0:hi - p0, 1:361] = pt_bf[:, lo:hi, :]
        t1g = np.arange(NR1) + (o0g - 1)
        m1 = np.broadcast_to(((t1g >= 0) & (t1g < 180))[None, :],
                             (128, NR1)).astype(bf16)
        maps.append(dict(shared, slab=slab, m1=np.ascontiguousarray(m1)))
    return maps


def kernel(**inputs):
    inputs = {k: np.asarray(v) for k, v in inputs.items()}
    flat, kept = _host_geometry(inputs["cam2lidar_rots"],
                                inputs["cam2lidar_trans"], inputs["intrins"],
                                inputs["post_rots"], inputs["post_trans"])
    depth_rows, feat_rows = run_launch_a(inputs)
    featflat_bf = feat_rows.astype(bf16)

    chunks = _build_chunks(flat, kept, depth_rows)
    bmaps, scatter, sizes = _prep_b_inputs(chunks, featflat_bf)
    key = ("B", sizes)
    if key not in _CACHE:
        _CACHE[key] = build_launch_b(sizes)
    res_b = run_bass_kernel_spmd(_CACHE[key], bmaps, list(range(NCORES)))

    allvox = np.concatenate([vids for c in range(NCORES)
                             for _, vids in scatter[c]])
    allval = np.concatenate(
        [res_b.results[c]["owin"][:, o0:o0 + len(vids)].T.astype(np.float32)
         for c in range(NCORES) for o0, vids in scatter[c]])
    o = np.argsort(allvox, kind="stable")
    allvox, allval = allvox[o], allval[o]
    starts = np.flatnonzero(np.r_[True, allvox[1:] != allvox[:-1]])
    pooled = np.zeros((NX * NX, CIMG), np.float32)
    pooled[allvox[starts]] = np.add.reduceat(allval, starts, axis=0)
    pooled_t = np.ascontiguousarray(
        pooled.reshape(NX, NX, CIMG).transpose(2, 0, 1))

    if "C" not in _CACHE:
        _CACHE["C"] = build_launch_c()
    cmaps = _prep_c_inputs(inputs, pooled_t)
    res_c = run_bass_kernel_spmd(_CACHE["C"], cmaps, list(range(NCORES)))
    out = np.zeros((1, CIMG, 180, 180), np.float32)
    for c in range(NCORES):
        o0g = C_OUT_ROWS * c
        nr = min(C_OUT_ROWS, 180 - o0g)
        if nr > 0:
            out[0, :, o0g:o0g + nr, :] = res_c.results[c]["yout"][:, 0:nr, :]
    return out



# revision 16
# speedup vs baseline: 1.6911x; 1.0438x over previous
"""DepthLSSTransform Trainium kernel: 3 SPMD launches over 8 NeuronCores.

Launch A: per-camera conv pipeline (dtransform + depthnet + softmax) on
          24-row bands (one 16-row + one 8-row segment per core).
Launch B: bev_pool segment-sum via one-hot matmuls over a host-built
          virtual-window schedule (sorted-by-voxel points).
Launch C: BEV downsample convs, spatially sharded.
Host: geometry/voxel indices, scheduling, gathers, folds (orchestration).
"""
import numpy as np
import ml_dtypes

import concourse.bass as bass
import concourse.tile as tile
from concourse import bacc, mybir
from concourse.bass_utils import run_bass_kernel_spmd

dt = mybir.dt
bf16 = ml_dtypes.bfloat16

# ---- problem constants (hardcoded per contract) ----
B, N = 1, 6
CIN, CIMG, DD = 256, 80, 59
FH, FW, IH, IW = 32, 88, 256, 704
XY0, DXY, NX = -54.0, 0.3, 360
Z0, DZ, NZ = -10.0, 20.0, 1
NPTS = N * DD * FH * FW
NPIX = N * FH * FW
NCORES = 8
QV = 4                      # chunks of 128 points per virtual window

# per-core segments: (camera, h0) for seg A (16 rows) and seg B (8 rows)
SEG_A = [(0, 0), (1, 0), (1, 16), (2, 16), (3, 0), (4, 0), (4, 16), (5, 16)]
SEG_B = [(0, 16), (0, 24), (2, 0), (2, 8), (3, 16), (3, 24), (5, 0), (5, 8)]
# band pixel ranges in global row order (row = n*32 + h)
ROWS_OF_CORE = [[(SEG_A[c][0] * FH + SEG_A[c][1] + r) for r in range(16)] +
                [(SEG_B[c][0] * FH + SEG_B[c][1] + r) for r in range(8)]
                for c in range(NCORES)]

# segment geometry: rows16 segment: d rows [8h0-34, 8h0+158) (192), dt2 out
# rows [2h0-8, 2h0+39) (47), dt3 [h0-3, h0+19) (22), dn1 [h0-1, h0+17) (18)
SEGS = [dict(nout=16, nd=192, nq=48, nt2=47, nt3=22, nn1=18),
        dict(nout=8, nd=128, nq=32, nt2=31, nt3=14, nn1=10)]


def _seg_ranges(h0, S):
    return dict(d0=8 * h0 - 34, q0=2 * h0 - 8, t0=h0 - 3, r0=h0 - 1, o0=h0)


# ---------------------------------------------------------------- launch A
def build_launch_a(debug=False, psum_bufs=3, work_bufs=3, stages=9):
    nc = bacc.Bacc("TRN2", target_bir_lowering=False, debug=False,
                   num_devices=NCORES)
    AP = {}

    def inp(name, shape, dtype=dt.bfloat16):
        AP[name] = nc.dram_tensor(name, shape, dtype, kind="ExternalInput").ap()
        return AP[name]

    # per segment inputs (s = 0: 16-row, 1: 8-row)
    for s, S in enumerate(SEGS):
        inp(f"dph{s}", [128, S["nq"], 177])
        inp(f"masks{s}", [128, S["nt2"] + S["nt3"] + S["nn1"]])
        inp(f"xseg{s}", [2, 128, S["nt3"], 92])         # x_img slice (padded)
    # packed f32 constants: [alpha, beta, s_dt2, t_dt2, s_dt3, t_dt3,
    #  s_dn1(2), t_dn1(2), s_dn2(2), t_dn2(2), b_dn3(139)] -> [128, 153]
    inp("consts", [128, 153], dt.float32)
    # conv weights (host-prepped layouts)
    inp("w_dt2", [4, 128, 32])                          # groups (dky,dmx)
    inp("w_dt3", [9, 128, 64])
    inp("w_dn1", [9, 3, 128, 256])                      # tap, icchunk(128,128,64pad) -> 256
    inp("w_dn2", [9, 2, 128, 256])
    inp("w_dn3", [2, 128, 139])

    DBG = {}
    dbg_specs = [] if not debug else [("dbg_t1", [128, SEGS[0]["nq"], 177], dt.bfloat16),
                        ("dbg_dt2o", [32, SEGS[0]["nt2"] + 1, 180], dt.bfloat16),
                        ("dbg_dtc", [64, SEGS[0]["nt3"], 92], dt.bfloat16),
                        ("dbg_n1o", [128, SEGS[0]["nn1"], 92], dt.bfloat16),
                        ("dbg_n2o", [128, SEGS[0]["nout"], 88], dt.bfloat16)]
    for nm, sh, dty in dbg_specs:
        DBG[nm] = nc.dram_tensor(nm, sh, dty, kind="ExternalOutput").ap()
    out_depth = nc.dram_tensor("out_depth", [24 * FW, DD], dt.float32,
                               kind="ExternalOutput").ap()
    out_feat = nc.dram_tensor("out_feat", [24 * FW, CIMG], dt.bfloat16,
                              kind="ExternalOutput").ap()

    # HBM scratch
    scr = {}
    for s, S in enumerate(SEGS):
        scr[f"dt2o{s}"] = nc.dram_tensor(f"dt2o{s}", [32, S["nt2"] + 1, 2, 90], dt.bfloat16).ap()

    RELU = mybir.ActivationFunctionType.Relu
    with tile.TileContext(nc) as tc:
        with tc.tile_pool(name="const", bufs=1) as cpool, \
             tc.tile_pool(name="work", bufs=work_bufs) as wpool, \
             tc.tile_pool(name="big", bufs=1) as bpool, \
             tc.tile_pool(name="psum", bufs=psum_bufs, space="PSUM") as ppool:
            # ---- load packed constants in one DMA ----
            cts = cpool.tile([128, 153], dt.float32, name="cts")
            nc.sync.dma_start(out=cts[:], in_=AP["consts"])
            ct = {"dt1_alpha": cts[:, 0:1], "dt1_beta": cts[:, 1:2],
                  "s_dt2": cts[:, 2:3], "t_dt2": cts[:, 3:4],
                  "s_dt3": cts[:, 4:5], "t_dt3": cts[:, 5:6],
                  "s_dn1": cts[:, 6:8], "t_dn1": cts[:, 8:10],
                  "s_dn2": cts[:, 10:12], "t_dn2": cts[:, 12:14],
                  "b_dn3": cts[:, 14:153]}
            wt = {}
            for nm, pat in [("w_dt2", "g p o -> p g o"),
                            ("w_dt3", "g p o -> p g o"),
                            ("w_dn1", "t i p o -> p (t i) o"),
                            ("w_dn2", "t i p o -> p (t i) o"),
                            ("w_dn3", "g p o -> p g o")]:
                sh = list(AP[nm].shape)
                wt[nm] = cpool.tile([sh[-2], int(np.prod(sh[:-2])), sh[-1]],
                                    dt.bfloat16, tag=nm, name=f'wt_{nm}')
                nc.sync.dma_start(out=wt[nm][:], in_=AP[nm].rearrange(pat))

            feat_sb = {}
            depth_sb = {}
            for s, S in enumerate(SEGS):
                nq, nt2, nt3, nn1, nout = S["nq"], S["nt2"], S["nt3"], S["nn1"], S["nout"]
                # ======== dt1: relu(alpha*d + beta) on Act; host bakes pad
                # values into dph so relu zeroes out-of-image positions ====
                dph = bpool.tile([128, nq, 177], dt.bfloat16, tag=f"dph{s}")
                for qq in range(0, nq, nq // 4):
                    nqq = min(nq // 4, nq - qq)
                    nc.sync.dma_start(out=dph[:, qq:qq + nqq, :],
                                      in_=AP[f"dph{s}"][:, qq:qq + nqq, :])
                t1 = bpool.tile([128, nq, 177], dt.bfloat16, tag=f"t1{s}")
                mall = wpool.tile([128, nt2 + nt3 + nn1], dt.bfloat16,
                                  tag=f"msk{s}", name="mall")
                nc.sync.dma_start(out=mall[:], in_=AP[f"masks{s}"])
                QCH = nq // 4
                for qq in range(0, nq, QCH):
                    nqq = min(QCH, nq - qq)
                    sl = (slice(None), slice(qq, qq + nqq), slice(None))
                    nc.scalar.activation(t1[sl], dph[sl], RELU,
                                         bias=ct["dt1_beta"][:, 0:1],
                                         scale=ct["dt1_alpha"][:, 0:1])
                if s == 0 and debug:
                    nc.sync.dma_start(out=DBG["dbg_t1"], in_=t1[:])

                if stages < 2:
                    continue
                # ======== dt2 ========
                o2 = bpool.tile([32, nt2 + 1, 180], dt.bfloat16, tag=f"o2{s}")
                # border strips only: cols 0, 89-90, 179 and the pad row nt2
                nc.vector.memset(o2[:, :, 0:1], 0.0)
                nc.vector.memset(o2[:, :, 89:91], 0.0)
                nc.vector.memset(o2[:, :, 179:180], 0.0)
                nc.vector.memset(o2[:, nt2:nt2 + 1, :], 0.0)
                m2 = bass.AP(mall.tensor, mall.offset, [mall.ap[0], [1, nt2]])
                RPP2 = 2
                for q0 in range(0, nt2, RPP2):
                    nr = min(RPP2, nt2 - q0)
                    ps = ppool.tile([32, nr, 176], dt.float32, tag=f"ps{s}", name="ps2")
                    gi = 0
                    for dky in range(2):
                        for dmx in range(2):
                            g = dky * 2 + dmx
                            rhs = bass.AP(
                                t1.tensor, t1.offset + (q0 + dky) * 177 + dmx,
                                [t1.ap[0], [177, nr], [1, 176]])
                            nc.tensor.matmul(ps[:], wt["w_dt2"][:, g, :], rhs,
                                             start=(gi == 0), stop=(gi == 3))
                            gi += 1
                    ev = wpool.tile([32, nr, 176], dt.bfloat16, tag=f"ev2{s}")
                    nc.scalar.activation(ev[:], ps[:], RELU,
                                         bias=ct["t_dt2"][0:32, 0:1],
                                         scale=ct["s_dt2"][0:32, 0:1])
                    mbb = bass.AP(m2.tensor, m2.offset + q0,
                                  [[m2.ap[0][0], 32], [1, nr], [0, 176]])
                    # write col c at (c%2)*90 + c//2 + 1  (phase-split layout)
                    o2dst = bass.AP(o2.tensor, o2.offset + q0 * 180 + 1,
                                    [[o2.ap[0][0], 32], [180, nr],
                                     [1, 88], [90, 2]])
                    nc.vector.tensor_tensor(out=o2dst, in0=ev[:], in1=mbb,
                                            op=mybir.AluOpType.mult)
                nc.sync.dma_start(out=scr[f"dt2o{s}"],
                                  in_=o2.rearrange("p q (b x) -> p q b x", b=2))
                if s == 0 and debug:
                    nc.sync.dma_start(out=DBG["dbg_dt2o"], in_=o2[:])

                if stages < 3:
                    continue
                # ======== dt3 ========
                nry3 = nt3 + 2
                ph3 = bpool.tile([128, nry3, 90], dt.bfloat16, tag=f"ph3{s}")
                sd2 = scr[f"dt2o{s}"]
                for a2 in range(2):
                    for b2 in range(2):
                        pap3 = bass.AP(sd2.tensor,
                                       sd2.offset + a2 * 180 + b2 * 90,
                                       [[(nt2 + 1) * 180, 32],
                                        [2 * 180, nry3], [1, 90]])
                        nc.sync.dma_start(
                            out=ph3[(a2 * 2 + b2) * 32:(a2 * 2 + b2 + 1) * 32],
                            in_=pap3)
                # concat input tile: [64 dt3 | pad] plus x_img tiles
                dtc = bpool.tile([64, nt3, 92], dt.bfloat16, tag=f"dtc{s}")
                nc.vector.memset(dtc[:, :, 0:2], 0.0)
                nc.vector.memset(dtc[:, :, 90:92], 0.0)
                m3 = bass.AP(mall.tensor, mall.offset + nt2,
                             [mall.ap[0], [1, nt3]])
                RPP3 = 4
                for t0 in range(0, nt3, RPP3):
                    nr = min(RPP3, nt3 - t0)
                    ps = ppool.tile([64, nr, 88], dt.float32, tag=f"ps{s}")
                    gi = 0
                    for dky in range(3):
                        for dmx in range(3):
                            g = dky * 3 + dmx
                            rhs = bass.AP(ph3.tensor,
                                          ph3.offset + (t0 + dky) * 90 + dmx,
                                          [ph3.ap[0], [90, nr], [1, 88]])
                            nc.tensor.matmul(ps[:], wt["w_dt3"][:, g, :], rhs,
                                             start=(gi == 0), stop=(gi == 8))
                            gi += 1
                    ev = wpool.tile([64, nr, 88], dt.bfloat16, tag=f"ev3{s}")
                    nc.scalar.activation(ev[:], ps[:], RELU,
                                         bias=ct["t_dt3"][0:64, 0:1],
                                         scale=ct["s_dt3"][0:64, 0:1])
                    mbb = bass.AP(m3.tensor, m3.offset + t0,
                                  [m3.ap[0], [1, nr], [0, 88]])
                    nc.vector.tensor_tensor(out=dtc[:, t0:t0 + nr, 2:90],
                                            in0=ev[:], in1=mbb[0:64],
                                            op=mybir.AluOpType.mult)

                if s == 0 and debug:
                    nc.sync.dma_start(out=DBG["dbg_dtc"], in_=dtc[:])
                if stages < 4:
                    continue
                # ======== dn1 ========
                xs = []
                for g in range(2):
                    xt = bpool.tile([128, nt3, 92], dt.bfloat16, tag=f"x{g}_{s}",
                                     name=f"xseg_t{g}")
                    nc.sync.dma_start(out=xt[:], in_=AP[f"xseg{s}"][g])
                    xs.append(xt)
                mn1 = bass.AP(mall.tensor, mall.offset + nt2 + nt3,
                              [mall.ap[0], [1, nn1]])
                n1o = []
                for g in range(2):
                    t = bpool.tile([128, nn1, 92], dt.bfloat16, tag=f"n1o{g}_{s}")
                    nc.vector.memset(t[:, :, 0:2], 0.0)
                    nc.vector.memset(t[:, :, 90:92], 0.0)
                    n1o.append(t)
                RPP = 5
                for ocg in range(2):
                    for r0 in range(0, nn1, RPP):
                        nr = min(RPP, nn1 - r0)
                        ps = ppool.tile([128, nr, 88], dt.float32, tag=f"ps{s}")
                        gi = 0
                        for ky in range(3):
                            for kx in range(3):
                                tap = ky * 3 + kx
                                for icc, srcT in enumerate((xs[0], xs[1], dtc)):
                                    kk = 128 if icc < 2 else 64
                                    rhs = bass.AP(
                                        srcT.tensor,
                                        srcT.offset + (r0 + ky + 1) * 92 + kx + 1,
                                        [srcT.ap[0], [92, nr], [1, 88]])
                                    lhs = wt["w_dn1"][0:kk, tap * 3 + icc,
                                                      ocg * 128:(ocg + 1) * 128]
                                    nc.tensor.matmul(ps[:], lhs, rhs,
                                                     start=(gi == 0),
                                                     stop=(gi == 26))
                                    gi += 1
                        ev = wpool.tile([128, nr, 88], dt.bfloat16, tag=f"evn1{s}")
                        nc.scalar.activation(ev[:], ps[:], RELU,
                                             bias=ct["t_dn1"][:, ocg:ocg + 1],
                                             scale=ct["s_dn1"][:, ocg:ocg + 1])
                        mbb = bass.AP(mn1.tensor, mn1.offset + r0,
                                      [mn1.ap[0], [1, nr], [0, 88]])
                        nc.vector.tensor_tensor(
                            out=n1o[ocg][:, r0:r0 + nr, 2:90],
                            in0=ev[:], in1=mbb, op=mybir.AluOpType.mult)

                if s == 0 and debug:
                    nc.sync.dma_start(out=DBG["dbg_n1o"], in_=n1o[0][:])
                if stages < 5:
                    continue
                # ======== dn2 ========
                n2o = []
                for g in range(2):
                    n2o.append(bpool.tile([128, nout, 88], dt.bfloat16,
                                          tag=f"n2o{g}_{s}", name=f"n2o{g}"))
                for ocg in range(2):
                    for r0 in range(0, nout, RPP):
                        nr = min(RPP, nout - r0)
                        ps = ppool.tile([128, nr, 88], dt.float32, tag=f"ps{s}")
                        gi = 0
                        for ky in range(3):
                            for kx in range(3):
                                tap = ky * 3 + kx
                                for icc in range(2):
                                    rhs = bass.AP(
                                        n1o[icc].tensor,
                                        n1o[icc].offset + (r0 + ky) * 92 + kx + 1,
                                        [n1o[icc].ap[0], [92, nr], [1, 88]])
                                    lhs = wt["w_dn2"][:, tap * 2 + icc,
                                                      ocg * 128:(ocg + 1) * 128]
                                    nc.tensor.matmul(ps[:], lhs, rhs,
                                                     start=(gi == 0),
                                                     stop=(gi == 17))
                                    gi += 1
                        ev = wpool.tile([128, nr, 88], dt.bfloat16, tag=f"evn2{s}")
                        nc.scalar.activation(ev[:], ps[:], RELU,
                                             bias=ct["t_dn2"][:, ocg:ocg + 1],
                                             scale=ct["s_dn2"][:, ocg:ocg + 1])
                        nc.vector.tensor_copy(n2o[ocg][:, r0:r0 + nr, :], ev[:])

                if s == 0 and debug:
                    nc.sync.dma_start(out=DBG["dbg_n2o"], in_=n2o[0][:])
                if stages < 6:
                    continue
                # ======== dn3 + softmax + feat ========
                npix = nout * FW
                feat_sb[s] = bpool.tile([128, (npix + 127) // 128, CIMG],
                                        dt.bfloat16, tag=f"feat{s}", name=f"feat_sb{s}")
                depth_sb[s] = bpool.tile([128, (npix + 127) // 128, DD],
                                         dt.float32, tag=f"depth{s}", name=f"depth_sb{s}")
                n2f = [t.rearrange("p a b -> p (a b)") for t in n2o]
                for pc in range((npix + 127) // 128):
                    m = min(128, npix - pc * 128)
                    ps = ppool.tile([m, 139], dt.float32, tag=f"ps{s}")
                    for icc in range(2):
                        nc.tensor.matmul(ps[:], n2f[icc][:, pc * 128:pc * 128 + m],
                                         wt["w_dn3"][:, icc, :],
                                         start=(icc == 0), stop=(icc == 1))
                    # add bias via vector then softmax over first 59
                    lg = wpool.tile([m, 139], dt.float32, tag=f"lg{s}")
                    nc.vector.tensor_tensor(out=lg[:], in0=ps[:],
                                            in1=ct["b_dn3"][0:m],
                                            op=mybir.AluOpType.add)
                    mx = wpool.tile([m, 1], dt.float32, tag=f"mx{s}")
                    nc.vector.reduce_max(mx[:], lg[:, 0:DD],
                                         axis=mybir.AxisListType.X, negate=True)
                    ex = wpool.tile([m, DD], dt.float32, tag=f"ex{s}")
                    nc.scalar.activation(ex[:], lg[:, 0:DD],
                                         mybir.ActivationFunctionType.Exp,
                                         bias=mx[:, 0:1], scale=1.0)
                    sm = wpool.tile([m, 1], dt.float32, tag=f"sm{s}")
                    nc.vector.reduce_sum(sm[:], ex[:], axis=mybir.AxisListType.X)
                    rc = wpool.tile([m, 1], dt.float32, tag=f"rc{s}")
                    nc.vector.reciprocal(rc[:], sm[:])
                    nc.vector.tensor_scalar(out=depth_sb[s][0:m, pc, :], in0=ex[:],
                                            scalar1=rc[:, 0:1], scalar2=None,
                                            op0=mybir.AluOpType.mult)
                    nc.vector.tensor_copy(feat_sb[s][0:m, pc, :],
                                          lg[:, DD:DD + CIMG])

            # DMA outputs: global pix index = seg-A pix then seg-B pix
            for s, S in (enumerate(SEGS) if stages >= 6 else []):
                npix = S["nout"] * FW
                base = 0 if s == 0 else 16 * FW
                nfull = npix // 128
                dsl = out_depth[base:base + nfull * 128].rearrange(
                    "(a p) d -> p a d", p=128)
                nc.sync.dma_start(out=dsl, in_=depth_sb[s][:, 0:nfull, :])
                fsl = out_feat[base:base + nfull * 128].rearrange(
                    "(a p) d -> p a d", p=128)
                nc.sync.dma_start(out=fsl, in_=feat_sb[s][:, 0:nfull, :])
                rem = npix - nfull * 128
                if rem:
                    nc.sync.dma_start(
                        out=out_depth[base + nfull * 128:base + npix],
                        in_=depth_sb[s][0:rem, nfull, :])
                    nc.sync.dma_start(
                        out=out_feat[base + nfull * 128:base + npix],
                        in_=feat_sb[s][0:rem, nfull, :])
    nc.compile()
    return nc


# ------------------------------------------------------------ host helpers
def _host_geometry(rots, trans, intr, post_rots, post_trans):
    import jax
    import jax.numpy as jnp
    with jax.default_device(jax.devices("cpu")[0]):
        f32 = jnp.float32
        ds = jnp.arange(1.0, 60.0, 1.0, dtype=f32)
        xs = jnp.linspace(0.0, IW - 1.0, FW, dtype=f32)
        ys = jnp.linspace(0.0, IH - 1.0, FH, dtype=f32)
        dm = jnp.broadcast_to(ds[:, None, None], (DD, FH, FW))
        xm = jnp.broadcast_to(xs[None, None, :], (DD, FH, FW))
        ym = jnp.broadcast_to(ys[None, :, None], (DD, FH, FW))
        fr = jnp.stack([xm, ym, dm], -1)
        pts = fr[None, None] - jnp.asarray(post_trans)[:, :, None, None, None, :]
        pts = jnp.einsum("bnij,bndhwj->bndhwi",
                         jnp.linalg.inv(jnp.asarray(post_rots)), pts)
        pts = jnp.concatenate([pts[..., :2] * pts[..., 2:3], pts[..., 2:3]], -1)
        comb = jnp.einsum("bnij,bnjk->bnik", jnp.asarray(rots),
                          jnp.linalg.inv(jnp.asarray(intr)))
        pts = jnp.einsum("bnij,bndhwj->bndhwi", comb, pts) \
            + jnp.asarray(trans)[:, :, None, None, None, :]
        lo = jnp.array([XY0, XY0, Z0], dtype=f32)
        dxv = jnp.array([DXY, DXY, DZ], dtype=f32)
        g = ((pts - lo) / dxv).astype(jnp.int32).reshape(-1, 3)
        kept = ((g[:, 0] >= 0) & (g[:, 0] < NX) & (g[:, 1] >= 0) & (g[:, 1] < NX)
                & (g[:, 2] >= 0) & (g[:, 2] < NZ))
        flat = (g[:, 2] * NX + g[:, 0]) * NX + g[:, 1]
        return np.asarray(flat, np.int64), np.asarray(kept)


def _prep_a_inputs(inputs):
    """Build per-core input maps for launch A."""
    d = np.asarray(inputs["d"], np.float32).reshape(N, IH, IW)
    x_img = np.asarray(inputs["x_img"], np.float32)

    # dt1 folded affine: relu(alpha*d + beta), alpha = s*w, beta = s*b + t
    a1 = (inputs["dt1_s"] * inputs["dt1_w"][:, 0, 0, 0]).astype(np.float32)
    b1 = (inputs["dt1_s"] * inputs["dt1_b"] + inputs["dt1_t"]).astype(np.float32)
    cab = np.arange(128)
    dt1_alpha = a1[cab // 16][:, None]
    dt1_beta = b1[cab // 16][:, None]

    def wprep_dt2():
        w = np.asarray(inputs["dt2_w"], np.float32)      # [32,8,5,5]
        out = np.zeros((4, 128, 32), np.float32)
        for ky in range(5):
            for kx in range(5):
                a, dky = ky % 4, ky // 4
                bph, dmx = (kx + 2) % 4, (kx + 2) // 4
                g = dky * 2 + dmx
                rows = (np.arange(8)) * 16 + a * 4 + bph
                out[g, rows, :] = w[:, :, ky, kx].T
        return out.astype(bf16)

    def wprep_dt3():
        w = np.asarray(inputs["dt3_w"], np.float32)      # [64,32,5,5]
        out = np.zeros((9, 128, 64), np.float32)
        for ky in range(5):
            for kx in range(5):
                a, dky = ky % 2, ky // 2
                bph, dmx = kx % 2, (kx + 2) // 2 - 1
                g = dky * 3 + dmx
                rows = (a * 2 + bph) * 32 + np.arange(32)
                out[g, rows, :] = w[:, :, ky, kx].T
        return out.astype(bf16)

    def wprep_3x3(w, icc_sizes):
        O, I = w.shape[0], w.shape[1]
        nic = len(icc_sizes)
        out = np.zeros((9, nic, 128, O), np.float32)
        for ky in range(3):
            for kx in range(3):
                tap = ky * 3 + kx
                ic0 = 0
                for icc, sz in enumerate(icc_sizes):
                    out[tap, icc, 0:sz, :] = w[:, ic0:ic0 + sz, ky, kx].T
                    ic0 += sz
        return out.astype(bf16)

    # NOTE: dn1 input concat order is [dt3(64) | x_img(256)] in the reference;
    # our matmul chunks are (x0:128, x1:128, dt3:64) -> weight cols must match:
    w_dn1_full = np.asarray(inputs["dn1_w"], np.float32)
    w_dn1 = np.zeros((9, 3, 128, 256), np.float32)
    for ky in range(3):
        for kx in range(3):
            tap = ky * 3 + kx
            w_dn1[tap, 0, :, :] = w_dn1_full[:, 64:192, ky, kx].T
            w_dn1[tap, 1, :, :] = w_dn1_full[:, 192:320, ky, kx].T
            w_dn1[tap, 2, 0:64, :] = w_dn1_full[:, 0:64, ky, kx].T
    w_dn1 = w_dn1.astype(bf16)
    w_dn2 = wprep_3x3(np.asarray(inputs["dn2_w"], np.float32), [128, 128])
    w_dn3 = np.asarray(inputs["dn3_w"], np.float32)[:, :, 0, 0]  # [139, 256]
    w_dn3p = np.zeros((2, 128, 139), np.float32)
    w_dn3p[0] = w_dn3[:, 0:128].T
    w_dn3p[1] = w_dn3[:, 128:256].T

    def fold_bias(b, s, t):
        # conv bias b then bn scale/shift: relu(s*(x+b) + t) = relu(s*x + (s*b+t))
        return np.asarray(s, np.float32), np.asarray(s * b + t, np.float32)

    s2, t2 = fold_bias(inputs["dt2_b"], inputs["dt2_s"], inputs["dt2_t"])
    s3, t3 = fold_bias(inputs["dt3_b"], inputs["dt3_s"], inputs["dt3_t"])
    sn1, tn1 = fold_bias(inputs["dn1_b"], inputs["dn1_s"], inputs["dn1_t"])
    sn2, tn2 = fold_bias(inputs["dn2_b"], inputs["dn2_s"], inputs["dn2_t"])
    b_dn3 = np.broadcast_to(np.asarray(inputs["dn3_b"], np.float32)[None, :],
                            (128, 139)).copy()

    consts = np.zeros((128, 153), np.float32)
    consts[:, 0] = dt1_alpha[:, 0]
    consts[:, 1] = dt1_beta[:, 0]
    consts[:, 2] = np.tile(s2, 4)
    consts[:, 3] = np.tile(t2, 4)
    consts[:, 4] = np.tile(s3, 2)
    consts[:, 5] = np.tile(t3, 2)
    consts[:, 6:8] = sn1.reshape(2, 128).T
    consts[:, 8:10] = tn1.reshape(2, 128).T
    consts[:, 10:12] = sn2.reshape(2, 128).T
    consts[:, 12:14] = tn2.reshape(2, 128).T
    consts[:, 14:153] = b_dn3
    shared = dict(
        consts=consts,
        w_dt2=wprep_dt2(), w_dt3=wprep_dt3(), w_dn1=w_dn1, w_dn2=w_dn2,
        w_dn3=w_dn3p.astype(bf16),
    )

    # per-channel pad value: alpha*v + beta <= -|alpha|*1e8 < 0 -> relu -> 0
    assert np.abs(a1).min() > 1e-5, "dt1 alpha too small for pad-value trick"
    vpad = (-np.sign(a1) * 1e8).astype(np.float32)           # [8] per channel

    maps = []
    for c in range(NCORES):
        m = dict(shared)
        for s, (cam, h0) in enumerate([SEG_A[c], SEG_B[c]]):
            S = SEGS[s]
            d0 = 8 * h0 - 34
            dseg = np.zeros((S["nd"], 712), np.float32)
            vseg = np.zeros((S["nd"], 712), bool)
            lo, hi = max(0, d0), min(IH, d0 + S["nd"])
            if hi > lo:
                dseg[lo - d0:hi - d0, 4:708] = d[cam, lo:hi]
                vseg[lo - d0:hi - d0, 4:708] = True
            nq = S["nq"]
            ph = dseg.reshape(nq, 4, 178, 4)[:, :, :177, :]     # ry a rx b
            ph = ph.transpose(1, 3, 0, 2)                        # a b ry rx
            vph = vseg.reshape(nq, 4, 178, 4)[:, :, :177, :].transpose(1, 3, 0, 2)
            dphc = np.where(vph[None], ph[None],
                            vpad[:, None, None, None, None])    # [8,4,4,nq,177]
            m[f"dph{s}"] = dphc.reshape(128, nq, 177).astype(bf16)
            q0, t0, r0 = 2 * h0 - 8, h0 - 3, h0 - 1
            qr = np.arange(S["nt2"]) + q0
            m2m = np.broadcast_to(((qr >= 0) & (qr < 64))[None, :],
                                  (128, S["nt2"]))
            tr = np.arange(S["nt3"]) + t0
            m3m = np.broadcast_to(((tr >= 0) & (tr < FH))[None, :],
                                  (128, S["nt3"]))
            rr = np.arange(S["nn1"]) + r0
            mn1m = np.broadcast_to(((rr >= 0) & (rr < FH))[None, :],
                                   (128, S["nn1"]))
            m[f"masks{s}"] = np.concatenate(
                [m2m, m3m, mn1m], axis=1).astype(bf16)
            xseg = np.zeros((2, 128, S["nt3"], 92), np.float32)
            lo2, hi2 = max(0, t0), min(FH, t0 + S["nt3"])
            if hi2 > lo2:
                xseg[:, :, lo2 - t0:hi2 - t0, 2:90] = \
                    x_img[cam, :, lo2:hi2, :].reshape(2, 128, hi2 - lo2, FW)
            m[f"xseg{s}"] = xseg.astype(bf16)
        maps.append(m)
    return maps


# ---------------------------------------------------------------- launch B
def build_launch_b(sizes):
    """Per chunk k: [128pix x 80ch] stationary feat tile x host-built
    [128pix x sizes[k] voxel-slot] depth-weight matrix -> [80, nv] window
    sums. W and out use packed (variable-size) layouts; W loads in a few
    batched DMAs, out in one."""
    nc = bacc.Bacc("TRN2", target_bir_lowering=False, debug=False,
                   num_devices=NCORES)
    NCH = len(sizes)
    offs = np.concatenate([[0], np.cumsum(sizes)]).astype(int)
    S = int(offs[-1])
    wmat = nc.dram_tensor("wmat", [128, S], dt.bfloat16,
                          kind="ExternalInput").ap()
    feats = nc.dram_tensor("feats", [128, NCH, CIMG], dt.bfloat16,
                           kind="ExternalInput").ap()
    owin = nc.dram_tensor("owin", [CIMG, S], dt.bfloat16,
                          kind="ExternalOutput").ap()
    NB = 4                                   # W DMA batches
    bnd = [int(round(NCH * i / NB)) for i in range(NB + 1)]
    with tile.TileContext(nc) as tc:
        with tc.tile_pool(name="const", bufs=1) as cpool, \
             tc.tile_pool(name="ps", bufs=4, space="PSUM") as pp:
            ft = cpool.tile([128, NCH, CIMG], dt.bfloat16, name="ft")
            nc.sync.dma_start(out=ft[:], in_=feats)
            wt = cpool.tile([128, S], dt.bfloat16, name="wt")
            for b in range(NB):
                lo, hi = offs[bnd[b]], offs[bnd[b + 1]]
                if hi > lo:
                    nc.sync.dma_start(out=wt[:, lo:hi], in_=wmat[:, lo:hi])
            ot = cpool.tile([CIMG, S], dt.bfloat16, name="ot")
            for k in range(NCH):
                nv, o0 = int(sizes[k]), int(offs[k])
                ps = pp.tile([CIMG, 512], dt.float32, tag="ps", name="ps")
                nc.tensor.matmul(ps[:, 0:nv], ft[:, k, :], wt[:, o0:o0 + nv],
                                 start=True, stop=True)
                if k % 2 == 0:
                    nc.scalar.activation(ot[:, o0:o0 + nv], ps[:, 0:nv],
                                         mybir.ActivationFunctionType.Copy)
                else:
                    nc.vector.tensor_copy(ot[:, o0:o0 + nv], ps[:, 0:nv])
            nc.sync.dma_start(out=owin, in_=ot[:])
    nc.compile()
    return nc


# ---------------------------------------------------------------- launch C
C_OUT_ROWS = 23              # ds2-out rows per core (8*23 = 184 >= 180)


def build_launch_c():
    nc = bacc.Bacc("TRN2", target_bir_lowering=False, debug=False,
                   num_devices=NCORES)
    NR1 = C_OUT_ROWS + 2                         # ds1-out rows incl halo (25)
    NRP = 2 * NR1 + 1                            # pooled rows needed (51)
    slab = nc.dram_tensor("slab", [CIMG, NRP, 362], dt.bfloat16,
                          kind="ExternalInput").ap()
    m1 = nc.dram_tensor("m1", [128, NR1], dt.bfloat16, kind="ExternalInput").ap()
    wd1 = nc.dram_tensor("wd1", [9, CIMG, CIMG], dt.bfloat16,
                         kind="ExternalInput").ap()
    wd2 = nc.dram_tensor("wd2", [9, CIMG, CIMG], dt.bfloat16,
                         kind="ExternalInput").ap()
    sb1 = nc.dram_tensor("sb1", [CIMG, 2], dt.float32, kind="ExternalInput").ap()
    sb2 = nc.dram_tensor("sb2", [CIMG, 2], dt.float32, kind="ExternalInput").ap()
    yout = nc.dram_tensor("yout", [CIMG, C_OUT_ROWS, 180], dt.float32,
                          kind="ExternalOutput").ap()
    RELU = mybir.ActivationFunctionType.Relu
    with tile.TileContext(nc) as tc:
        with tc.tile_pool(name="const", bufs=1) as cpool,              tc.tile_pool(name="work", bufs=2) as wp,              tc.tile_pool(name="big", bufs=1) as bp,              tc.tile_pool(name="ps", bufs=3, space="PSUM") as pp:
            # weights/consts first so ds1 can start on the first slab chunk
            w1 = cpool.tile([CIMG, 9, CIMG], dt.bfloat16, name="w1")
            nc.sync.dma_start(out=w1[:], in_=wd1.rearrange("t p o -> p t o"))
            sb1t = cpool.tile([CIMG, 2], dt.float32, name="sb1t")
            nc.sync.dma_start(out=sb1t[:], in_=sb1)
            m1t = wp.tile([128, NR1], dt.bfloat16, name="m1t")
            nc.sync.dma_start(out=m1t[:], in_=m1)
            slabt = bp.tile([CIMG, NRP, 362], dt.bfloat16, name="slabt")
            for rr in range(0, NRP, 9):
                nrr = min(9, NRP - rr)
                nc.sync.dma_start(out=slabt[:, rr:rr + nrr, :],
                                  in_=slab[:, rr:rr + nrr, :])
            w2 = cpool.tile([CIMG, 9, CIMG], dt.bfloat16, name="w2")
            nc.sync.dma_start(out=w2[:], in_=wd2.rearrange("t p o -> p t o"))
            sb2t = cpool.tile([CIMG, 2], dt.float32, name="sb2t")
            nc.sync.dma_start(out=sb2t[:], in_=sb2)
            h1 = bp.tile([CIMG, NR1, 182], dt.bfloat16, name="h1")
            nc.vector.memset(h1[:, :, 0:1], 0.0)
            nc.vector.memset(h1[:, :, 181:182], 0.0)
            # ds1: stride-2 3x3; out row t reads slab rows 2t..2t+2 (slab row 0
            # = pooled row 2o0-3, so out row t (global o0-1+t) reads
            # 2(o0-1+t)-1..+1 - (2o0-3) = 2t..2t+2); col c reads 2c..2c+2
            RP = 2
            for t0 in range(0, NR1, RP):
                nr = min(RP, NR1 - t0)
                ps = pp.tile([CIMG, nr, 180], dt.float32, tag="ps1", name="ps")
                gi = 0
                for ky in range(3):
                    for kx in range(3):
                        rhs = bass.AP(slabt.tensor,
                                      slabt.offset + (2 * t0 + ky) * 362 + kx,
                                      [slabt.ap[0], [2 * 362, nr], [2, 180]])
                        nc.tensor.matmul(ps[:], w1[:, ky * 3 + kx, :], rhs,
                                         start=(gi == 0), stop=(gi == 8))
                        gi += 1
                ev = wp.tile([CIMG, nr, 180], dt.bfloat16, tag="ev", name="ev")
                nc.scalar.activation(ev[:], ps[:], RELU, bias=sb1t[:, 1:2],
                                     scale=sb1t[:, 0:1])
                mbb = bass.AP(m1t.tensor, m1t.offset + t0,
                              [[m1t.ap[0][0], CIMG], [1, nr], [0, 180]])
                nc.vector.tensor_tensor(out=h1[:, t0:t0 + nr, 1:181],
                                        in0=ev[:], in1=mbb,
                                        op=mybir.AluOpType.mult)
            # ds2: 3x3 pad 1: out row o reads h1 rows o..o+2, col c: c..c+2
            yo = bp.tile([CIMG, C_OUT_ROWS, 180], dt.float32, name="yo")
            for o0 in range(0, C_OUT_ROWS, RP):
                nr = min(RP, C_OUT_ROWS - o0)
                ps = pp.tile([CIMG, nr, 180], dt.float32, tag="ps2", name="ps")
                gi = 0
                for ky in range(3):
                    for kx in range(3):
                        rhs = bass.AP(h1.tensor,
                                      h1.offset + (o0 + ky) * 182 + kx,
                                      [h1.ap[0], [182, nr], [1, 180]])
                        nc.tensor.matmul(ps[:], w2[:, ky * 3 + kx, :], rhs,
                                         start=(gi == 0), stop=(gi == 8))
                        gi += 1
                nc.scalar.activation(yo[:, o0:o0 + nr, :], ps[:], RELU,
                                     bias=sb2t[:, 1:2], scale=sb2t[:, 0:1])
                if (o0 // RP) % 3 == 2 or o0 + nr >= C_OUT_ROWS:
                    lo = (o0 // (3 * RP)) * 3 * RP
                    nc.sync.dma_start(out=yout[:, lo:o0 + nr, :],
                                      in_=yo[:, lo:o0 + nr, :])
    nc.compile()
    return nc


_CACHE = {}


def run_launch_a(inputs):
    if "A" not in _CACHE:
        _CACHE["A"] = build_launch_a()
    nc = _CACHE["A"]
    maps = _prep_a_inputs(inputs)
    res = run_bass_kernel_spmd(nc, maps, list(range(NCORES)))
    depth = np.zeros((NPIX, DD), np.float32)
    feat = np.zeros((NPIX, CIMG), np.float32)
    for c in range(NCORES):
        r = res.results[c]
        for s, (cam, h0) in enumerate([SEG_A[c], SEG_B[c]]):
            S = SEGS[s]
            npix = S["nout"] * FW
            base = (cam * FH + h0) * FW
            off = 0 if s == 0 else 16 * FW
            depth[base:base + npix] = r["out_depth"][off:off + npix]
            feat[base:base + npix] = r["out_feat"][off:off + npix].astype(np.float32)
    return depth, feat


def _build_chunks(flat, kept, depth_rows):
    """Group points by (camera, column-block); per group build the
    [pix, voxel-slot] depth-weight matrix over the group's voxel union.
    Splits column blocks whose union exceeds the PSUM window (512)."""
    fl = flat.reshape(N, DD, FH, FW)
    kp = kept.reshape(N, DD, FH, FW)
    chunks = []                      # (pix_ids, Wdense[npix, nv], vox_ids)

    def add_group(n, w0, w1):
        nw = w1 - w0
        f = fl[n, :, :, w0:w1]                       # [DD, FH, nw]
        k = kp[n, :, :, w0:w1]
        vids = np.unique(f[k])
        if len(vids) > 512 and nw > 1:
            mid = w0 + nw // 2
            add_group(n, w0, mid)
            add_group(n, mid, w1)
            return
        nv = max(len(vids), 1)
        # pixel local idx = (w - w0) * FH + h; point (d, h, w)
        slot = np.searchsorted(vids, f[k]) if len(vids) else np.zeros(0, np.int64)
        dd, hh, ww = np.nonzero(k)
        pix_loc = ww * FH + hh
        pixcol = n * FH * FW + hh * FW + (ww + w0)
        dep = depth_rows[pixcol, dd]
        Wd = np.bincount(pix_loc * nv + slot, weights=dep,
                         minlength=nw * FH * nv).reshape(nw * FH, nv)
        pix_ids = (n * FH * FW + np.arange(FH)[None, :] * FW
                   + (w0 + np.arange(nw))[:, None]).reshape(-1)
        chunks.append((pix_ids, Wd, vids))

    for n in range(N):
        for w0 in range(0, FW, 4):
            add_group(n, w0, w0 + 4)
    return chunks


def _prep_b_inputs(chunks, featflat_bf):
    """Balance chunks across cores by window size; build per-core maps with
    the packed per-slot layout (chunk k size = max over cores, desc-sorted)."""
    order = sorted(range(len(chunks)), key=lambda i: -chunks[i][1].shape[1])
    load = np.zeros(NCORES, np.int64)
    per_core = [[] for _ in range(NCORES)]
    for i in order:
        c = int(np.argmin(load))
        per_core[c].append(i)
        load[c] += chunks[i][1].shape[1]
    NCH = max(len(p) for p in per_core)
    sizes = np.zeros(NCH, np.int64)
    for p in per_core:
        for k, i in enumerate(p):
            sizes[k] = max(sizes[k], chunks[i][1].shape[1])
    sizes = (sizes + 15) // 16 * 16
    offs = np.concatenate([[0], np.cumsum(sizes)]).astype(int)
    S = int(offs[-1])
    maps, scatter = [], []
    for c in range(NCORES):
        wm = np.zeros((128, S), bf16)
        ft = np.zeros((128, NCH, CIMG), bf16)
        sc = []
        for k, i in enumerate(per_core[c]):
            pix_ids, Wd, vids = chunks[i]
            npix, nv = Wd.shape
            wm[0:npix, offs[k]:offs[k] + nv] = Wd
            ft[0:npix, k, :] = featflat_bf[pix_ids]
            sc.append((int(offs[k]), vids))
        maps.append(dict(wmat=wm, feats=ft))
        scatter.append(sc)
    return maps, scatter, tuple(int(s) for s in sizes)


def _prep_c_inputs(inputs, pooled_t):
    """pooled_t: [CIMG, 360, 360] f32 -> per-core slabs + masks + weights."""
    NR1 = C_OUT_ROWS + 2
    NRP = 2 * NR1 + 1
    w1 = np.asarray(inputs["ds1_w"], np.float32)
    w2 = np.asarray(inputs["ds2_w"], np.float32)
    wd1 = np.stack([w1[:, :, ky, kx].T for ky in range(3) for kx in range(3)])
    wd2 = np.stack([w2[:, :, ky, kx].T for ky in range(3) for kx in range(3)])
    sb1 = np.stack([np.asarray(inputs["ds1_s"], np.float32),
                    np.asarray(inputs["ds1_t"], np.float32)], 1)
    sb2 = np.stack([np.asarray(inputs["ds2_s"], np.float32),
                    np.asarray(inputs["ds2_t"], np.float32)], 1)
    shared = dict(wd1=wd1.astype(bf16), wd2=wd2.astype(bf16), sb1=sb1, sb2=sb2)
    maps = []
    pt_bf = pooled_t.astype(bf16)
    for c in range(NCORES):
        o0g = C_OUT_ROWS * c
        p0 = 2 * o0g - 3
        slab = np.zeros((CIMG, NRP, 362), bf16)
        lo, hi = max(0, p0), min(NX, p0 + NRP)
        if hi > lo:
            slab[:, lo - p0:hi - p0, 1:361] = pt_bf[:, lo:hi, :]
        t1g = np.arange(NR1) + (o0g - 1)
        m1 = np.broadcast_to(((t1g >= 0) & (t1g < 180))[None, :],
                             (128, NR1)).astype(bf16)
        maps.append(dict(shared, slab=slab, m1=np.ascontiguousarray(m1)))
    return maps


def kernel(**inputs):
    inputs = {k: np.asarray(v) for k, v in inputs.items()}
    flat, kept = _host_geometry(inputs["cam2lidar_rots"],
                                inputs["cam2lidar_trans"], inputs["intrins"],
                                inputs["post_rots"], inputs["post_trans"])
    depth_rows, feat_rows = run_launch_a(inputs)
    featflat_bf = feat_rows.astype(bf16)

    chunks = _build_chunks(flat, kept, depth_rows)
    bmaps, scatter, sizes = _prep_b_inputs(chunks, featflat_bf)
    key = ("B", sizes)
    if key not in _CACHE:
        _CACHE[key] = build_launch_b(sizes)
    res_b = run_bass_kernel_spmd(_CACHE[key], bmaps, list(range(NCORES)))

    allvox = np.concatenate([vids for c in range(NCORES)
                             for _, vids in scatter[c]])
    allval = np.concatenate(
        [res_b.results[c]["owin"][:, o0:o0 + len(vids)].T.astype(np.float32)
         for c in range(NCORES) for o0, vids in scatter[c]])
    o = np.argsort(allvox, kind="stable")
    allvox, allval = allvox[o], allval[o]
    starts = np.flatnonzero(np.r_[True, allvox[1:] != allvox[:-1]])
    pooled = np.zeros((NX * NX, CIMG), np.float32)
    pooled[allvox[starts]] = np.add.reduceat(allval, starts, axis=0)
    pooled_t = np.ascontiguousarray(
        pooled.reshape(NX, NX, CIMG).transpose(2, 0, 1))

    if "C" not in _CACHE:
        _CACHE["C"] = build_launch_c()
    cmaps = _prep_c_inputs(inputs, pooled_t)
    res_c = run_bass_kernel_spmd(_CACHE["C"], cmaps, list(range(NCORES)))
    out = np.zeros((1, CIMG, 180, 180), np.float32)
    for c in range(NCORES):
        o0g = C_OUT_ROWS * c
        nr = min(C_OUT_ROWS, 180 - o0g)
        if nr > 0:
            out[0, :, o0g:o0g + nr, :] = res_c.results[c]["yout"][:, 0:nr, :]
    return out



# revision 19
# speedup vs baseline: 1.7640x; 1.0431x over previous
"""DepthLSSTransform Trainium kernel: 3 SPMD launches over 8 NeuronCores.

Launch A: per-camera conv pipeline (dtransform + depthnet + softmax) on
          24-row bands (one 16-row + one 8-row segment per core).
Launch B: bev_pool segment-sum via one-hot matmuls over a host-built
          virtual-window schedule (sorted-by-voxel points).
Launch C: BEV downsample convs, spatially sharded.
Host: geometry/voxel indices, scheduling, gathers, folds (orchestration).
"""
import numpy as np
import ml_dtypes

import concourse.bass as bass
import concourse.tile as tile
from concourse import bacc, mybir
from concourse.bass_utils import run_bass_kernel_spmd

dt = mybir.dt
bf16 = ml_dtypes.bfloat16

# ---- problem constants (hardcoded per contract) ----
B, N = 1, 6
CIN, CIMG, DD = 256, 80, 59
FH, FW, IH, IW = 32, 88, 256, 704
XY0, DXY, NX = -54.0, 0.3, 360
Z0, DZ, NZ = -10.0, 20.0, 1
NPTS = N * DD * FH * FW
NPIX = N * FH * FW
NCORES = 8
QV = 4                      # chunks of 128 points per virtual window

# per-core segments: (camera, h0) for seg A (16 rows) and seg B (8 rows)
SEG_A = [(0, 0), (1, 0), (1, 16), (2, 16), (3, 0), (4, 0), (4, 16), (5, 16)]
SEG_B = [(0, 16), (0, 24), (2, 0), (2, 8), (3, 16), (3, 24), (5, 0), (5, 8)]
# band pixel ranges in global row order (row = n*32 + h)
ROWS_OF_CORE = [[(SEG_A[c][0] * FH + SEG_A[c][1] + r) for r in range(16)] +
                [(SEG_B[c][0] * FH + SEG_B[c][1] + r) for r in range(8)]
                for c in range(NCORES)]

# segment geometry: rows16 segment: d rows [8h0-34, 8h0+158) (192), dt2 out
# rows [2h0-8, 2h0+39) (47), dt3 [h0-3, h0+19) (22), dn1 [h0-1, h0+17) (18)
SEGS = [dict(nout=16, nd=192, nq=48, nt2=47, nt3=22, nn1=18),
        dict(nout=8, nd=128, nq=32, nt2=31, nt3=14, nn1=10)]


def _seg_ranges(h0, S):
    return dict(d0=8 * h0 - 34, q0=2 * h0 - 8, t0=h0 - 3, r0=h0 - 1, o0=h0)


# ---------------------------------------------------------------- launch A
def build_launch_a(debug=False, psum_bufs=3, work_bufs=3, stages=9):
    nc = bacc.Bacc("TRN2", target_bir_lowering=False, debug=False,
                   num_devices=NCORES)
    AP = {}

    def inp(name, shape, dtype=dt.bfloat16):
        AP[name] = nc.dram_tensor(name, shape, dtype, kind="ExternalInput").ap()
        return AP[name]

    # per segment inputs (s = 0: 16-row, 1: 8-row)
    for s, S in enumerate(SEGS):
        inp(f"dph{s}", [128, S["nq"], 177])
        inp(f"masks{s}", [128, S["nt2"] + S["nt3"] + S["nn1"]])
        inp(f"xseg{s}", [2, 128, S["nt3"], 92])         # x_img slice (padded)
    # packed f32 constants: [alpha, beta, s_dt2, t_dt2, s_dt3, t_dt3,
    #  s_dn1(2), t_dn1(2), s_dn2(2), t_dn2(2), b_dn3(139)] -> [128, 153]
    inp("consts", [128, 153], dt.float32)
    # conv weights (host-prepped layouts)
    inp("w_dt2", [4, 128, 32])                          # groups (dky,dmx)
    inp("w_dt3", [9, 128, 64])
    inp("w_dn1", [9, 3, 128, 256])                      # tap, icchunk(128,128,64pad) -> 256
    inp("w_dn2", [9, 2, 128, 256])
    inp("w_dn3", [2, 128, 139])

    DBG = {}
    dbg_specs = [] if not debug else [("dbg_t1", [128, SEGS[0]["nq"], 177], dt.bfloat16),
                        ("dbg_dt2o", [32, SEGS[0]["nt2"] + 1, 180], dt.bfloat16),
                        ("dbg_dtc", [64, SEGS[0]["nt3"], 92], dt.bfloat16),
                        ("dbg_n1o", [128, SEGS[0]["nn1"], 92], dt.bfloat16),
                        ("dbg_n2o", [128, SEGS[0]["nout"], 88], dt.bfloat16)]
    for nm, sh, dty in dbg_specs:
        DBG[nm] = nc.dram_tensor(nm, sh, dty, kind="ExternalOutput").ap()
    out_depth = nc.dram_tensor("out_depth", [24 * FW, DD], dt.float32,
                               kind="ExternalOutput").ap()
    out_feat = nc.dram_tensor("out_feat", [24 * FW, CIMG], dt.bfloat16,
                              kind="ExternalOutput").ap()

    # HBM scratch
    scr = {}
    for s, S in enumerate(SEGS):
        scr[f"dt2o{s}"] = nc.dram_tensor(f"dt2o{s}", [32, S["nt2"] + 1, 2, 90], dt.bfloat16).ap()

    RELU = mybir.ActivationFunctionType.Relu
    with tile.TileContext(nc) as tc:
        with tc.tile_pool(name="const", bufs=1) as cpool, \
             tc.tile_pool(name="work", bufs=work_bufs) as wpool, \
             tc.tile_pool(name="big", bufs=1) as bpool, \
             tc.tile_pool(name="psum", bufs=psum_bufs, space="PSUM") as ppool:
            # ---- DMA issue order = consumption order (the SP queue and the
            # modeled DMA engines serialize; early-stage inputs must land first)
            cts = cpool.tile([128, 153], dt.float32, name="cts")
            nc.sync.dma_start(out=cts[:], in_=AP["consts"])
            ct = {"dt1_alpha": cts[:, 0:1], "dt1_beta": cts[:, 1:2],
                  "s_dt2": cts[:, 2:3], "t_dt2": cts[:, 3:4],
                  "s_dt3": cts[:, 4:5], "t_dt3": cts[:, 5:6],
                  "s_dn1": cts[:, 6:8], "t_dn1": cts[:, 8:10],
                  "s_dn2": cts[:, 10:12], "t_dn2": cts[:, 12:14],
                  "b_dn3": cts[:, 14:153]}
            wt = {}

            def load_w(nm, pat):
                sh = list(AP[nm].shape)
                wt[nm] = cpool.tile([sh[-2], int(np.prod(sh[:-2])), sh[-1]],
                                    dt.bfloat16, tag=nm, name=f'wt_{nm}')
                nc.sync.dma_start(out=wt[nm][:], in_=AP[nm].rearrange(pat))

            load_w("w_dt2", "g p o -> p g o")
            dphs, malls = {}, {}
            for s, S in enumerate(SEGS):
                nq = S["nq"]
                dphs[s] = bpool.tile([128, nq, 177], dt.bfloat16, tag=f"dph{s}",
                                     name=f"dph{s}")
                for qq in range(0, nq, nq // 4):
                    nqq = min(nq // 4, nq - qq)
                    nc.sync.dma_start(out=dphs[s][:, qq:qq + nqq, :],
                                      in_=AP[f"dph{s}"][:, qq:qq + nqq, :])
                malls[s] = wpool.tile([128, S["nt2"] + S["nt3"] + S["nn1"]],
                                      dt.bfloat16, tag=f"msk{s}", name="mall")
                nc.sync.dma_start(out=malls[s][:], in_=AP[f"masks{s}"])
            load_w("w_dt3", "g p o -> p g o")
            load_w("w_dn1", "t i p o -> p (t i) o")
            load_w("w_dn2", "t i p o -> p (t i) o")
            load_w("w_dn3", "g p o -> p g o")

            feat_sb = {}
            depth_sb = {}
            for s, S in enumerate(SEGS):
                nq, nt2, nt3, nn1, nout = S["nq"], S["nt2"], S["nt3"], S["nn1"], S["nout"]
                # ======== dt1: relu(alpha*d + beta) on Act; host bakes pad
                # values into dph so relu zeroes out-of-image positions ====
                dph = dphs[s]
                t1 = bpool.tile([128, nq, 177], dt.bfloat16, tag=f"t1{s}")
                mall = malls[s]
                QCH = nq // 4
                for qq in range(0, nq, QCH):
                    nqq = min(QCH, nq - qq)
                    sl = (slice(None), slice(qq, qq + nqq), slice(None))
                    nc.scalar.activation(t1[sl], dph[sl], RELU,
                                         bias=ct["dt1_beta"][:, 0:1],
                                         scale=ct["dt1_alpha"][:, 0:1])
                if s == 0 and debug:
                    nc.sync.dma_start(out=DBG["dbg_t1"], in_=t1[:])

                if stages < 2:
                    continue
                # ======== dt2 ========
                o2 = bpool.tile([32, nt2 + 1, 180], dt.bfloat16, tag=f"o2{s}")
                # border strips only: cols 0, 89-90, 179 and the pad row nt2
                nc.vector.memset(o2[:, :, 0:1], 0.0)
                nc.vector.memset(o2[:, :, 89:91], 0.0)
                nc.vector.memset(o2[:, :, 179:180], 0.0)
                nc.vector.memset(o2[:, nt2:nt2 + 1, :], 0.0)
                m2 = bass.AP(mall.tensor, mall.offset, [mall.ap[0], [1, nt2]])
                RPP2 = 2
                for q0 in range(0, nt2, RPP2):
                    nr = min(RPP2, nt2 - q0)
                    ps = ppool.tile([32, nr, 176], dt.float32, tag=f"ps{s}", name="ps2")
                    gi = 0
                    for dky in range(2):
                        for dmx in range(2):
                            g = dky * 2 + dmx
                            rhs = bass.AP(
                                t1.tensor, t1.offset + (q0 + dky) * 177 + dmx,
                                [t1.ap[0], [177, nr], [1, 176]])
                            nc.tensor.matmul(ps[:], wt["w_dt2"][:, g, :], rhs,
                                             start=(gi == 0), stop=(gi == 3))
                            gi += 1
                    ev = wpool.tile([32, nr, 176], dt.bfloat16, tag=f"ev2{s}")
                    nc.scalar.activation(ev[:], ps[:], RELU,
                                         bias=ct["t_dt2"][0:32, 0:1],
                                         scale=ct["s_dt2"][0:32, 0:1])
                    mbb = bass.AP(m2.tensor, m2.offset + q0,
                                  [[m2.ap[0][0], 32], [1, nr], [0, 176]])
                    # write col c at (c%2)*90 + c//2 + 1  (phase-split layout)
                    o2dst = bass.AP(o2.tensor, o2.offset + q0 * 180 + 1,
                                    [[o2.ap[0][0], 32], [180, nr],
                                     [1, 88], [90, 2]])
                    nc.vector.tensor_tensor(out=o2dst, in0=ev[:], in1=mbb,
                                            op=mybir.AluOpType.mult)
                nc.sync.dma_start(out=scr[f"dt2o{s}"],
                                  in_=o2.rearrange("p q (b x) -> p q b x", b=2))
                if s == 0 and debug:
                    nc.sync.dma_start(out=DBG["dbg_dt2o"], in_=o2[:])

                if stages < 3:
                    continue
                # ======== dt3 ========
                nry3 = nt3 + 2
                ph3 = bpool.tile([128, nry3, 90], dt.bfloat16, tag=f"ph3{s}")
                sd2 = scr[f"dt2o{s}"]
                for a2 in range(2):
                    for b2 in range(2):
                        pap3 = bass.AP(sd2.tensor,
                                       sd2.offset + a2 * 180 + b2 * 90,
                                       [[(nt2 + 1) * 180, 32],
                                        [2 * 180, nry3], [1, 90]])
                        nc.sync.dma_start(
                            out=ph3[(a2 * 2 + b2) * 32:(a2 * 2 + b2 + 1) * 32],
                            in_=pap3)
                # concat input tile: [64 dt3 | pad] plus x_img tiles
                dtc = bpool.tile([64, nt3, 92], dt.bfloat16, tag=f"dtc{s}")
                nc.vector.memset(dtc[:, :, 0:2], 0.0)
                nc.vector.memset(dtc[:, :, 90:92], 0.0)
                m3 = bass.AP(mall.tensor, mall.offset + nt2,
                             [mall.ap[0], [1, nt3]])
                RPP3 = 4
                for t0 in range(0, nt3, RPP3):
                    nr = min(RPP3, nt3 - t0)
                    ps = ppool.tile([64, nr, 88], dt.float32, tag=f"ps{s}")
                    gi = 0
                    for dky in range(3):
                        for dmx in range(3):
                            g = dky * 3 + dmx
                            rhs = bass.AP(ph3.tensor,
                                          ph3.offset + (t0 + dky) * 90 + dmx,
                                          [ph3.ap[0], [90, nr], [1, 88]])
                            nc.tensor.matmul(ps[:], wt["w_dt3"][:, g, :], rhs,
                                             start=(gi == 0), stop=(gi == 8))
                            gi += 1
                    ev = wpool.tile([64, nr, 88], dt.bfloat16, tag=f"ev3{s}")
                    nc.scalar.activation(ev[:], ps[:], RELU,
                                         bias=ct["t_dt3"][0:64, 0:1],
                                         scale=ct["s_dt3"][0:64, 0:1])
                    mbb = bass.AP(m3.tensor, m3.offset + t0,
                                  [m3.ap[0], [1, nr], [0, 88]])
                    nc.vector.tensor_tensor(out=dtc[:, t0:t0 + nr, 2:90],
                                            in0=ev[:], in1=mbb[0:64],
                                            op=mybir.AluOpType.mult)

                if s == 0 and debug:
                    nc.sync.dma_start(out=DBG["dbg_dtc"], in_=dtc[:])
                if stages < 4:
                    continue
                # ======== dn1 ========
                xs = []
                for g in range(2):
                    xt = bpool.tile([128, nt3, 92], dt.bfloat16, tag=f"x{g}_{s}",
                                     name=f"xseg_t{g}")
                    nc.sync.dma_start(out=xt[:], in_=AP[f"xseg{s}"][g])
                    xs.append(xt)
                mn1 = bass.AP(mall.tensor, mall.offset + nt2 + nt3,
                              [mall.ap[0], [1, nn1]])
                n1o = []
                for g in range(2):
                    t = bpool.tile([128, nn1, 92], dt.bfloat16, tag=f"n1o{g}_{s}")
                    nc.vector.memset(t[:, :, 0:2], 0.0)
                    nc.vector.memset(t[:, :, 90:92], 0.0)
                    n1o.append(t)
                RPP = 5
                for ocg in range(2):
                    for r0 in range(0, nn1, RPP):
                        nr = min(RPP, nn1 - r0)
                        ps = ppool.tile([128, nr, 88], dt.float32, tag=f"ps{s}")
                        gi = 0
                        for ky in range(3):
                            for kx in range(3):
                                tap = ky * 3 + kx
                                for icc, srcT in enumerate((xs[0], xs[1], dtc)):
                                    kk = 128 if icc < 2 else 64
                                    rhs = bass.AP(
                                        srcT.tensor,
                                        srcT.offset + (r0 + ky + 1) * 92 + kx + 1,
                                        [srcT.ap[0], [92, nr], [1, 88]])
                                    lhs = wt["w_dn1"][0:kk, tap * 3 + icc,
                                                      ocg * 128:(ocg + 1) * 128]
                                    nc.tensor.matmul(ps[:], lhs, rhs,
                                                     start=(gi == 0),
                                                     stop=(gi == 26))
                                    gi += 1
                        ev = wpool.tile([128, nr, 88], dt.bfloat16, tag=f"evn1{s}")
                        nc.scalar.activation(ev[:], ps[:], RELU,
                                             bias=ct["t_dn1"][:, ocg:ocg + 1],
                                             scale=ct["s_dn1"][:, ocg:ocg + 1])
                        mbb = bass.AP(mn1.tensor, mn1.offset + r0,
                                      [mn1.ap[0], [1, nr], [0, 88]])
                        nc.vector.tensor_tensor(
                            out=n1o[ocg][:, r0:r0 + nr, 2:90],
                            in0=ev[:], in1=mbb, op=mybir.AluOpType.mult)

                if s == 0 and debug:
                    nc.sync.dma_start(out=DBG["dbg_n1o"], in_=n1o[0][:])
                if stages < 5:
                    continue
                # ======== dn2 ========
                n2o = []
                for g in range(2):
                    n2o.append(bpool.tile([128, nout, 88], dt.bfloat16,
                                          tag=f"n2o{g}_{s}", name=f"n2o{g}"))
                for ocg in range(2):
                    for r0 in range(0, nout, RPP):
                        nr = min(RPP, nout - r0)
                        ps = ppool.tile([128, nr, 88], dt.float32, tag=f"ps{s}")
                        gi = 0
                        for ky in range(3):
                            for kx in range(3):
                                tap = ky * 3 + kx
                                for icc in range(2):
                                    rhs = bass.AP(
                                        n1o[icc].tensor,
                                        n1o[icc].offset + (r0 + ky) * 92 + kx + 1,
                                        [n1o[icc].ap[0], [92, nr], [1, 88]])
                                    lhs = wt["w_dn2"][:, tap * 2 + icc,
                                                      ocg * 128:(ocg + 1) * 128]
                                    nc.tensor.matmul(ps[:], lhs, rhs,
                                                     start=(gi == 0),
                                                     stop=(gi == 17))
                                    gi += 1
                        ev = wpool.tile([128, nr, 88], dt.bfloat16, tag=f"evn2{s}")
                        nc.scalar.activation(ev[:], ps[:], RELU,
                                             bias=ct["t_dn2"][:, ocg:ocg + 1],
                                             scale=ct["s_dn2"][:, ocg:ocg + 1])
                        nc.vector.tensor_copy(n2o[ocg][:, r0:r0 + nr, :], ev[:])

                if s == 0 and debug:
                    nc.sync.dma_start(out=DBG["dbg_n2o"], in_=n2o[0][:])
                if stages < 6:
                    continue
                # ======== dn3 + softmax + feat ========
                npix = nout * FW
                feat_sb[s] = bpool.tile([128, (npix + 127) // 128, CIMG],
                                        dt.bfloat16, tag=f"feat{s}", name=f"feat_sb{s}")
                depth_sb[s] = bpool.tile([128, (npix + 127) // 128, DD],
                                         dt.float32, tag=f"depth{s}", name=f"depth_sb{s}")
                n2f = [t.rearrange("p a b -> p (a b)") for t in n2o]
                for pc in range((npix + 127) // 128):
                    m = min(128, npix - pc * 128)
                    ps = ppool.tile([m, 139], dt.float32, tag=f"ps{s}")
                    for icc in range(2):
                        nc.tensor.matmul(ps[:], n2f[icc][:, pc * 128:pc * 128 + m],
                                         wt["w_dn3"][:, icc, :],
                                         start=(icc == 0), stop=(icc == 1))
                    # add bias via vector then softmax over first 59
                    lg = wpool.tile([m, 139], dt.float32, tag=f"lg{s}")
                    nc.vector.tensor_tensor(out=lg[:], in0=ps[:],
                                            in1=ct["b_dn3"][0:m],
                                            op=mybir.AluOpType.add)
                    mx = wpool.tile([m, 1], dt.float32, tag=f"mx{s}")
                    nc.vector.reduce_max(mx[:], lg[:, 0:DD],
                                         axis=mybir.AxisListType.X, negate=True)
                    ex = wpool.tile([m, DD], dt.float32, tag=f"ex{s}")
                    nc.scalar.activation(ex[:], lg[:, 0:DD],
                                         mybir.ActivationFunctionType.Exp,
                                         bias=mx[:, 0:1], scale=1.0)
                    sm = wpool.tile([m, 1], dt.float32, tag=f"sm{s}")
                    nc.vector.reduce_sum(sm[:], ex[:], axis=mybir.AxisListType.X)
                    rc = wpool.tile([m, 1], dt.float32, tag=f"rc{s}")
                    nc.vector.reciprocal(rc[:], sm[:])
                    nc.vector.tensor_scalar(out=depth_sb[s][0:m, pc, :], in0=ex[:],
                                            scalar1=rc[:, 0:1], scalar2=None,
                                            op0=mybir.AluOpType.mult)
                    nc.vector.tensor_copy(feat_sb[s][0:m, pc, :],
                                          lg[:, DD:DD + CIMG])

                # DMA this segment's outputs now (overlaps the next segment)
                base = 0 if s == 0 else 16 * FW
                nfull = npix // 128
                dsl = out_depth[base:base + nfull * 128].rearrange(
                    "(a p) d -> p a d", p=128)
                nc.sync.dma_start(out=dsl, in_=depth_sb[s][:, 0:nfull, :])
                fsl = out_feat[base:base + nfull * 128].rearrange(
                    "(a p) d -> p a d", p=128)
                nc.sync.dma_start(out=fsl, in_=feat_sb[s][:, 0:nfull, :])
                rem = npix - nfull * 128
                if rem:
                    nc.sync.dma_start(
                        out=out_depth[base + nfull * 128:base + npix],
                        in_=depth_sb[s][0:rem, nfull, :])
                    nc.sync.dma_start(
                        out=out_feat[base + nfull * 128:base + npix],
                        in_=feat_sb[s][0:rem, nfull, :])
    nc.compile()
    return nc


# ------------------------------------------------------------ host helpers
def _host_geometry(rots, trans, intr, post_rots, post_trans):
    import jax
    import jax.numpy as jnp
    with jax.default_device(jax.devices("cpu")[0]):
        f32 = jnp.float32
        ds = jnp.arange(1.0, 60.0, 1.0, dtype=f32)
        xs = jnp.linspace(0.0, IW - 1.0, FW, dtype=f32)
        ys = jnp.linspace(0.0, IH - 1.0, FH, dtype=f32)
        dm = jnp.broadcast_to(ds[:, None, None], (DD, FH, FW))
        xm = jnp.broadcast_to(xs[None, None, :], (DD, FH, FW))
        ym = jnp.broadcast_to(ys[None, :, None], (DD, FH, FW))
        fr = jnp.stack([xm, ym, dm], -1)
        pts = fr[None, None] - jnp.asarray(post_trans)[:, :, None, None, None, :]
        pts = jnp.einsum("bnij,bndhwj->bndhwi",
                         jnp.linalg.inv(jnp.asarray(post_rots)), pts)
        pts = jnp.concatenate([pts[..., :2] * pts[..., 2:3], pts[..., 2:3]], -1)
        comb = jnp.einsum("bnij,bnjk->bnik", jnp.asarray(rots),
                          jnp.linalg.inv(jnp.asarray(intr)))
        pts = jnp.einsum("bnij,bndhwj->bndhwi", comb, pts) \
            + jnp.asarray(trans)[:, :, None, None, None, :]
        lo = jnp.array([XY0, XY0, Z0], dtype=f32)
        dxv = jnp.array([DXY, DXY, DZ], dtype=f32)
        g = ((pts - lo) / dxv).astype(jnp.int32).reshape(-1, 3)
        kept = ((g[:, 0] >= 0) & (g[:, 0] < NX) & (g[:, 1] >= 0) & (g[:, 1] < NX)
                & (g[:, 2] >= 0) & (g[:, 2] < NZ))
        flat = (g[:, 2] * NX + g[:, 0]) * NX + g[:, 1]
        return np.asarray(flat, np.int64), np.asarray(kept)


def _prep_a_inputs(inputs):
    """Build per-core input maps for launch A."""
    d = np.asarray(inputs["d"], np.float32).reshape(N, IH, IW)
    x_img = np.asarray(inputs["x_img"], np.float32)

    # dt1 folded affine: relu(alpha*d + beta), alpha = s*w, beta = s*b + t
    a1 = (inputs["dt1_s"] * inputs["dt1_w"][:, 0, 0, 0]).astype(np.float32)
    b1 = (inputs["dt1_s"] * inputs["dt1_b"] + inputs["dt1_t"]).astype(np.float32)
    cab = np.arange(128)
    dt1_alpha = a1[cab // 16][:, None]
    dt1_beta = b1[cab // 16][:, None]

    def wprep_dt2():
        w = np.asarray(inputs["dt2_w"], np.float32)      # [32,8,5,5]
        out = np.zeros((4, 128, 32), np.float32)
        for ky in range(5):
            for kx in range(5):
                a, dky = ky % 4, ky // 4
                bph, dmx = (kx + 2) % 4, (kx + 2) // 4
                g = dky * 2 + dmx
                rows = (np.arange(8)) * 16 + a * 4 + bph
                out[g, rows, :] = w[:, :, ky, kx].T
        return out.astype(bf16)

    def wprep_dt3():
        w = np.asarray(inputs["dt3_w"], np.float32)      # [64,32,5,5]
        out = np.zeros((9, 128, 64), np.float32)
        for ky in range(5):
            for kx in range(5):
                a, dky = ky % 2, ky // 2
                bph, dmx = kx % 2, (kx + 2) // 2 - 1
                g = dky * 3 + dmx
                rows = (a * 2 + bph) * 32 + np.arange(32)
                out[g, rows, :] = w[:, :, ky, kx].T
        return out.astype(bf16)

    def wprep_3x3(w, icc_sizes):
        O, I = w.shape[0], w.shape[1]
        nic = len(icc_sizes)
        out = np.zeros((9, nic, 128, O), np.float32)
        for ky in range(3):
            for kx in range(3):
                tap = ky * 3 + kx
                ic0 = 0
                for icc, sz in enumerate(icc_sizes):
                    out[tap, icc, 0:sz, :] = w[:, ic0:ic0 + sz, ky, kx].T
                    ic0 += sz
        return out.astype(bf16)

    # NOTE: dn1 input concat order is [dt3(64) | x_img(256)] in the reference;
    # our matmul chunks are (x0:128, x1:128, dt3:64) -> weight cols must match:
    w_dn1_full = np.asarray(inputs["dn1_w"], np.float32)
    w_dn1 = np.zeros((9, 3, 128, 256), np.float32)
    for ky in range(3):
        for kx in range(3):
            tap = ky * 3 + kx
            w_dn1[tap, 0, :, :] = w_dn1_full[:, 64:192, ky, kx].T
            w_dn1[tap, 1, :, :] = w_dn1_full[:, 192:320, ky, kx].T
            w_dn1[tap, 2, 0:64, :] = w_dn1_full[:, 0:64, ky, kx].T
    w_dn1 = w_dn1.astype(bf16)
    w_dn2 = wprep_3x3(np.asarray(inputs["dn2_w"], np.float32), [128, 128])
    w_dn3 = np.asarray(inputs["dn3_w"], np.float32)[:, :, 0, 0]  # [139, 256]
    w_dn3p = np.zeros((2, 128, 139), np.float32)
    w_dn3p[0] = w_dn3[:, 0:128].T
    w_dn3p[1] = w_dn3[:, 128:256].T

    def fold_bias(b, s, t):
        # conv bias b then bn scale/shift: relu(s*(x+b) + t) = relu(s*x + (s*b+t))
        return np.asarray(s, np.float32), np.asarray(s * b + t, np.float32)

    s2, t2 = fold_bias(inputs["dt2_b"], inputs["dt2_s"], inputs["dt2_t"])
    s3, t3 = fold_bias(inputs["dt3_b"], inputs["dt3_s"], inputs["dt3_t"])
    sn1, tn1 = fold_bias(inputs["dn1_b"], inputs["dn1_s"], inputs["dn1_t"])
    sn2, tn2 = fold_bias(inputs["dn2_b"], inputs["dn2_s"], inputs["dn2_t"])
    b_dn3 = np.broadcast_to(np.asarray(inputs["dn3_b"], np.float32)[None, :],
                            (128, 139)).copy()

    consts = np.zeros((128, 153), np.float32)
    consts[:, 0] = dt1_alpha[:, 0]
    consts[:, 1] = dt1_beta[:, 0]
    consts[:, 2] = np.tile(s2, 4)
    consts[:, 3] = np.tile(t2, 4)
    consts[:, 4] = np.tile(s3, 2)
    consts[:, 5] = np.tile(t3, 2)
    consts[:, 6:8] = sn1.reshape(2, 128).T
    consts[:, 8:10] = tn1.reshape(2, 128).T
    consts[:, 10:12] = sn2.reshape(2, 128).T
    consts[:, 12:14] = tn2.reshape(2, 128).T
    consts[:, 14:153] = b_dn3
    shared = dict(
        consts=consts,
        w_dt2=wprep_dt2(), w_dt3=wprep_dt3(), w_dn1=w_dn1, w_dn2=w_dn2,
        w_dn3=w_dn3p.astype(bf16),
    )

    # per-channel pad value: alpha*v + beta <= -|alpha|*1e8 < 0 -> relu -> 0
    assert np.abs(a1).min() > 1e-5, "dt1 alpha too small for pad-value trick"
    vpad = (-np.sign(a1) * 1e8).astype(np.float32)           # [8] per channel

    maps = []
    for c in range(NCORES):
        m = dict(shared)
        for s, (cam, h0) in enumerate([SEG_A[c], SEG_B[c]]):
            S = SEGS[s]
            d0 = 8 * h0 - 34
            dseg = np.zeros((S["nd"], 712), np.float32)
            vseg = np.zeros((S["nd"], 712), bool)
            lo, hi = max(0, d0), min(IH, d0 + S["nd"])
            if hi > lo:
                dseg[lo - d0:hi - d0, 4:708] = d[cam, lo:hi]
                vseg[lo - d0:hi - d0, 4:708] = True
            nq = S["nq"]
            ph = dseg.reshape(nq, 4, 178, 4)[:, :, :177, :]     # ry a rx b
            ph = ph.transpose(1, 3, 0, 2)                        # a b ry rx
            vph = vseg.reshape(nq, 4, 178, 4)[:, :, :177, :].transpose(1, 3, 0, 2)
            dphc = np.where(vph[None], ph[None],
                            vpad[:, None, None, None, None])    # [8,4,4,nq,177]
            m[f"dph{s}"] = dphc.reshape(128, nq, 177).astype(bf16)
            q0, t0, r0 = 2 * h0 - 8, h0 - 3, h0 - 1
            qr = np.arange(S["nt2"]) + q0
            m2m = np.broadcast_to(((qr >= 0) & (qr < 64))[None, :],
                                  (128, S["nt2"]))
            tr = np.arange(S["nt3"]) + t0
            m3m = np.broadcast_to(((tr >= 0) & (tr < FH))[None, :],
                                  (128, S["nt3"]))
            rr = np.arange(S["nn1"]) + r0
            mn1m = np.broadcast_to(((rr >= 0) & (rr < FH))[None, :],
                                   (128, S["nn1"]))
            m[f"masks{s}"] = np.concatenate(
                [m2m, m3m, mn1m], axis=1).astype(bf16)
            xseg = np.zeros((2, 128, S["nt3"], 92), np.float32)
            lo2, hi2 = max(0, t0), min(FH, t0 + S["nt3"])
            if hi2 > lo2:
                xseg[:, :, lo2 - t0:hi2 - t0, 2:90] = \
                    x_img[cam, :, lo2:hi2, :].reshape(2, 128, hi2 - lo2, FW)
            m[f"xseg{s}"] = xseg.astype(bf16)
        maps.append(m)
    return maps


# ---------------------------------------------------------------- launch B
def build_launch_b(sizes):
    """Per chunk k: [128pix x 80ch] stationary feat tile x host-built
    [128pix x sizes[k] voxel-slot] depth-weight matrix -> [80, nv] window
    sums. W and out use packed (variable-size) layouts; W loads in a few
    batched DMAs, out in one."""
    nc = bacc.Bacc("TRN2", target_bir_lowering=False, debug=False,
                   num_devices=NCORES)
    NCH = len(sizes)
    offs = np.concatenate([[0], np.cumsum(sizes)]).astype(int)
    S = int(offs[-1])
    wmat = nc.dram_tensor("wmat", [128, S], dt.bfloat16,
                          kind="ExternalInput").ap()
    feats = nc.dram_tensor("feats", [128, NCH, CIMG], dt.bfloat16,
                           kind="ExternalInput").ap()
    owin = nc.dram_tensor("owin", [CIMG, S], dt.bfloat16,
                          kind="ExternalOutput").ap()
    NB = 4                                   # W DMA batches
    bnd = [int(round(NCH * i / NB)) for i in range(NB + 1)]
    with tile.TileContext(nc) as tc:
        with tc.tile_pool(name="const", bufs=1) as cpool, \
             tc.tile_pool(name="ps", bufs=4, space="PSUM") as pp:
            ft = cpool.tile([128, NCH, CIMG], dt.bfloat16, name="ft")
            nc.sync.dma_start(out=ft[:], in_=feats)
            wt = cpool.tile([128, S], dt.bfloat16, name="wt")
            for b in range(NB):
                lo, hi = offs[bnd[b]], offs[bnd[b + 1]]
                if hi > lo:
                    nc.sync.dma_start(out=wt[:, lo:hi], in_=wmat[:, lo:hi])
            ot = cpool.tile([CIMG, S], dt.bfloat16, name="ot")
            for k in range(NCH):
                nv, o0 = int(sizes[k]), int(offs[k])
                ps = pp.tile([CIMG, 512], dt.float32, tag="ps", name="ps")
                nc.tensor.matmul(ps[:, 0:nv], ft[:, k, :], wt[:, o0:o0 + nv],
                                 start=True, stop=True)
                if k % 2 == 0:
                    nc.scalar.activation(ot[:, o0:o0 + nv], ps[:, 0:nv],
                                         mybir.ActivationFunctionType.Copy)
                else:
                    nc.vector.tensor_copy(ot[:, o0:o0 + nv], ps[:, 0:nv])
            nc.sync.dma_start(out=owin, in_=ot[:])
    nc.compile()
    return nc


# ---------------------------------------------------------------- launch C
C_OUT_ROWS = 23              # ds2-out rows per core (8*23 = 184 >= 180)


def build_launch_c():
    nc = bacc.Bacc("TRN2", target_bir_lowering=False, debug=False,
                   num_devices=NCORES)
    NR1 = C_OUT_ROWS + 2                         # ds1-out rows incl halo (25)
    NRP = 2 * NR1 + 1                            # pooled rows needed (51)
    slab = nc.dram_tensor("slab", [CIMG, NRP, 362], dt.bfloat16,
                          kind="ExternalInput").ap()
    m1 = nc.dram_tensor("m1", [128, NR1], dt.bfloat16, kind="ExternalInput").ap()
    wd1 = nc.dram_tensor("wd1", [9, CIMG, CIMG], dt.bfloat16,
                         kind="ExternalInput").ap()
    wd2 = nc.dram_tensor("wd2", [9, CIMG, CIMG], dt.bfloat16,
                         kind="ExternalInput").ap()
    sb1 = nc.dram_tensor("sb1", [CIMG, 2], dt.float32, kind="ExternalInput").ap()
    sb2 = nc.dram_tensor("sb2", [CIMG, 2], dt.float32, kind="ExternalInput").ap()
    yout = nc.dram_tensor("yout", [CIMG, C_OUT_ROWS, 180], dt.float32,
                          kind="ExternalOutput").ap()
    RELU = mybir.ActivationFunctionType.Relu
    with tile.TileContext(nc) as tc:
        with tc.tile_pool(name="const", bufs=1) as cpool,              tc.tile_pool(name="work", bufs=2) as wp,              tc.tile_pool(name="big", bufs=1) as bp,              tc.tile_pool(name="ps", bufs=3, space="PSUM") as pp:
            # weights/consts first so ds1 can start on the first slab chunk
            w1 = cpool.tile([CIMG, 9, CIMG], dt.bfloat16, name="w1")
            nc.sync.dma_start(out=w1[:], in_=wd1.rearrange("t p o -> p t o"))
            sb1t = cpool.tile([CIMG, 2], dt.float32, name="sb1t")
            nc.sync.dma_start(out=sb1t[:], in_=sb1)
            m1t = wp.tile([128, NR1], dt.bfloat16, name="m1t")
            nc.sync.dma_start(out=m1t[:], in_=m1)
            slabt = bp.tile([CIMG, NRP, 362], dt.bfloat16, name="slabt")
            for rr in range(0, NRP, 9):
                nrr = min(9, NRP - rr)
                nc.sync.dma_start(out=slabt[:, rr:rr + nrr, :],
                                  in_=slab[:, rr:rr + nrr, :])
            w2 = cpool.tile([CIMG, 9, CIMG], dt.bfloat16, name="w2")
            nc.sync.dma_start(out=w2[:], in_=wd2.rearrange("t p o -> p t o"))
            sb2t = cpool.tile([CIMG, 2], dt.float32, name="sb2t")
            nc.sync.dma_start(out=sb2t[:], in_=sb2)
            h1 = bp.tile([CIMG, NR1, 182], dt.bfloat16, name="h1")
            nc.vector.memset(h1[:, :, 0:1], 0.0)
            nc.vector.memset(h1[:, :, 181:182], 0.0)
            # ds1: stride-2 3x3; out row t reads slab rows 2t..2t+2 (slab row 0
            # = pooled row 2o0-3, so out row t (global o0-1+t) reads
            # 2(o0-1+t)-1..+1 - (2o0-3) = 2t..2t+2); col c reads 2c..2c+2
            RP = 2
            for t0 in range(0, NR1, RP):
                nr = min(RP, NR1 - t0)
                ps = pp.tile([CIMG, nr, 180], dt.float32, tag="ps1", name="ps")
                gi = 0
                for ky in range(3):
                    for kx in range(3):
                        rhs = bass.AP(slabt.tensor,
                                      slabt.offset + (2 * t0 + ky) * 362 + kx,
                                      [slabt.ap[0], [2 * 362, nr], [2, 180]])
                        nc.tensor.matmul(ps[:], w1[:, ky * 3 + kx, :], rhs,
                                         start=(gi == 0), stop=(gi == 8))
                        gi += 1
                ev = wp.tile([CIMG, nr, 180], dt.bfloat16, tag="ev", name="ev")
                nc.scalar.activation(ev[:], ps[:], RELU, bias=sb1t[:, 1:2],
                                     scale=sb1t[:, 0:1])
                mbb = bass.AP(m1t.tensor, m1t.offset + t0,
                              [[m1t.ap[0][0], CIMG], [1, nr], [0, 180]])
                nc.vector.tensor_tensor(out=h1[:, t0:t0 + nr, 1:181],
                                        in0=ev[:], in1=mbb,
                                        op=mybir.AluOpType.mult)
            # ds2: 3x3 pad 1: out row o reads h1 rows o..o+2, col c: c..c+2
            yo = bp.tile([CIMG, C_OUT_ROWS, 180], dt.float32, name="yo")
            for o0 in range(0, C_OUT_ROWS, RP):
                nr = min(RP, C_OUT_ROWS - o0)
                ps = pp.tile([CIMG, nr, 180], dt.float32, tag="ps2", name="ps")
                gi = 0
                for ky in range(3):
                    for kx in range(3):
                        rhs = bass.AP(h1.tensor,
                                      h1.offset + (o0 + ky) * 182 + kx,
                                      [h1.ap[0], [182, nr], [1, 180]])
                        nc.tensor.matmul(ps[:], w2[:, ky * 3 + kx, :], rhs,
                                         start=(gi == 0), stop=(gi == 8))
                        gi += 1
                nc.scalar.activation(yo[:, o0:o0 + nr, :], ps[:], RELU,
                                     bias=sb2t[:, 1:2], scale=sb2t[:, 0:1])
                if (o0 // RP) % 3 == 2 or o0 + nr >= C_OUT_ROWS:
                    lo = (o0 // (3 * RP)) * 3 * RP
                    nc.sync.dma_start(out=yout[:, lo:o0 + nr, :],
                                      in_=yo[:, lo:o0 + nr, :])
    nc.compile()
    return nc


_CACHE = {}


def run_launch_a(inputs):
    if "A" not in _CACHE:
        _CACHE["A"] = build_launch_a()
    nc = _CACHE["A"]
    maps = _prep_a_inputs(inputs)
    res = run_bass_kernel_spmd(nc, maps, list(range(NCORES)))
    depth = np.zeros((NPIX, DD), np.float32)
    feat = np.zeros((NPIX, CIMG), np.float32)
    for c in range(NCORES):
        r = res.results[c]
        for s, (cam, h0) in enumerate([SEG_A[c], SEG_B[c]]):
            S = SEGS[s]
            npix = S["nout"] * FW
            base = (cam * FH + h0) * FW
            off = 0 if s == 0 else 16 * FW
            depth[base:base + npix] = r["out_depth"][off:off + npix]
            feat[base:base + npix] = r["out_feat"][off:off + npix].astype(np.float32)
    return depth, feat


def _build_chunks(flat, kept, depth_rows):
    """Group points by (camera, column-block); per group build the
    [pix, voxel-slot] depth-weight matrix over the group's voxel union.
    Splits column blocks whose union exceeds the PSUM window (512)."""
    fl = flat.reshape(N, DD, FH, FW)
    kp = kept.reshape(N, DD, FH, FW)
    chunks = []                      # (pix_ids, Wdense[npix, nv], vox_ids)

    def add_group(n, w0, w1):
        nw = w1 - w0
        f = fl[n, :, :, w0:w1]                       # [DD, FH, nw]
        k = kp[n, :, :, w0:w1]
        vids = np.unique(f[k])
        if len(vids) > 512 and nw > 1:
            mid = w0 + nw // 2
            add_group(n, w0, mid)
            add_group(n, mid, w1)
            return
        nv = max(len(vids), 1)
        # pixel local idx = (w - w0) * FH + h; point (d, h, w)
        slot = np.searchsorted(vids, f[k]) if len(vids) else np.zeros(0, np.int64)
        dd, hh, ww = np.nonzero(k)
        pix_loc = ww * FH + hh
        pixcol = n * FH * FW + hh * FW + (ww + w0)
        dep = depth_rows[pixcol, dd]
        Wd = np.bincount(pix_loc * nv + slot, weights=dep,
                         minlength=nw * FH * nv).reshape(nw * FH, nv)
        pix_ids = (n * FH * FW + np.arange(FH)[None, :] * FW
                   + (w0 + np.arange(nw))[:, None]).reshape(-1)
        chunks.append((pix_ids, Wd, vids))

    for n in range(N):
        for w0 in range(0, FW, 4):
            add_group(n, w0, w0 + 4)
    return chunks


def _prep_b_inputs(chunks, featflat_bf):
    """Balance chunks across cores by window size; build per-core maps with
    the packed per-slot layout (chunk k size = max over cores, desc-sorted)."""
    order = sorted(range(len(chunks)), key=lambda i: -chunks[i][1].shape[1])
    load = np.zeros(NCORES, np.int64)
    per_core = [[] for _ in range(NCORES)]
    for i in order:
        c = int(np.argmin(load))
        per_core[c].append(i)
        load[c] += chunks[i][1].shape[1]
    NCH = max(len(p) for p in per_core)
    sizes = np.zeros(NCH, np.int64)
    for p in per_core:
        for k, i in enumerate(p):
            sizes[k] = max(sizes[k], chunks[i][1].shape[1])
    sizes = (sizes + 15) // 16 * 16
    offs = np.concatenate([[0], np.cumsum(sizes)]).astype(int)
    S = int(offs[-1])
    maps, scatter = [], []
    for c in range(NCORES):
        wm = np.zeros((128, S), bf16)
        ft = np.zeros((128, NCH, CIMG), bf16)
        sc = []
        for k, i in enumerate(per_core[c]):
            pix_ids, Wd, vids = chunks[i]
            npix, nv = Wd.shape
            wm[0:npix, offs[k]:offs[k] + nv] = Wd
            ft[0:npix, k, :] = featflat_bf[pix_ids]
            sc.append((int(offs[k]), vids))
        maps.append(dict(wmat=wm, feats=ft))
        scatter.append(sc)
    return maps, scatter, tuple(int(s) for s in sizes)


def _prep_c_inputs(inputs, pooled_t):
    """pooled_t: [CIMG, 360, 360] f32 -> per-core slabs + masks + weights."""
    NR1 = C_OUT_ROWS + 2
    NRP = 2 * NR1 + 1
    w1 = np.asarray(inputs["ds1_w"], np.float32)
    w2 = np.asarray(inputs["ds2_w"], np.float32)
    wd1 = np.stack([w1[:, :, ky, kx].T for ky in range(3) for kx in range(3)])
    wd2 = np.stack([w2[:, :, ky, kx].T for ky in range(3) for kx in range(3)])
    sb1 = np.stack([np.asarray(inputs["ds1_s"], np.float32),
                    np.asarray(inputs["ds1_t"], np.float32)], 1)
    sb2 = np.stack([np.asarray(inputs["ds2_s"], np.float32),
                    np.asarray(inputs["ds2_t"], np.float32)], 1)
    shared = dict(wd1=wd1.astype(bf16), wd2=wd2.astype(bf16), sb1=sb1, sb2=sb2)
    maps = []
    pt_bf = pooled_t.astype(bf16)
    for c in range(NCORES):
        o0g = C_OUT_ROWS * c
        p0 = 2 * o0g - 3
        slab = np.zeros((CIMG, NRP, 362), bf16)
        lo, hi = max(0, p0), min(NX, p0 + NRP)
        if hi > lo:
            slab[:, lo - p0:hi - p0, 1:361] = pt_bf[:, lo:hi, :]
        t1g = np.arange(NR1) + (o0g - 1)
        m1 = np.broadcast_to(((t1g >= 0) & (t1g < 180))[None, :],
                             (128, NR1)).astype(bf16)
        maps.append(dict(shared, slab=slab, m1=np.ascontiguousarray(m1)))
    return maps


def kernel(**inputs):
    inputs = {k: np.asarray(v) for k, v in inputs.items()}
    flat, kept = _host_geometry(inputs["cam2lidar_rots"],
                                inputs["cam2lidar_trans"], inputs["intrins"],
                                inputs["post_rots"], inputs["post_trans"])
    depth_rows, feat_rows = run_launch_a(inputs)
    featflat_bf = feat_rows.astype(bf16)

    chunks = _build_chunks(flat, kept, depth_rows)
    bmaps, scatter, sizes = _prep_b_inputs(chunks, featflat_bf)
    key = ("B", sizes)
    if key not in _CACHE:
        _CACHE[key] = build_launch_b(sizes)
    res_b = run_bass_kernel_spmd(_CACHE[key], bmaps, list(range(NCORES)))

    allvox = np.concatenate([vids for c in range(NCORES)
                             for _, vids in scatter[c]])
    allval = np.concatenate(
        [res_b.results[c]["owin"][:, o0:o0 + len(vids)].T.astype(np.float32)
         for c in range(NCORES) for o0, vids in scatter[c]])
    o = np.argsort(allvox, kind="stable")
    allvox, allval = allvox[o], allval[o]
    starts = np.flatnonzero(np.r_[True, allvox[1:] != allvox[:-1]])
    pooled = np.zeros((NX * NX, CIMG), np.float32)
    pooled[allvox[starts]] = np.add.reduceat(allval, starts, axis=0)
    pooled_t = np.ascontiguousarray(
        pooled.reshape(NX, NX, CIMG).transpose(2, 0, 1))

    if "C" not in _CACHE:
        _CACHE["C"] = build_launch_c()
    cmaps = _prep_c_inputs(inputs, pooled_t)
    res_c = run_bass_kernel_spmd(_CACHE["C"], cmaps, list(range(NCORES)))
    out = np.zeros((1, CIMG, 180, 180), np.float32)
    for c in range(NCORES):
        o0g = C_OUT_ROWS * c
        nr = min(C_OUT_ROWS, 180 - o0g)
        if nr > 0:
            out[0, :, o0g:o0g + nr, :] = res_c.results[c]["yout"][:, 0:nr, :]
    return out

